# revision 6
# baseline (speedup 1.0000x reference)
"""Self-contained GAT kernel for 8 TRN2 NeuronCores.

kernel(**inputs) takes the FULL unsharded inputs (as produced by
setup_inputs) and returns the FULL [100000, 64] float32 output.

Architecture (see module gat_kernel-style doc):
- nodes dst-partitioned across 8 cores; edges dst-sorted into 128-dst windows,
  128-edge tiles.
- per-node table rows [h bf16 x64 | a_src f32 x8] packed as uint16[80];
  per-tile [128,1]-offset indirect-DMA gather.
- segment softmax/sums via one-hot selection matrices + PE matmuls; a_dst
  expanded per edge via DMA-transposed one-hot (S^T) matmuls from SBUF
  tables (bf16 hi+lo split for f32 accuracy).
- layer 2 aggregates 64-dim h2 per head and applies W2 after aggregation;
  head-mean via PSUM-accumulated per-head matmuls.
- AllGather collectives replicate node tables between phases.
"""
import os
import sys
import types

import numpy as np

sys.path.insert(0, "/opt/trn_rl_repo")

import ml_dtypes

import concourse.bass as bass
import concourse.bacc as bacc
import concourse.mybir as mybir
import concourse.tile as tile

BF16 = mybir.dt.bfloat16
F32 = mybir.dt.float32
I32 = mybir.dt.int32
U16 = mybir.dt.uint16

P = 128
H = 8
F1 = 8
F2 = 64
D1 = H * F1
IN_DIM = 256
NEG = 0.2
GHOST_AS = -300.0
TCOL = 80
NC = 8
N = 100000

LAST_EXEC_NS = None

_hook_registered = [False]


def _register_profile_hook():
    if _hook_registered[0]:
        return
    try:
        import antenv
        mod = types.ModuleType("antenv.axon_hooks")
        _h = [None]
        mod.set_axon_ntff_profile_hook = lambda f: _h.__setitem__(0, f)
        mod.get_axon_ntff_profile_hook = lambda: _h[0]
        sys.modules.setdefault("antenv.axon_hooks", mod)
        if not hasattr(antenv, "axon_hooks"):
            antenv.axon_hooks = mod
        from trn_agent_boot.trn_boot import _ntff_profile_via_ctypes
        sys.modules["antenv.axon_hooks"].set_axon_ntff_profile_hook(
            _ntff_profile_via_ctypes('/opt/axon/libaxon_pjrt.so'))
        _hook_registered[0] = True
    except Exception:
        pass


def mid_bcast(ap2d, reps):
    return bass.AP(ap2d.tensor, ap2d.offset, [ap2d.ap[0], [0, reps], ap2d.ap[1]])


def host_prep(inputs):
    SLICE = N // NC
    NW = (SLICE + P - 1) // P
    SPAD = NW * P
    GHOST = NC * SPAD

    edge = np.asarray(inputs["edge"])
    src = np.concatenate([np.asarray(edge[0]), np.arange(N, dtype=np.int64)])
    dst = np.concatenate([np.asarray(edge[1]), np.arange(N, dtype=np.int64)])

    core = (dst // SLICE).astype(np.int32)
    srcpad = ((src // SLICE) * SPAD + (src % SLICE)).astype(np.int32)
    dstl = (dst % SLICE).astype(np.int32)
    win = dstl // P

    counts = np.zeros((NC, NW), np.int64)
    for c in range(NC):
        m = core == c
        w, cnt = np.unique(win[m], return_counts=True)
        counts[c, w] = cnt
    T_w = np.maximum(1, (counts.max(axis=0) + P - 1) // P).astype(np.int64)
    T_tot = int(T_w.sum())
    col0 = np.concatenate([[0], np.cumsum(T_w)[:-1]])

    srcoff = np.full((NC, P, T_tot), GHOST, np.int32)
    dstrel = np.zeros((NC, P, T_tot), np.float32)
    order = np.argsort(core * np.int64(SLICE * 2) + dstl, kind="stable")
    s_s, d_s, c_s, w_s = srcpad[order], dstl[order], core[order], win[order]
    for c in range(NC):
        m = c_s == c
        sc, dc, wc = s_s[m], d_s[m], w_s[m]
        for w in range(NW):
            mw = wc == w
            k = int(mw.sum())
            tw = int(T_w[w])
            sl = np.full(tw * P, GHOST, np.int32)
            rl = np.zeros(tw * P, np.float32)
            sl[:k] = sc[mw]
            rl[:k] = (dc[mw] - w * P).astype(np.float32)
            cw = int(col0[w])
            srcoff[c, :, cw:cw + tw] = sl.reshape(tw, P).T
            dstrel[c, :, cw:cw + tw] = rl.reshape(tw, P).T

    grow = np.zeros(TCOL, np.uint16)
    grow[64:80] = np.full(8, GHOST_AS, np.float32).view(np.uint16)

    W1 = np.asarray(inputs["W1"], np.float32)
    a_src1 = np.asarray(inputs["a_src1"], np.float32)
    a_dst1 = np.asarray(inputs["a_dst1"], np.float32)
    b1 = np.asarray(inputs["b1"], np.float32)
    W2 = np.asarray(inputs["W2"], np.float32)
    a_src2 = np.asarray(inputs["a_src2"], np.float32)
    a_dst2 = np.asarray(inputs["a_dst2"], np.float32)
    b2 = np.asarray(inputs["b2"], np.float32)
    x = np.asarray(inputs["x"], np.float32)

    A1s = np.zeros((D1, H), np.float32)
    A1d = np.zeros((D1, H), np.float32)
    for h in range(H):
        A1s[h * F1:(h + 1) * F1, h] = a_src1[h]
        A1d[h * F1:(h + 1) * F1, h] = a_dst1[h]
    A2s = np.zeros((H * F2, H), np.float32)
    A2d = np.zeros((H * F2, H), np.float32)
    for h in range(H):
        A2s[h * F2:(h + 1) * F2, h] = a_src2[h]
        A2d[h * F2:(h + 1) * F2, h] = a_dst2[h]

    iotaC = np.broadcast_to(np.arange(P, dtype=np.float32), (P, P)).astype(ml_dtypes.bfloat16)

    shared = dict(
        W1b=W1.astype(ml_dtypes.bfloat16),
        A1s=A1s, A1d=A1d,
        W2Tb=np.ascontiguousarray(W2.T).astype(ml_dtypes.bfloat16),
        A2sb=A2s.astype(ml_dtypes.bfloat16), A2db=A2d.astype(ml_dtypes.bfloat16),
        W2f=W2,
        b1rep=np.broadcast_to(b1, (P, D1)).copy(),
        b2col=np.ascontiguousarray(b2.reshape(F2, 1)),
        iotaC=np.ascontiguousarray(iotaC),
        I128=np.eye(P, dtype=np.float32),
        I128b=np.eye(P, dtype=ml_dtypes.bfloat16),
        I64=np.eye(F2, dtype=np.float32),
        I8=np.eye(H, dtype=np.float32),
        ghostrow=grow.reshape(1, TCOL),
    )
    in_maps = []
    for c in range(NC):
        xs = np.zeros((SPAD, IN_DIM), np.float32)
        xs[:SLICE] = x[c * SLICE:(c + 1) * SLICE]
        m = dict(shared)
        m["xT"] = np.ascontiguousarray(xs.T)
        m["srcoff"] = np.ascontiguousarray(srcoff[c])
        m["dstrel"] = np.ascontiguousarray(dstrel[c]).astype(ml_dtypes.bfloat16)
        in_maps.append(m)

    meta = dict(SLICE=SLICE, NW=NW, SPAD=SPAD, GHOST=GHOST,
                T_w=[int(t) for t in T_w], col0=[int(cc) for cc in col0],
                T_tot=T_tot, NC=NC)
    return in_maps, meta


def build(meta):
    SLICE, NW, SPAD, GHOST, T_tot = (meta["SLICE"], meta["NW"], meta["SPAD"],
                                     meta["GHOST"], meta["T_tot"])
    T_w, col0 = meta["T_w"], meta["col0"]
    CH = min(512, SPAD)
    n_chunks = (SPAD + CH - 1) // CH

    nc = bacc.Bacc('TRN2', num_devices=NC)
    xT = nc.dram_tensor("xT", [IN_DIM, SPAD], F32, kind="ExternalInput")
    srcoff = nc.dram_tensor("srcoff", [P, T_tot], I32, kind="ExternalInput")
    dstrel = nc.dram_tensor("dstrel", [P, T_tot], BF16, kind="ExternalInput")
    W1b_d = nc.dram_tensor("W1b", [IN_DIM, D1], BF16, kind="ExternalInput")
    A1s_d = nc.dram_tensor("A1s", [D1, H], F32, kind="ExternalInput")
    A1d_d = nc.dram_tensor("A1d", [D1, H], F32, kind="ExternalInput")
    W2Tb_d = nc.dram_tensor("W2Tb", [H * F2, F2], BF16, kind="ExternalInput")
    A2sb_d = nc.dram_tensor("A2sb", [H * F2, H], BF16, kind="ExternalInput")
    A2db_d = nc.dram_tensor("A2db", [H * F2, H], BF16, kind="ExternalInput")
    W2f_d = nc.dram_tensor("W2f", [F2, H * F2], F32, kind="ExternalInput")
    b1rep_d = nc.dram_tensor("b1rep", [P, D1], F32, kind="ExternalInput")
    b2col_d = nc.dram_tensor("b2col", [F2, 1], F32, kind="ExternalInput")
    iotaC_d = nc.dram_tensor("iotaC", [P, P], BF16, kind="ExternalInput")
    I128_d = nc.dram_tensor("I128", [P, P], F32, kind="ExternalInput")
    I128b_d = nc.dram_tensor("I128b", [P, P], BF16, kind="ExternalInput")
    I64_d = nc.dram_tensor("I64", [F2, F2], F32, kind="ExternalInput")
    I8_d = nc.dram_tensor("I8", [H, H], F32, kind="ExternalInput")
    ghostrow_d = nc.dram_tensor("ghostrow", [1, TCOL], U16, kind="ExternalInput")
    out_d = nc.dram_tensor("out", [SLICE, F2], F32, kind="ExternalOutput")
    t1loc = nc.dram_tensor("t1loc", [SPAD, TCOL], U16)
    t1full = nc.dram_tensor("t1full", [NC * SPAD + 1, TCOL], U16)
    t2loc = nc.dram_tensor("t2loc", [SPAD, TCOL], U16)
    t2full = nc.dram_tensor("t2full", [NC * SPAD + 1, TCOL], U16)

    with tile.TileContext(nc) as tc:
        with tc.tile_pool(name="consts", bufs=1) as cpool, \
             tc.tile_pool(name="sb", bufs=3) as sb, \
             tc.tile_pool(name="sb8", bufs=8) as sb8, \
             tc.tile_pool(name="pp", bufs=2, space="PSUM") as pp, \
             tc.tile_pool(name="pp1", bufs=1, space="PSUM") as pp1:

            def cload(dram, shape, dtype, tag):
                t = cpool.tile(shape, dtype, tag=tag)
                nc.sync.dma_start(out=t[:], in_=dram[:, :])
                return t

            A1s = cload(A1s_d, [D1, H], F32, "cA1s")
            A1d = cload(A1d_d, [D1, H], F32, "cA1d")
            W2f = cload(W2f_d, [F2, H * F2], F32, "cW2f")
            b1rep = cload(b1rep_d, [P, D1], F32, "cb1")
            b2col = cload(b2col_d, [F2, 1], F32, "cb2")
            iotaC = cload(iotaC_d, [P, P], BF16, "ciota")
            I128 = cload(I128_d, [P, P], F32, "cI128")
            I128b = cload(I128b_d, [P, P], BF16, "cI128b")
            I64 = cload(I64_d, [F2, F2], F32, "cI64")
            I8 = cload(I8_d, [H, H], F32, "cI8")

            W1a = cpool.tile([P, D1], BF16, tag="W1a")
            W1c = cpool.tile([P, D1], BF16, tag="W1c")
            nc.sync.dma_start(out=W1a[:], in_=W1b_d[0:P, :])
            nc.sync.dma_start(out=W1c[:], in_=W1b_d[P:2 * P, :])

            As2 = cpool.tile([F2, H], F32, tag="As2")
            Ad2 = cpool.tile([F2, H], F32, tag="Ad2")
            As2_ps = pp1.tile([F2, H], F32, space="PSUM", tag="t1")
            Ad2_ps = pp1.tile([F2, H], F32, space="PSUM", tag="t2")
            nchk = (H * F2) // P
            w2t_ch, a2s_ch, a2d_ch = [], [], []
            for i in range(nchk):
                wt = cpool.tile([P, F2], BF16, tag=f"w2t{i}")
                as_ = cpool.tile([P, H], BF16, tag=f"a2s{i}")
                ad_ = cpool.tile([P, H], BF16, tag=f"a2d{i}")
                nc.sync.dma_start(out=wt[:], in_=W2Tb_d[i * P:(i + 1) * P, :])
                nc.sync.dma_start(out=as_[:], in_=A2sb_d[i * P:(i + 1) * P, :])
                nc.sync.dma_start(out=ad_[:], in_=A2db_d[i * P:(i + 1) * P, :])
                w2t_ch.append(wt); a2s_ch.append(as_); a2d_ch.append(ad_)
            for i in range(nchk):
                nc.tensor.matmul(out=As2_ps[:], lhsT=w2t_ch[i][:], rhs=a2s_ch[i][:],
                                 start=(i == 0), stop=(i == nchk - 1))
            for i in range(nchk):
                nc.tensor.matmul(out=Ad2_ps[:], lhsT=w2t_ch[i][:], rhs=a2d_ch[i][:],
                                 start=(i == 0), stop=(i == nchk - 1))
            nc.vector.tensor_copy(out=As2[:], in_=As2_ps[:])
            nc.vector.tensor_copy(out=Ad2[:], in_=Ad2_ps[:])

            grow_sb = cpool.tile([1, TCOL], U16, tag="grow")
            nc.sync.dma_start(out=grow_sb[:], in_=ghostrow_d[:, :])
            nc.sync.dma_start(out=t1full[GHOST:GHOST + 1, :], in_=grow_sb[:])
            nc.sync.dma_start(out=t2full[GHOST:GHOST + 1, :], in_=grow_sb[:])

            ad1_sb = cpool.tile([P, NW * 16], BF16, tag="ad1sb")
            ad2_sb = cpool.tile([P, NW * 16], BF16, tag="ad2sb")

            def split_hilo(hi_ap, lo_ap, src_f32):
                nc.vector.tensor_copy(out=hi_ap, in_=src_f32)
                nc.vector.tensor_tensor(out=lo_ap, in0=src_f32, in1=hi_ap,
                                        op=mybir.AluOpType.subtract)

            for k in range(n_chunks):
                c0, c1 = k * CH, min((k + 1) * CH, SPAD)
                cw = c1 - c0
                w0 = c0 // P
                xa = sb.tile([P, CH], F32, tag="xa")
                xb = sb.tile([P, CH], F32, tag="xb")
                nc.sync.dma_start(out=xa[:, :cw], in_=xT[0:P, c0:c1])
                nc.sync.dma_start(out=xb[:, :cw], in_=xT[P:2 * P, c0:c1])
                xab = sb.tile([P, CH], BF16, tag="xab")
                xbb = sb.tile([P, CH], BF16, tag="xbb")
                nc.vector.tensor_copy(out=xab[:, :cw], in_=xa[:, :cw])
                nc.vector.tensor_copy(out=xbb[:, :cw], in_=xb[:, :cw])
                h1T_ps = pp.tile([D1, CH], F32, space="PSUM", tag="U")
                nc.tensor.matmul(out=h1T_ps[:, :cw], lhsT=W1a[:], rhs=xab[:, :cw],
                                 start=True, stop=False)
                nc.tensor.matmul(out=h1T_ps[:, :cw], lhsT=W1c[:], rhs=xbb[:, :cw],
                                 start=False, stop=True)
                h1T_f = sb.tile([D1, CH], F32, tag="h1Tf")
                h1T_b = sb.tile([D1, CH], BF16, tag="h1Tb")
                nc.vector.tensor_copy(out=h1T_f[:, :cw], in_=h1T_ps[:, :cw])
                nc.vector.tensor_copy(out=h1T_b[:, :cw], in_=h1T_ps[:, :cw])
                as1T_ps = pp1.tile([H, CH], F32, space="PSUM", tag="den")
                ad1T_ps = pp1.tile([H, CH], F32, space="PSUM", tag="adps")
                nc.tensor.matmul(out=as1T_ps[:, :cw], lhsT=A1s[:], rhs=h1T_f[:, :cw],
                                 start=True, stop=True)
                nc.tensor.matmul(out=ad1T_ps[:, :cw], lhsT=A1d[:], rhs=h1T_f[:, :cw],
                                 start=True, stop=True)
                as1T_f = sb.tile([H, CH], F32, tag="as1Tf")
                ad1T_f = sb.tile([H, CH], F32, tag="ad1Tf")
                nc.vector.tensor_copy(out=as1T_f[:, :cw], in_=as1T_ps[:, :cw])
                nc.vector.tensor_copy(out=ad1T_f[:, :cw], in_=ad1T_ps[:, :cw])
                for b in range(cw // P):
                    nn = c0 + b * P
                    w = w0 + b
                    hnm = sb.tile([P, D1], BF16, tag="hnm")
                    nc.sync.dma_start_transpose(hnm[:], h1T_b[:, b * P:(b + 1) * P])
                    nc.sync.dma_start(out=t1loc[nn:nn + P, 0:D1].bitcast(BF16),
                                      in_=hnm[:])
                    asT_ps = pp1.tile([P, H], F32, space="PSUM", tag="t1")
                    nc.tensor.matmul(out=asT_ps[:], lhsT=as1T_f[:, b * P:(b + 1) * P],
                                     rhs=I8[:], is_transpose=True, start=True, stop=True)
                    asnm = sb.tile([P, H], F32, tag="asnm")
                    nc.vector.tensor_copy(out=asnm[:], in_=asT_ps[:])
                    nc.sync.dma_start(out=t1loc[nn:nn + P, D1:TCOL].bitcast(F32),
                                      in_=asnm[:])
                    adT_ps = pp1.tile([P, H], F32, space="PSUM", tag="t2")
                    nc.tensor.matmul(out=adT_ps[:], lhsT=ad1T_f[:, b * P:(b + 1) * P],
                                     rhs=I8[:], is_transpose=True, start=True, stop=True)
                    adnm = sb.tile([P, H], F32, tag="adnm")
                    nc.vector.tensor_copy(out=adnm[:], in_=adT_ps[:])
                    split_hilo(ad1_sb[:, w * 16:w * 16 + 8],
                               ad1_sb[:, w * 16 + 8:w * 16 + 16], adnm[:])

            nc.gpsimd.collective_compute(
                "AllGather", mybir.AluOpType.bypass,
                replica_groups=[list(range(NC))],
                ins=[t1loc[:, :].opt()],
                outs=[t1full[0:NC * SPAD, :].opt()],
            )

            def edge_phase(tfull, ad_sb, layer):
                NCOLS = D1 if layer == 1 else H * F2
                FV = F1 if layer == 1 else F2
                for w in range(NW):
                    tw = T_w[w]
                    cwid = col0[w]
                    so_w = sb.tile([P, tw], I32, tag="so_w")
                    dr_w = sb.tile([P, tw], BF16, tag="dr_w")
                    nc.sync.dma_start(out=so_w[:], in_=srcoff[:, cwid:cwid + tw])
                    nc.sync.dma_start(out=dr_w[:], in_=dstrel[:, cwid:cwid + tw])
                    U_ps = pp.tile([P, NCOLS], F32, space="PSUM", tag="U")
                    den_ps = pp1.tile([P, H], F32, space="PSUM", tag="den")
                    for t in range(tw):
                        g = sb8.tile([P, TCOL], U16, tag="g")
                        nc.gpsimd.indirect_dma_start(
                            out=g[:], out_offset=None, in_=tfull[:, :],
                            in_offset=bass.IndirectOffsetOnAxis(
                                ap=so_w[:, t:t + 1], axis=0),
                        )
                        s_t = sb8.tile([P, P], BF16, tag="s")
                        nc.vector.tensor_tensor(
                            out=s_t[:], in0=dr_w[:, t:t + 1].to_broadcast([P, P]),
                            in1=iotaC[:], op=mybir.AluOpType.is_equal)
                        st_ps = pp1.tile([P, P], BF16, space="PSUM", tag="stp")
                        nc.tensor.matmul(out=st_ps[:], lhsT=s_t[:], rhs=I128b[:],
                                         is_transpose=True, start=True, stop=True)
                        st_t = sb8.tile([P, P], BF16, tag="st")
                        nc.vector.tensor_copy(out=st_t[:], in_=st_ps[:])
                        ad_ps = pp1.tile([P, H], F32, space="PSUM", tag="adps")
                        nc.tensor.matmul(out=ad_ps[:], lhsT=st_t[:],
                                         rhs=ad_sb[:, w * 16:w * 16 + 8],
                                         start=True, stop=False)
                        nc.tensor.matmul(out=ad_ps[:], lhsT=st_t[:],
                                         rhs=ad_sb[:, w * 16 + 8:w * 16 + 16],
                                         start=False, stop=True)
                        e_t = sb8.tile([P, H], F32, tag="e")
                        nc.vector.tensor_tensor(out=e_t[:],
                                                in0=g[:, D1:TCOL].bitcast(F32),
                                                in1=ad_ps[:], op=mybir.AluOpType.add)
                        lr = sb8.tile([P, H], F32, tag="lr")
                        nc.vector.tensor_scalar_mul(out=lr[:], in0=e_t[:], scalar1=NEG)
                        nc.vector.tensor_tensor(out=lr[:], in0=lr[:], in1=e_t[:],
                                                op=mybir.AluOpType.max)
                        p_b = sb8.tile([P, H], BF16, tag="p")
                        nc.scalar.activation(p_b[:], lr[:],
                                             mybir.ActivationFunctionType.Exp)
                        w_t = sb8.tile([P, H * FV], BF16, tag="wv")
                        gh = g[:, 0:D1].bitcast(BF16)
                        if layer == 1:
                            in0 = gh.rearrange("p (h f) -> p h f", h=H)
                        else:
                            in0 = mid_bcast(gh, H)
                        nc.vector.tensor_tensor(
                            out=w_t[:].rearrange("p (h f) -> p h f", h=H),
                            in0=in0,
                            in1=p_b[:].to_broadcast([P, H, FV]),
                            op=mybir.AluOpType.mult)
                        nc.tensor.matmul(out=U_ps[:], lhsT=s_t[:], rhs=w_t[:],
                                         start=(t == 0), stop=(t == tw - 1))
                        nc.tensor.matmul(out=den_ps[:], lhsT=s_t[:], rhs=p_b[:],
                                         start=(t == 0), stop=(t == tw - 1))
                    dse = sb.tile([P, H], F32, tag="dse")
                    nc.vector.tensor_scalar_add(out=dse[:], in0=den_ps[:], scalar1=1e-30)
                    rd = sb.tile([P, H], F32, tag="rd")
                    nc.vector.reciprocal(out=rd[:], in_=dse[:])
                    if layer == 1:
                        h2a = sb.tile([P, D1], F32, tag="h2a")
                        nc.vector.tensor_tensor(
                            out=h2a[:].rearrange("p (h f) -> p h f", h=H),
                            in0=U_ps[:].rearrange("p (h f) -> p h f", h=H),
                            in1=rd[:].to_broadcast([P, H, F1]),
                            op=mybir.AluOpType.mult)
                        nc.vector.tensor_tensor(out=h2a[:], in0=h2a[:], in1=b1rep[:],
                                                op=mybir.AluOpType.add)
                        ex = sb.tile([P, D1], F32, tag="ex")
                        nc.scalar.activation(ex[:], h2a[:],
                                             mybir.ActivationFunctionType.Exp)
                        exm = sb.tile([P, D1], F32, tag="exm")
                        nc.vector.tensor_scalar(out=exm[:], in0=ex[:], scalar1=1.0,
                                                scalar2=-1.0, op0=mybir.AluOpType.min,
                                                op1=mybir.AluOpType.add)
                        rl = sb.tile([P, D1], F32, tag="rl")
                        nc.vector.tensor_scalar_max(out=rl[:], in0=h2a[:], scalar1=0.0)
                        h2e = sb.tile([P, D1], F32, tag="h2e")
                        nc.vector.tensor_tensor(out=h2e[:], in0=exm[:], in1=rl[:],
                                                op=mybir.AluOpType.add)
                        h2eb = sb.tile([P, D1], BF16, tag="h2eb")
                        nc.vector.tensor_copy(out=h2eb[:], in_=h2e[:])
                        nc.sync.dma_start(
                            out=t2loc[w * P:(w + 1) * P, 0:D1].bitcast(BF16),
                            in_=h2eb[:])
                        hT_ps = pp1.tile([D1, P], F32, space="PSUM", tag="t1")
                        nc.tensor.matmul(out=hT_ps[:], lhsT=h2e[:], rhs=I128[:],
                                         is_transpose=True, start=True, stop=True)
                        hT = sb.tile([D1, P], F32, tag="hT")
                        nc.vector.tensor_copy(out=hT[:], in_=hT_ps[:])
                        a2T_ps = pp1.tile([H, P], F32, space="PSUM", tag="t2")
                        nc.tensor.matmul(out=a2T_ps[:], lhsT=As2[:], rhs=hT[:],
                                         start=True, stop=True)
                        d2T_ps = pp1.tile([H, P], F32, space="PSUM", tag="t3")
                        nc.tensor.matmul(out=d2T_ps[:], lhsT=Ad2[:], rhs=hT[:],
                                         start=True, stop=True)
                        a2T = sb.tile([H, P], F32, tag="a2T")
                        d2T = sb.tile([H, P], F32, tag="d2T")
                        nc.vector.tensor_copy(out=a2T[:], in_=a2T_ps[:])
                        nc.vector.tensor_copy(out=d2T[:], in_=d2T_ps[:])
                        a2nm_ps = pp1.tile([P, H], F32, space="PSUM", tag="t1")
                        nc.tensor.matmul(out=a2nm_ps[:], lhsT=a2T[:], rhs=I8[:],
                                         is_transpose=True, start=True, stop=True)
                        a2nm = sb.tile([P, H], F32, tag="a2nm")
                        nc.vector.tensor_copy(out=a2nm[:], in_=a2nm_ps[:])
                        nc.sync.dma_start(
                            out=t2loc[w * P:(w + 1) * P, D1:TCOL].bitcast(F32),
                            in_=a2nm[:])
                        d2nm_ps = pp1.tile([P, H], F32, space="PSUM", tag="t2")
                        nc.tensor.matmul(out=d2nm_ps[:], lhsT=d2T[:], rhs=I8[:],
                                         is_transpose=True, start=True, stop=True)
                        d2nm = sb.tile([P, H], F32, tag="d2nm")
                        nc.vector.tensor_copy(out=d2nm[:], in_=d2nm_ps[:])
                        split_hilo(ad2_sb[:, w * 16:w * 16 + 8],
                                   ad2_sb[:, w * 16 + 8:w * 16 + 16], d2nm[:])
                    else:
                        U2n = sb.tile([P, H * F2], F32, tag="U2n")
                        nc.vector.tensor_tensor(
                            out=U2n[:].rearrange("p (h f) -> p h f", h=H),
                            in0=U_ps[:].rearrange("p (h f) -> p h f", h=H),
                            in1=rd[:].to_broadcast([P, H, F2]),
                            op=mybir.AluOpType.mult)
                        YT_ps = pp1.tile([F2, P], F32, space="PSUM", tag="t3")
                        for h in range(H):
                            uT_ps = pp1.tile([F2, P], F32, space="PSUM", tag="t1")
                            nc.tensor.matmul(out=uT_ps[:],
                                             lhsT=U2n[:, h * F2:(h + 1) * F2],
                                             rhs=I128[:], is_transpose=True,
                                             start=True, stop=True)
                            uT = sb.tile([F2, P], F32, tag="uTs")
                            nc.vector.tensor_copy(out=uT[:], in_=uT_ps[:])
                            nc.tensor.matmul(out=YT_ps[:],
                                             lhsT=W2f[:, h * F2:(h + 1) * F2],
                                             rhs=uT[:], start=(h == 0),
                                             stop=(h == H - 1))
                        Y = sb.tile([F2, P], F32, tag="Y")
                        nc.scalar.activation(Y[:], YT_ps[:],
                                             mybir.ActivationFunctionType.Identity,
                                             bias=b2col[:], scale=1.0 / H)
                        o_ps = pp1.tile([P, F2], F32, space="PSUM", tag="t2")
                        nc.tensor.matmul(out=o_ps[:], lhsT=Y[:], rhs=I64[:],
                                         is_transpose=True, start=True, stop=True)
                        ow = sb.tile([P, F2], F32, tag="ow")
                        nc.vector.tensor_copy(out=ow[:], in_=o_ps[:])
                        rows = min(P, SLICE - w * P)
                        nc.sync.dma_start(out=out_d[w * P:w * P + rows, :],
                                          in_=ow[:rows, :])

            edge_phase(t1full, ad1_sb, 1)
            nc.gpsimd.collective_compute(
                "AllGather", mybir.AluOpType.bypass,
                replica_groups=[list(range(NC))],
                ins=[t2loc[:, :].opt()],
                outs=[t2full[0:NC * SPAD, :].opt()],
            )
            edge_phase(t2full, ad2_sb, 2)

    nc.compile()
    return nc


def kernel(**inputs):
    global LAST_EXEC_NS
    _register_profile_hook()
    from concourse import bass_utils

    in_maps, meta = host_prep(inputs)
    nc = build(meta)
    trace = os.environ.get("GAT_TRACE", "1") == "1"
    try:
        res = bass_utils.run_bass_kernel_spmd(
            nc, in_maps, core_ids=list(range(NC)), trace=trace)
    except Exception:
        if not trace:
            raise
        res = bass_utils.run_bass_kernel_spmd(
            nc, in_maps, core_ids=list(range(NC)), trace=False)
    LAST_EXEC_NS = res.exec_time_ns
    SLICE = meta["SLICE"]
    out = np.empty((N, F2), np.float32)
    for c in range(NC):
        out[c * SLICE:(c + 1) * SLICE] = res.results[c]["out"]
    return out



# revision 7
# speedup vs baseline: 1.0189x; 1.0189x over previous
"""Self-contained GAT kernel for 8 TRN2 NeuronCores.

kernel(**inputs) takes the FULL unsharded inputs (as produced by
setup_inputs) and returns the FULL [100000, 64] float32 output.

Architecture (see module gat_kernel-style doc):
- nodes dst-partitioned across 8 cores; edges dst-sorted into 128-dst windows,
  128-edge tiles.
- per-node table rows [h bf16 x64 | a_src f32 x8] packed as uint16[80];
  per-tile [128,1]-offset indirect-DMA gather.
- segment softmax/sums via one-hot selection matrices + PE matmuls; a_dst
  expanded per edge via DMA-transposed one-hot (S^T) matmuls from SBUF
  tables (bf16 hi+lo split for f32 accuracy).
- layer 2 aggregates 64-dim h2 per head and applies W2 after aggregation;
  head-mean via PSUM-accumulated per-head matmuls.
- AllGather collectives replicate node tables between phases.
"""
import os
import sys
import types

import numpy as np

sys.path.insert(0, "/opt/trn_rl_repo")

import ml_dtypes

import concourse.bass as bass
import concourse.bacc as bacc
import concourse.mybir as mybir
import concourse.tile as tile

BF16 = mybir.dt.bfloat16
F32 = mybir.dt.float32
I32 = mybir.dt.int32
U16 = mybir.dt.uint16

P = 128
H = 8
F1 = 8
F2 = 64
D1 = H * F1
IN_DIM = 256
NEG = 0.2
GHOST_AS = -300.0
TCOL = 80
NC = 8
N = 100000

LAST_EXEC_NS = None

_hook_registered = [False]


def _register_profile_hook():
    if _hook_registered[0]:
        return
    try:
        import antenv
        mod = types.ModuleType("antenv.axon_hooks")
        _h = [None]
        mod.set_axon_ntff_profile_hook = lambda f: _h.__setitem__(0, f)
        mod.get_axon_ntff_profile_hook = lambda: _h[0]
        sys.modules.setdefault("antenv.axon_hooks", mod)
        if not hasattr(antenv, "axon_hooks"):
            antenv.axon_hooks = mod
        from trn_agent_boot.trn_boot import _ntff_profile_via_ctypes
        sys.modules["antenv.axon_hooks"].set_axon_ntff_profile_hook(
            _ntff_profile_via_ctypes('/opt/axon/libaxon_pjrt.so'))
        _hook_registered[0] = True
    except Exception:
        pass


def mid_bcast(ap2d, reps):
    return bass.AP(ap2d.tensor, ap2d.offset, [ap2d.ap[0], [0, reps], ap2d.ap[1]])


def view(ap, off_elems, dims):
    """Custom strided view: dims = [[stride, count], ...] in ap-dtype elems."""
    return bass.AP(ap.tensor, ap.offset + off_elems, [ap.ap[0]] + dims)


def host_prep(inputs):
    SLICE = N // NC
    NW = (SLICE + P - 1) // P
    SPAD = NW * P
    GHOST = NC * SPAD

    edge = np.asarray(inputs["edge"])
    src = np.concatenate([np.asarray(edge[0]), np.arange(N, dtype=np.int64)])
    dst = np.concatenate([np.asarray(edge[1]), np.arange(N, dtype=np.int64)])

    core = (dst // SLICE).astype(np.int32)
    srcpad = ((src // SLICE) * SPAD + (src % SLICE)).astype(np.int32)
    dstl = (dst % SLICE).astype(np.int32)
    win = dstl // P

    counts = np.zeros((NC, NW), np.int64)
    for c in range(NC):
        m = core == c
        w, cnt = np.unique(win[m], return_counts=True)
        counts[c, w] = cnt
    T_w = np.maximum(1, (counts.max(axis=0) + P - 1) // P).astype(np.int64)
    T_tot = int(T_w.sum())
    col0 = np.concatenate([[0], np.cumsum(T_w)[:-1]])

    srcoff = np.full((NC, P, T_tot), GHOST, np.int32)
    dstrel = np.zeros((NC, P, T_tot), np.float32)
    order = np.argsort(core * np.int64(SLICE * 2) + dstl, kind="stable")
    s_s, d_s, c_s, w_s = srcpad[order], dstl[order], core[order], win[order]
    for c in range(NC):
        m = c_s == c
        sc, dc, wc = s_s[m], d_s[m], w_s[m]
        for w in range(NW):
            mw = wc == w
            k = int(mw.sum())
            tw = int(T_w[w])
            sl = np.full(tw * P, GHOST, np.int32)
            rl = np.zeros(tw * P, np.float32)
            sl[:k] = sc[mw]
            rl[:k] = (dc[mw] - w * P).astype(np.float32)
            cw = int(col0[w])
            srcoff[c, :, cw:cw + tw] = sl.reshape(tw, P).T
            dstrel[c, :, cw:cw + tw] = rl.reshape(tw, P).T

    grow = np.zeros(TCOL, np.uint16)
    grow[64:80] = np.full(8, GHOST_AS, np.float32).view(np.uint16)

    W1 = np.asarray(inputs["W1"], np.float32)
    a_src1 = np.asarray(inputs["a_src1"], np.float32)
    a_dst1 = np.asarray(inputs["a_dst1"], np.float32)
    b1 = np.asarray(inputs["b1"], np.float32)
    W2 = np.asarray(inputs["W2"], np.float32)
    a_src2 = np.asarray(inputs["a_src2"], np.float32)
    a_dst2 = np.asarray(inputs["a_dst2"], np.float32)
    b2 = np.asarray(inputs["b2"], np.float32)
    x = np.asarray(inputs["x"], np.float32)

    A1s = np.zeros((D1, H), np.float32)
    A1d = np.zeros((D1, H), np.float32)
    for h in range(H):
        A1s[h * F1:(h + 1) * F1, h] = a_src1[h]
        A1d[h * F1:(h + 1) * F1, h] = a_dst1[h]
    A2s = np.zeros((H * F2, H), np.float32)
    A2d = np.zeros((H * F2, H), np.float32)
    for h in range(H):
        A2s[h * F2:(h + 1) * F2, h] = a_src2[h]
        A2d[h * F2:(h + 1) * F2, h] = a_dst2[h]

    iotaC = np.broadcast_to(np.arange(P, dtype=np.float32), (P, P)).astype(ml_dtypes.bfloat16)

    shared = dict(
        W1b=W1.astype(ml_dtypes.bfloat16),
        A1s=A1s, A1d=A1d,
        W2Tb=np.ascontiguousarray(W2.T).astype(ml_dtypes.bfloat16),
        A2sb=A2s.astype(ml_dtypes.bfloat16), A2db=A2d.astype(ml_dtypes.bfloat16),
        W2f=W2,
        b1rep=np.broadcast_to(b1, (P, D1)).copy(),
        b2col=np.ascontiguousarray(b2.reshape(F2, 1)),
        iotaC=np.ascontiguousarray(iotaC),
        I128=np.eye(P, dtype=np.float32),
        I128b=np.eye(P, dtype=ml_dtypes.bfloat16),
        I64=np.eye(F2, dtype=np.float32),
        I8=np.eye(H, dtype=np.float32),
        ghostrow=grow.reshape(1, TCOL),
    )
    in_maps = []
    for c in range(NC):
        xs = np.zeros((SPAD, IN_DIM), np.float32)
        xs[:SLICE] = x[c * SLICE:(c + 1) * SLICE]
        m = dict(shared)
        m["xT"] = np.ascontiguousarray(xs.T)
        m["srcoff"] = np.ascontiguousarray(srcoff[c])
        m["dstrel"] = np.ascontiguousarray(dstrel[c]).astype(ml_dtypes.bfloat16)
        in_maps.append(m)

    meta = dict(SLICE=SLICE, NW=NW, SPAD=SPAD, GHOST=GHOST,
                T_w=[int(t) for t in T_w], col0=[int(cc) for cc in col0],
                T_tot=T_tot, NC=NC)
    return in_maps, meta


def build(meta):
    SLICE, NW, SPAD, GHOST, T_tot = (meta["SLICE"], meta["NW"], meta["SPAD"],
                                     meta["GHOST"], meta["T_tot"])
    T_w, col0 = meta["T_w"], meta["col0"]
    TMAX = max(T_w)
    CH = min(512, SPAD)
    n_chunks = (SPAD + CH - 1) // CH

    nc = bacc.Bacc('TRN2', num_devices=NC)
    xT = nc.dram_tensor("xT", [IN_DIM, SPAD], F32, kind="ExternalInput")
    srcoff = nc.dram_tensor("srcoff", [P, T_tot], I32, kind="ExternalInput")
    dstrel = nc.dram_tensor("dstrel", [P, T_tot], BF16, kind="ExternalInput")
    W1b_d = nc.dram_tensor("W1b", [IN_DIM, D1], BF16, kind="ExternalInput")
    A1s_d = nc.dram_tensor("A1s", [D1, H], F32, kind="ExternalInput")
    A1d_d = nc.dram_tensor("A1d", [D1, H], F32, kind="ExternalInput")
    W2Tb_d = nc.dram_tensor("W2Tb", [H * F2, F2], BF16, kind="ExternalInput")
    A2sb_d = nc.dram_tensor("A2sb", [H * F2, H], BF16, kind="ExternalInput")
    A2db_d = nc.dram_tensor("A2db", [H * F2, H], BF16, kind="ExternalInput")
    W2f_d = nc.dram_tensor("W2f", [F2, H * F2], F32, kind="ExternalInput")
    b1rep_d = nc.dram_tensor("b1rep", [P, D1], F32, kind="ExternalInput")
    b2col_d = nc.dram_tensor("b2col", [F2, 1], F32, kind="ExternalInput")
    iotaC_d = nc.dram_tensor("iotaC", [P, P], BF16, kind="ExternalInput")
    I128_d = nc.dram_tensor("I128", [P, P], F32, kind="ExternalInput")
    I128b_d = nc.dram_tensor("I128b", [P, P], BF16, kind="ExternalInput")
    I64_d = nc.dram_tensor("I64", [F2, F2], F32, kind="ExternalInput")
    I8_d = nc.dram_tensor("I8", [H, H], F32, kind="ExternalInput")
    ghostrow_d = nc.dram_tensor("ghostrow", [1, TCOL], U16, kind="ExternalInput")
    out_d = nc.dram_tensor("out", [SLICE, F2], F32, kind="ExternalOutput")
    t1loc = nc.dram_tensor("t1loc", [SPAD, TCOL], U16)
    t1full = nc.dram_tensor("t1full", [NC * SPAD + 1, TCOL], U16)
    t2loc = nc.dram_tensor("t2loc", [SPAD, TCOL], U16)
    t2full = nc.dram_tensor("t2full", [NC * SPAD + 1, TCOL], U16)

    with tile.TileContext(nc) as tc:
        with tc.tile_pool(name="consts", bufs=1) as cpool, \
             tc.tile_pool(name="sb", bufs=3) as sb, \
             tc.tile_pool(name="sb8", bufs=8) as sb8, \
             tc.tile_pool(name="gp", bufs=3) as gp, \
             tc.tile_pool(name="pp", bufs=2, space="PSUM") as pp, \
             tc.tile_pool(name="pp1", bufs=1, space="PSUM") as pp1:

            def cload(dram, shape, dtype, tag):
                t = cpool.tile(shape, dtype, tag=tag)
                nc.sync.dma_start(out=t[:], in_=dram[:, :])
                return t

            A1s = cload(A1s_d, [D1, H], F32, "cA1s")
            A1d = cload(A1d_d, [D1, H], F32, "cA1d")
            W2f = cload(W2f_d, [F2, H * F2], F32, "cW2f")
            b1rep = cload(b1rep_d, [P, D1], F32, "cb1")
            b2col = cload(b2col_d, [F2, 1], F32, "cb2")
            iotaC = cload(iotaC_d, [P, P], BF16, "ciota")
            I128 = cload(I128_d, [P, P], F32, "cI128")
            I128b = cload(I128b_d, [P, P], BF16, "cI128b")
            I64 = cload(I64_d, [F2, F2], F32, "cI64")
            I8 = cload(I8_d, [H, H], F32, "cI8")

            W1a = cpool.tile([P, D1], BF16, tag="W1a")
            W1c = cpool.tile([P, D1], BF16, tag="W1c")
            nc.sync.dma_start(out=W1a[:], in_=W1b_d[0:P, :])
            nc.sync.dma_start(out=W1c[:], in_=W1b_d[P:2 * P, :])

            As2 = cpool.tile([F2, H], F32, tag="As2")
            Ad2 = cpool.tile([F2, H], F32, tag="Ad2")
            As2_ps = pp1.tile([F2, H], F32, space="PSUM", tag="t1")
            Ad2_ps = pp1.tile([F2, H], F32, space="PSUM", tag="t2")
            nchk = (H * F2) // P
            w2t_ch, a2s_ch, a2d_ch = [], [], []
            for i in range(nchk):
                wt = cpool.tile([P, F2], BF16, tag=f"w2t{i}")
                as_ = cpool.tile([P, H], BF16, tag=f"a2s{i}")
                ad_ = cpool.tile([P, H], BF16, tag=f"a2d{i}")
                nc.sync.dma_start(out=wt[:], in_=W2Tb_d[i * P:(i + 1) * P, :])
                nc.sync.dma_start(out=as_[:], in_=A2sb_d[i * P:(i + 1) * P, :])
                nc.sync.dma_start(out=ad_[:], in_=A2db_d[i * P:(i + 1) * P, :])
                w2t_ch.append(wt); a2s_ch.append(as_); a2d_ch.append(ad_)
            for i in range(nchk):
                nc.tensor.matmul(out=As2_ps[:], lhsT=w2t_ch[i][:], rhs=a2s_ch[i][:],
                                 start=(i == 0), stop=(i == nchk - 1))
            for i in range(nchk):
                nc.tensor.matmul(out=Ad2_ps[:], lhsT=w2t_ch[i][:], rhs=a2d_ch[i][:],
                                 start=(i == 0), stop=(i == nchk - 1))
            nc.vector.tensor_copy(out=As2[:], in_=As2_ps[:])
            nc.vector.tensor_copy(out=Ad2[:], in_=Ad2_ps[:])

            grow_sb = cpool.tile([1, TCOL], U16, tag="grow")
            nc.sync.dma_start(out=grow_sb[:], in_=ghostrow_d[:, :])
            nc.sync.dma_start(out=t1full[GHOST:GHOST + 1, :], in_=grow_sb[:])
            nc.sync.dma_start(out=t2full[GHOST:GHOST + 1, :], in_=grow_sb[:])

            ad1_sb = cpool.tile([P, NW * 16], BF16, tag="ad1sb")
            ad2_sb = cpool.tile([P, NW * 16], BF16, tag="ad2sb")

            so_sb = cpool.tile([P, T_tot], I32, tag="sosb")
            dr_sb = cpool.tile([P, T_tot], BF16, tag="drsb")
            nc.sync.dma_start(out=so_sb[:], in_=srcoff[:, :])
            nc.sync.dma_start(out=dr_sb[:], in_=dstrel[:, :])

            def split_hilo(hi_ap, lo_ap, src_f32):
                nc.vector.tensor_copy(out=hi_ap, in_=src_f32)
                nc.vector.tensor_tensor(out=lo_ap, in0=src_f32, in1=hi_ap,
                                        op=mybir.AluOpType.subtract)

            for k in range(n_chunks):
                c0, c1 = k * CH, min((k + 1) * CH, SPAD)
                cw = c1 - c0
                w0 = c0 // P
                xa = sb.tile([P, CH], F32, tag="xa")
                xb = sb.tile([P, CH], F32, tag="xb")
                nc.sync.dma_start(out=xa[:, :cw], in_=xT[0:P, c0:c1])
                nc.sync.dma_start(out=xb[:, :cw], in_=xT[P:2 * P, c0:c1])
                xab = sb.tile([P, CH], BF16, tag="xab")
                xbb = sb.tile([P, CH], BF16, tag="xbb")
                nc.vector.tensor_copy(out=xab[:, :cw], in_=xa[:, :cw])
                nc.vector.tensor_copy(out=xbb[:, :cw], in_=xb[:, :cw])
                h1T_ps = pp.tile([D1, CH], F32, space="PSUM", tag="U")
                nc.tensor.matmul(out=h1T_ps[:, :cw], lhsT=W1a[:], rhs=xab[:, :cw],
                                 start=True, stop=False)
                nc.tensor.matmul(out=h1T_ps[:, :cw], lhsT=W1c[:], rhs=xbb[:, :cw],
                                 start=False, stop=True)
                h1T_f = sb.tile([D1, CH], F32, tag="h1Tf")
                h1T_b = sb.tile([D1, CH], BF16, tag="h1Tb")
                nc.vector.tensor_copy(out=h1T_f[:, :cw], in_=h1T_ps[:, :cw])
                nc.vector.tensor_copy(out=h1T_b[:, :cw], in_=h1T_ps[:, :cw])
                as1T_ps = pp1.tile([H, CH], F32, space="PSUM", tag="den")
                ad1T_ps = pp1.tile([H, CH], F32, space="PSUM", tag="adps")
                nc.tensor.matmul(out=as1T_ps[:, :cw], lhsT=A1s[:], rhs=h1T_f[:, :cw],
                                 start=True, stop=True)
                nc.tensor.matmul(out=ad1T_ps[:, :cw], lhsT=A1d[:], rhs=h1T_f[:, :cw],
                                 start=True, stop=True)
                as1T_f = sb.tile([H, CH], F32, tag="as1Tf")
                ad1T_f = sb.tile([H, CH], F32, tag="ad1Tf")
                nc.vector.tensor_copy(out=as1T_f[:, :cw], in_=as1T_ps[:, :cw])
                nc.vector.tensor_copy(out=ad1T_f[:, :cw], in_=ad1T_ps[:, :cw])
                for b in range(cw // P):
                    nn = c0 + b * P
                    w = w0 + b
                    hnm = sb.tile([P, D1], BF16, tag="hnm")
                    nc.sync.dma_start_transpose(hnm[:], h1T_b[:, b * P:(b + 1) * P])
                    nc.sync.dma_start(out=t1loc[nn:nn + P, 0:D1].bitcast(BF16),
                                      in_=hnm[:])
                    asT_ps = pp1.tile([P, H], F32, space="PSUM", tag="t1")
                    nc.tensor.matmul(out=asT_ps[:], lhsT=as1T_f[:, b * P:(b + 1) * P],
                                     rhs=I8[:], is_transpose=True, start=True, stop=True)
                    asnm = sb.tile([P, H], F32, tag="asnm")
                    nc.vector.tensor_copy(out=asnm[:], in_=asT_ps[:])
                    nc.sync.dma_start(out=t1loc[nn:nn + P, D1:TCOL].bitcast(F32),
                                      in_=asnm[:])
                    adT_ps = pp1.tile([P, H], F32, space="PSUM", tag="t2")
                    nc.tensor.matmul(out=adT_ps[:], lhsT=ad1T_f[:, b * P:(b + 1) * P],
                                     rhs=I8[:], is_transpose=True, start=True, stop=True)
                    adnm = sb.tile([P, H], F32, tag="adnm")
                    nc.vector.tensor_copy(out=adnm[:], in_=adT_ps[:])
                    split_hilo(ad1_sb[:, w * 16:w * 16 + 8],
                               ad1_sb[:, w * 16 + 8:w * 16 + 16], adnm[:])

            nc.gpsimd.collective_compute(
                "AllGather", mybir.AluOpType.bypass,
                replica_groups=[list(range(NC))],
                ins=[t1loc[:, :].opt()],
                outs=[t1full[0:NC * SPAD, :].opt()],
            )

            def edge_phase(tfull, ad_sb, layer):
                NCOLS = D1 if layer == 1 else H * F2
                FV = F1 if layer == 1 else F2
                for w in range(NW):
                    tw = T_w[w]
                    cwid = col0[w]
                    U_ps = pp.tile([P, NCOLS], F32, space="PSUM", tag="U")
                    den_ps = pp1.tile([P, H], F32, space="PSUM", tag="den")
                    g_all = gp.tile([P, TMAX * TCOL], U16, tag="ga")
                    s_all = gp.tile([P, TMAX * P], BF16, tag="sa")
                    ad_all = gp.tile([P, TMAX * 16], F32, tag="ada")
                    e_all = gp.tile([P, TMAX * H], F32, tag="ea")
                    lr_all = gp.tile([P, TMAX * H], F32, tag="la")
                    p_all = gp.tile([P, TMAX * H], BF16, tag="pa")
                    for t in range(tw):
                        nc.gpsimd.indirect_dma_start(
                            out=g_all[:, t * TCOL:(t + 1) * TCOL], out_offset=None,
                            in_=tfull[:, :],
                            in_offset=bass.IndirectOffsetOnAxis(
                                ap=so_sb[:, cwid + t:cwid + t + 1], axis=0),
                        )
                    nc.vector.tensor_tensor(
                        out=view(s_all[:], 0, [[P, tw], [1, P]]),
                        in0=view(dr_sb[:], cwid, [[1, tw], [0, P]]),
                        in1=view(iotaC[:], 0, [[0, tw], [1, P]]),
                        op=mybir.AluOpType.is_equal)
                    for t in range(tw):
                        st_ps = pp1.tile([P, P], BF16, space="PSUM", tag="stp")
                        nc.tensor.matmul(out=st_ps[:],
                                         lhsT=s_all[:, t * P:(t + 1) * P],
                                         rhs=I128b[:], is_transpose=True,
                                         start=True, stop=True)
                        st_t = sb8.tile([P, P], BF16, tag="st")
                        nc.vector.tensor_copy(out=st_t[:], in_=st_ps[:])
                        ad_ps = pp1.tile([P, 16], F32, space="PSUM", tag="adps")
                        nc.tensor.matmul(out=ad_ps[:], lhsT=st_t[:],
                                         rhs=ad_sb[:, w * 16:(w + 1) * 16],
                                         start=True, stop=True)
                        nc.scalar.activation(ad_all[:, t * 16:(t + 1) * 16],
                                             ad_ps[:],
                                             mybir.ActivationFunctionType.Identity)
                    gf = g_all[:].bitcast(F32)
                    nc.vector.tensor_tensor(
                        out=view(e_all[:], 0, [[H, tw], [1, H]]),
                        in0=view(gf, 32, [[40, tw], [1, H]]),
                        in1=view(ad_all[:], 0, [[16, tw], [1, H]]),
                        op=mybir.AluOpType.add)
                    nc.vector.tensor_tensor(
                        out=view(e_all[:], 0, [[H, tw], [1, H]]),
                        in0=view(e_all[:], 0, [[H, tw], [1, H]]),
                        in1=view(ad_all[:], 8, [[16, tw], [1, H]]),
                        op=mybir.AluOpType.add)
                    nc.vector.tensor_scalar_mul(out=lr_all[:, :tw * H],
                                                in0=e_all[:, :tw * H], scalar1=NEG)
                    nc.vector.tensor_tensor(out=lr_all[:, :tw * H],
                                            in0=lr_all[:, :tw * H],
                                            in1=e_all[:, :tw * H],
                                            op=mybir.AluOpType.max)
                    nc.scalar.activation(p_all[:, :tw * H], lr_all[:, :tw * H],
                                         mybir.ActivationFunctionType.Exp)
                    for t in range(tw):
                        w_t = sb8.tile([P, H * FV], BF16, tag="wv")
                        gh = g_all[:, t * TCOL:t * TCOL + D1].bitcast(BF16)
                        if layer == 1:
                            in0 = gh.rearrange("p (h f) -> p h f", h=H)
                        else:
                            in0 = mid_bcast(gh, H)
                        nc.vector.tensor_tensor(
                            out=w_t[:].rearrange("p (h f) -> p h f", h=H),
                            in0=in0,
                            in1=p_all[:, t * H:(t + 1) * H].to_broadcast([P, H, FV]),
                            op=mybir.AluOpType.mult)
                        nc.tensor.matmul(out=U_ps[:],
                                         lhsT=s_all[:, t * P:(t + 1) * P],
                                         rhs=w_t[:], start=(t == 0),
                                         stop=(t == tw - 1))
                        nc.tensor.matmul(out=den_ps[:],
                                         lhsT=s_all[:, t * P:(t + 1) * P],
                                         rhs=p_all[:, t * H:(t + 1) * H],
                                         start=(t == 0), stop=(t == tw - 1))
                    dse = sb.tile([P, H], F32, tag="dse")
                    nc.vector.tensor_scalar_add(out=dse[:], in0=den_ps[:], scalar1=1e-30)
                    rd = sb.tile([P, H], F32, tag="rd")
                    nc.vector.reciprocal(out=rd[:], in_=dse[:])
                    if layer == 1:
                        h2a = sb.tile([P, D1], F32, tag="h2a")
                        nc.vector.tensor_tensor(
                            out=h2a[:].rearrange("p (h f) -> p h f", h=H),
                            in0=U_ps[:].rearrange("p (h f) -> p h f", h=H),
                            in1=rd[:].to_broadcast([P, H, F1]),
                            op=mybir.AluOpType.mult)
                        nc.vector.tensor_tensor(out=h2a[:], in0=h2a[:], in1=b1rep[:],
                                                op=mybir.AluOpType.add)
                        ex = sb.tile([P, D1], F32, tag="ex")
                        nc.scalar.activation(ex[:], h2a[:],
                                             mybir.ActivationFunctionType.Exp)
                        exm = sb.tile([P, D1], F32, tag="exm")
                        nc.vector.tensor_scalar(out=exm[:], in0=ex[:], scalar1=1.0,
                                                scalar2=-1.0, op0=mybir.AluOpType.min,
                                                op1=mybir.AluOpType.add)
                        rl = sb.tile([P, D1], F32, tag="rl")
                        nc.vector.tensor_scalar_max(out=rl[:], in0=h2a[:], scalar1=0.0)
                        h2e = sb.tile([P, D1], F32, tag="h2e")
                        nc.vector.tensor_tensor(out=h2e[:], in0=exm[:], in1=rl[:],
                                                op=mybir.AluOpType.add)
                        h2eb = sb.tile([P, D1], BF16, tag="h2eb")
                        nc.vector.tensor_copy(out=h2eb[:], in_=h2e[:])
                        nc.sync.dma_start(
                            out=t2loc[w * P:(w + 1) * P, 0:D1].bitcast(BF16),
                            in_=h2eb[:])
                        hT_ps = pp1.tile([D1, P], F32, space="PSUM", tag="t1")
                        nc.tensor.matmul(out=hT_ps[:], lhsT=h2e[:], rhs=I128[:],
                                         is_transpose=True, start=True, stop=True)
                        hT = sb.tile([D1, P], F32, tag="hT")
                        nc.vector.tensor_copy(out=hT[:], in_=hT_ps[:])
                        a2T_ps = pp1.tile([H, P], F32, space="PSUM", tag="t2")
                        nc.tensor.matmul(out=a2T_ps[:], lhsT=As2[:], rhs=hT[:],
                                         start=True, stop=True)
                        d2T_ps = pp1.tile([H, P], F32, space="PSUM", tag="t3")
                        nc.tensor.matmul(out=d2T_ps[:], lhsT=Ad2[:], rhs=hT[:],
                                         start=True, stop=True)
                        a2T = sb.tile([H, P], F32, tag="a2T")
                        d2T = sb.tile([H, P], F32, tag="d2T")
                        nc.vector.tensor_copy(out=a2T[:], in_=a2T_ps[:])
                        nc.vector.tensor_copy(out=d2T[:], in_=d2T_ps[:])
                        a2nm_ps = pp1.tile([P, H], F32, space="PSUM", tag="t1")
                        nc.tensor.matmul(out=a2nm_ps[:], lhsT=a2T[:], rhs=I8[:],
                                         is_transpose=True, start=True, stop=True)
                        a2nm = sb.tile([P, H], F32, tag="a2nm")
                        nc.vector.tensor_copy(out=a2nm[:], in_=a2nm_ps[:])
                        nc.sync.dma_start(
                            out=t2loc[w * P:(w + 1) * P, D1:TCOL].bitcast(F32),
                            in_=a2nm[:])
                        d2nm_ps = pp1.tile([P, H], F32, space="PSUM", tag="t2")
                        nc.tensor.matmul(out=d2nm_ps[:], lhsT=d2T[:], rhs=I8[:],
                                         is_transpose=True, start=True, stop=True)
                        d2nm = sb.tile([P, H], F32, tag="d2nm")
                        nc.vector.tensor_copy(out=d2nm[:], in_=d2nm_ps[:])
                        split_hilo(ad2_sb[:, w * 16:w * 16 + 8],
                                   ad2_sb[:, w * 16 + 8:w * 16 + 16], d2nm[:])
                    else:
                        U2n = sb.tile([P, H * F2], F32, tag="U2n")
                        nc.vector.tensor_tensor(
                            out=U2n[:].rearrange("p (h f) -> p h f", h=H),
                            in0=U_ps[:].rearrange("p (h f) -> p h f", h=H),
                            in1=rd[:].to_broadcast([P, H, F2]),
                            op=mybir.AluOpType.mult)
                        YT_ps = pp1.tile([F2, P], F32, space="PSUM", tag="t3")
                        for h in range(H):
                            uT_ps = pp1.tile([F2, P], F32, space="PSUM", tag="t1")
                            nc.tensor.matmul(out=uT_ps[:],
                                             lhsT=U2n[:, h * F2:(h + 1) * F2],
                                             rhs=I128[:], is_transpose=True,
                                             start=True, stop=True)
                            uT = sb.tile([F2, P], F32, tag="uTs")
                            nc.vector.tensor_copy(out=uT[:], in_=uT_ps[:])
                            nc.tensor.matmul(out=YT_ps[:],
                                             lhsT=W2f[:, h * F2:(h + 1) * F2],
                                             rhs=uT[:], start=(h == 0),
                                             stop=(h == H - 1))
                        Y = sb.tile([F2, P], F32, tag="Y")
                        nc.scalar.activation(Y[:], YT_ps[:],
                                             mybir.ActivationFunctionType.Identity,
                                             bias=b2col[:], scale=1.0 / H)
                        o_ps = pp1.tile([P, F2], F32, space="PSUM", tag="t2")
                        nc.tensor.matmul(out=o_ps[:], lhsT=Y[:], rhs=I64[:],
                                         is_transpose=True, start=True, stop=True)
                        ow = sb.tile([P, F2], F32, tag="ow")
                        nc.vector.tensor_copy(out=ow[:], in_=o_ps[:])
                        rows = min(P, SLICE - w * P)
                        nc.sync.dma_start(out=out_d[w * P:w * P + rows, :],
                                          in_=ow[:rows, :])

            edge_phase(t1full, ad1_sb, 1)
            nc.gpsimd.collective_compute(
                "AllGather", mybir.AluOpType.bypass,
                replica_groups=[list(range(NC))],
                ins=[t2loc[:, :].opt()],
                outs=[t2full[0:NC * SPAD, :].opt()],
            )
            edge_phase(t2full, ad2_sb, 2)

    nc.compile()
    return nc


def kernel(**inputs):
    global LAST_EXEC_NS
    _register_profile_hook()
    from concourse import bass_utils

    in_maps, meta = host_prep(inputs)
    nc = build(meta)
    trace = os.environ.get("GAT_TRACE", "1") == "1"
    try:
        res = bass_utils.run_bass_kernel_spmd(
            nc, in_maps, core_ids=list(range(NC)), trace=trace)
    except Exception:
        if not trace:
            raise
        res = bass_utils.run_bass_kernel_spmd(
            nc, in_maps, core_ids=list(range(NC)), trace=False)
    LAST_EXEC_NS = res.exec_time_ns
    SLICE = meta["SLICE"]
    out = np.empty((N, F2), np.float32)
    for c in range(NC):
        out[c * SLICE:(c + 1) * SLICE] = res.results[c]["out"]
    return out



# revision 8
# speedup vs baseline: 1.0661x; 1.0464x over previous
"""Self-contained GAT kernel for 8 TRN2 NeuronCores.

kernel(**inputs) takes the FULL unsharded inputs (as produced by
setup_inputs) and returns the FULL [100000, 64] float32 output.

Architecture (see module gat_kernel-style doc):
- nodes dst-partitioned across 8 cores; edges dst-sorted into 128-dst windows,
  128-edge tiles.
- per-node table rows [h bf16 x64 | a_src f32 x8] packed as uint16[80];
  per-tile [128,1]-offset indirect-DMA gather.
- segment softmax/sums via one-hot selection matrices + PE matmuls; a_dst
  expanded per edge via DMA-transposed one-hot (S^T) matmuls from SBUF
  tables (bf16 hi+lo split for f32 accuracy).
- layer 2 aggregates 64-dim h2 per head and applies W2 after aggregation;
  head-mean via PSUM-accumulated per-head matmuls.
- AllGather collectives replicate node tables between phases.
"""
import os
import sys
import types

import numpy as np

sys.path.insert(0, "/opt/trn_rl_repo")

import ml_dtypes

import concourse.bass as bass
import concourse.bacc as bacc
import concourse.mybir as mybir
import concourse.tile as tile

BF16 = mybir.dt.bfloat16
F32 = mybir.dt.float32
I32 = mybir.dt.int32
U16 = mybir.dt.uint16

P = 128
H = 8
F1 = 8
F2 = 64
D1 = H * F1
IN_DIM = 256
NEG = 0.2
GHOST_AS = -300.0
TCOL = 80
NC = 8
N = 100000

LAST_EXEC_NS = None

_hook_registered = [False]


def _register_profile_hook():
    if _hook_registered[0]:
        return
    try:
        import antenv
        mod = types.ModuleType("antenv.axon_hooks")
        _h = [None]
        mod.set_axon_ntff_profile_hook = lambda f: _h.__setitem__(0, f)
        mod.get_axon_ntff_profile_hook = lambda: _h[0]
        sys.modules.setdefault("antenv.axon_hooks", mod)
        if not hasattr(antenv, "axon_hooks"):
            antenv.axon_hooks = mod
        from trn_agent_boot.trn_boot import _ntff_profile_via_ctypes
        sys.modules["antenv.axon_hooks"].set_axon_ntff_profile_hook(
            _ntff_profile_via_ctypes('/opt/axon/libaxon_pjrt.so'))
        _hook_registered[0] = True
    except Exception:
        pass


def mid_bcast(ap2d, reps):
    return bass.AP(ap2d.tensor, ap2d.offset, [ap2d.ap[0], [0, reps], ap2d.ap[1]])


def view(ap, off_elems, dims):
    """Custom strided view: dims = [[stride, count], ...] in ap-dtype elems."""
    return bass.AP(ap.tensor, ap.offset + off_elems, [ap.ap[0]] + dims)


def host_prep(inputs):
    SLICE = N // NC
    NW = (SLICE + P - 1) // P
    SPAD = NW * P
    GHOST = NC * SPAD

    edge = np.asarray(inputs["edge"])
    src = np.concatenate([np.asarray(edge[0]), np.arange(N, dtype=np.int64)])
    dst = np.concatenate([np.asarray(edge[1]), np.arange(N, dtype=np.int64)])

    core = (dst // SLICE).astype(np.int32)
    srcpad = ((src // SLICE) * SPAD + (src % SLICE)).astype(np.int32)
    dstl = (dst % SLICE).astype(np.int32)
    win = dstl // P

    counts = np.zeros((NC, NW), np.int64)
    for c in range(NC):
        m = core == c
        w, cnt = np.unique(win[m], return_counts=True)
        counts[c, w] = cnt
    T_w = np.maximum(1, (counts.max(axis=0) + P - 1) // P).astype(np.int64)
    T_tot = int(T_w.sum())
    col0 = np.concatenate([[0], np.cumsum(T_w)[:-1]])

    srcoff = np.full((NC, P, T_tot), GHOST, np.int32)
    dstrel = np.zeros((NC, P, T_tot), np.float32)
    order = np.argsort(core * np.int64(SLICE * 2) + dstl, kind="stable")
    s_s, d_s, c_s, w_s = srcpad[order], dstl[order], core[order], win[order]
    for c in range(NC):
        m = c_s == c
        sc, dc, wc = s_s[m], d_s[m], w_s[m]
        for w in range(NW):
            mw = wc == w
            k = int(mw.sum())
            tw = int(T_w[w])
            sl = np.full(tw * P, GHOST, np.int32)
            rl = np.zeros(tw * P, np.float32)
            sl[:k] = sc[mw]
            rl[:k] = (dc[mw] - w * P).astype(np.float32)
            cw = int(col0[w])
            srcoff[c, :, cw:cw + tw] = sl.reshape(tw, P).T
            dstrel[c, :, cw:cw + tw] = rl.reshape(tw, P).T

    grow = np.zeros(TCOL, np.uint16)
    grow[64:80] = np.full(8, GHOST_AS, np.float32).view(np.uint16)

    W1 = np.asarray(inputs["W1"], np.float32)
    a_src1 = np.asarray(inputs["a_src1"], np.float32)
    a_dst1 = np.asarray(inputs["a_dst1"], np.float32)
    b1 = np.asarray(inputs["b1"], np.float32)
    W2 = np.asarray(inputs["W2"], np.float32)
    a_src2 = np.asarray(inputs["a_src2"], np.float32)
    a_dst2 = np.asarray(inputs["a_dst2"], np.float32)
    b2 = np.asarray(inputs["b2"], np.float32)
    x = np.asarray(inputs["x"], np.float32)

    A1s = np.zeros((D1, H), np.float32)
    A1d = np.zeros((D1, H), np.float32)
    for h in range(H):
        A1s[h * F1:(h + 1) * F1, h] = a_src1[h]
        A1d[h * F1:(h + 1) * F1, h] = a_dst1[h]
    A2s = np.zeros((H * F2, H), np.float32)
    A2d = np.zeros((H * F2, H), np.float32)
    for h in range(H):
        A2s[h * F2:(h + 1) * F2, h] = a_src2[h]
        A2d[h * F2:(h + 1) * F2, h] = a_dst2[h]

    iotaC = np.broadcast_to(np.arange(P, dtype=np.float32), (P, P)).astype(ml_dtypes.bfloat16)

    shared = dict(
        W1b=W1.astype(ml_dtypes.bfloat16),
        A1s=A1s, A1d=A1d,
        W2Tb=np.ascontiguousarray(W2.T).astype(ml_dtypes.bfloat16),
        A2sb=A2s.astype(ml_dtypes.bfloat16), A2db=A2d.astype(ml_dtypes.bfloat16),
        W2f=W2,
        b1rep=np.broadcast_to(b1, (P, D1)).copy(),
        b2col=np.ascontiguousarray(b2.reshape(F2, 1)),
        iotaC=np.ascontiguousarray(iotaC),
        I128=np.eye(P, dtype=np.float32),
        I128b=np.eye(P, dtype=ml_dtypes.bfloat16),
        I64=np.eye(F2, dtype=np.float32),
        I8=np.eye(H, dtype=np.float32),
        ghostrow=grow.reshape(1, TCOL),
    )
    in_maps = []
    for c in range(NC):
        xs = np.zeros((SPAD, IN_DIM), np.float32)
        xs[:SLICE] = x[c * SLICE:(c + 1) * SLICE]
        m = dict(shared)
        m["xT"] = np.ascontiguousarray(xs.T)
        m["srcoff"] = np.ascontiguousarray(srcoff[c])
        m["dstrel"] = np.ascontiguousarray(dstrel[c]).astype(ml_dtypes.bfloat16)
        in_maps.append(m)

    meta = dict(SLICE=SLICE, NW=NW, SPAD=SPAD, GHOST=GHOST,
                T_w=[int(t) for t in T_w], col0=[int(cc) for cc in col0],
                T_tot=T_tot, NC=NC)
    return in_maps, meta


def build(meta):
    SLICE, NW, SPAD, GHOST, T_tot = (meta["SLICE"], meta["NW"], meta["SPAD"],
                                     meta["GHOST"], meta["T_tot"])
    T_w, col0 = meta["T_w"], meta["col0"]
    TMAX = max(T_w)
    CH = min(512, SPAD)
    n_chunks = (SPAD + CH - 1) // CH

    nc = bacc.Bacc('TRN2', num_devices=NC)
    xT = nc.dram_tensor("xT", [IN_DIM, SPAD], F32, kind="ExternalInput")
    srcoff = nc.dram_tensor("srcoff", [P, T_tot], I32, kind="ExternalInput")
    dstrel = nc.dram_tensor("dstrel", [P, T_tot], BF16, kind="ExternalInput")
    W1b_d = nc.dram_tensor("W1b", [IN_DIM, D1], BF16, kind="ExternalInput")
    A1s_d = nc.dram_tensor("A1s", [D1, H], F32, kind="ExternalInput")
    A1d_d = nc.dram_tensor("A1d", [D1, H], F32, kind="ExternalInput")
    W2Tb_d = nc.dram_tensor("W2Tb", [H * F2, F2], BF16, kind="ExternalInput")
    A2sb_d = nc.dram_tensor("A2sb", [H * F2, H], BF16, kind="ExternalInput")
    A2db_d = nc.dram_tensor("A2db", [H * F2, H], BF16, kind="ExternalInput")
    W2f_d = nc.dram_tensor("W2f", [F2, H * F2], F32, kind="ExternalInput")
    b1rep_d = nc.dram_tensor("b1rep", [P, D1], F32, kind="ExternalInput")
    b2col_d = nc.dram_tensor("b2col", [F2, 1], F32, kind="ExternalInput")
    iotaC_d = nc.dram_tensor("iotaC", [P, P], BF16, kind="ExternalInput")
    I128_d = nc.dram_tensor("I128", [P, P], F32, kind="ExternalInput")
    I128b_d = nc.dram_tensor("I128b", [P, P], BF16, kind="ExternalInput")
    I64_d = nc.dram_tensor("I64", [F2, F2], F32, kind="ExternalInput")
    I8_d = nc.dram_tensor("I8", [H, H], F32, kind="ExternalInput")
    ghostrow_d = nc.dram_tensor("ghostrow", [1, TCOL], U16, kind="ExternalInput")
    out_d = nc.dram_tensor("out", [SLICE, F2], F32, kind="ExternalOutput")
    t1loc = nc.dram_tensor("t1loc", [SPAD, TCOL], U16)
    t1full = nc.dram_tensor("t1full", [NC * SPAD + 1, TCOL], U16)
    t2loc = nc.dram_tensor("t2loc", [SPAD, TCOL], U16)
    t2full = nc.dram_tensor("t2full", [NC * SPAD + 1, TCOL], U16)

    with tile.TileContext(nc) as tc:
        with tc.tile_pool(name="consts", bufs=1) as cpool, \
             tc.tile_pool(name="sb", bufs=3) as sb, \
             tc.tile_pool(name="sb8", bufs=8) as sb8, \
             tc.tile_pool(name="gp", bufs=3) as gp, \
             tc.tile_pool(name="pp", bufs=2, space="PSUM") as pp, \
             tc.tile_pool(name="pp1", bufs=1, space="PSUM") as pp1:

            def cload(dram, shape, dtype, tag):
                t = cpool.tile(shape, dtype, tag=tag)
                nc.sync.dma_start(out=t[:], in_=dram[:, :])
                return t

            A1s = cload(A1s_d, [D1, H], F32, "cA1s")
            A1d = cload(A1d_d, [D1, H], F32, "cA1d")
            W2f = cload(W2f_d, [F2, H * F2], F32, "cW2f")
            b1rep = cload(b1rep_d, [P, D1], F32, "cb1")
            b2col = cload(b2col_d, [F2, 1], F32, "cb2")
            iotaC = cload(iotaC_d, [P, P], BF16, "ciota")
            I128 = cload(I128_d, [P, P], F32, "cI128")
            I128b = cload(I128b_d, [P, P], BF16, "cI128b")
            I64 = cload(I64_d, [F2, F2], F32, "cI64")
            I8 = cload(I8_d, [H, H], F32, "cI8")

            W1a = cpool.tile([P, D1], BF16, tag="W1a")
            W1c = cpool.tile([P, D1], BF16, tag="W1c")
            nc.sync.dma_start(out=W1a[:], in_=W1b_d[0:P, :])
            nc.sync.dma_start(out=W1c[:], in_=W1b_d[P:2 * P, :])

            As2 = cpool.tile([F2, H], F32, tag="As2")
            Ad2 = cpool.tile([F2, H], F32, tag="Ad2")
            As2_ps = pp1.tile([F2, H], F32, space="PSUM", tag="t1")
            Ad2_ps = pp1.tile([F2, H], F32, space="PSUM", tag="t2")
            nchk = (H * F2) // P
            w2t_ch, a2s_ch, a2d_ch = [], [], []
            for i in range(nchk):
                wt = cpool.tile([P, F2], BF16, tag=f"w2t{i}")
                as_ = cpool.tile([P, H], BF16, tag=f"a2s{i}")
                ad_ = cpool.tile([P, H], BF16, tag=f"a2d{i}")
                nc.sync.dma_start(out=wt[:], in_=W2Tb_d[i * P:(i + 1) * P, :])
                nc.sync.dma_start(out=as_[:], in_=A2sb_d[i * P:(i + 1) * P, :])
                nc.sync.dma_start(out=ad_[:], in_=A2db_d[i * P:(i + 1) * P, :])
                w2t_ch.append(wt); a2s_ch.append(as_); a2d_ch.append(ad_)
            for i in range(nchk):
                nc.tensor.matmul(out=As2_ps[:], lhsT=w2t_ch[i][:], rhs=a2s_ch[i][:],
                                 start=(i == 0), stop=(i == nchk - 1))
            for i in range(nchk):
                nc.tensor.matmul(out=Ad2_ps[:], lhsT=w2t_ch[i][:], rhs=a2d_ch[i][:],
                                 start=(i == 0), stop=(i == nchk - 1))
            nc.vector.tensor_copy(out=As2[:], in_=As2_ps[:])
            nc.vector.tensor_copy(out=Ad2[:], in_=Ad2_ps[:])

            grow_sb = cpool.tile([1, TCOL], U16, tag="grow")
            nc.sync.dma_start(out=grow_sb[:], in_=ghostrow_d[:, :])
            nc.sync.dma_start(out=t1full[GHOST:GHOST + 1, :], in_=grow_sb[:])
            nc.sync.dma_start(out=t2full[GHOST:GHOST + 1, :], in_=grow_sb[:])

            ad1_sb = cpool.tile([P, NW * 16], BF16, tag="ad1sb")
            ad2_sb = cpool.tile([P, NW * 16], BF16, tag="ad2sb")

            so_sb = cpool.tile([P, T_tot], I32, tag="sosb")
            dr_sb = cpool.tile([P, T_tot], BF16, tag="drsb")
            nc.sync.dma_start(out=so_sb[:], in_=srcoff[:, :])
            nc.sync.dma_start(out=dr_sb[:], in_=dstrel[:, :])

            def split_hilo(hi_ap, lo_ap, src_f32):
                nc.vector.tensor_copy(out=hi_ap, in_=src_f32)
                nc.vector.tensor_tensor(out=lo_ap, in0=src_f32, in1=hi_ap,
                                        op=mybir.AluOpType.subtract)

            for k in range(n_chunks):
                c0, c1 = k * CH, min((k + 1) * CH, SPAD)
                cw = c1 - c0
                w0 = c0 // P
                xa = sb.tile([P, CH], F32, tag="xa")
                xb = sb.tile([P, CH], F32, tag="xb")
                nc.sync.dma_start(out=xa[:, :cw], in_=xT[0:P, c0:c1])
                nc.sync.dma_start(out=xb[:, :cw], in_=xT[P:2 * P, c0:c1])
                xab = sb.tile([P, CH], BF16, tag="xab")
                xbb = sb.tile([P, CH], BF16, tag="xbb")
                nc.vector.tensor_copy(out=xab[:, :cw], in_=xa[:, :cw])
                nc.vector.tensor_copy(out=xbb[:, :cw], in_=xb[:, :cw])
                h1T_ps = pp.tile([D1, CH], F32, space="PSUM", tag="U")
                nc.tensor.matmul(out=h1T_ps[:, :cw], lhsT=W1a[:], rhs=xab[:, :cw],
                                 start=True, stop=False)
                nc.tensor.matmul(out=h1T_ps[:, :cw], lhsT=W1c[:], rhs=xbb[:, :cw],
                                 start=False, stop=True)
                h1T_f = sb.tile([D1, CH], F32, tag="h1Tf")
                h1T_b = sb.tile([D1, CH], BF16, tag="h1Tb")
                nc.vector.tensor_copy(out=h1T_f[:, :cw], in_=h1T_ps[:, :cw])
                nc.vector.tensor_copy(out=h1T_b[:, :cw], in_=h1T_ps[:, :cw])
                as1T_ps = pp1.tile([H, CH], F32, space="PSUM", tag="den")
                ad1T_ps = pp1.tile([H, CH], F32, space="PSUM", tag="adps")
                nc.tensor.matmul(out=as1T_ps[:, :cw], lhsT=A1s[:], rhs=h1T_f[:, :cw],
                                 start=True, stop=True)
                nc.tensor.matmul(out=ad1T_ps[:, :cw], lhsT=A1d[:], rhs=h1T_f[:, :cw],
                                 start=True, stop=True)
                as1T_f = sb.tile([H, CH], F32, tag="as1Tf")
                ad1T_f = sb.tile([H, CH], F32, tag="ad1Tf")
                nc.vector.tensor_copy(out=as1T_f[:, :cw], in_=as1T_ps[:, :cw])
                nc.vector.tensor_copy(out=ad1T_f[:, :cw], in_=ad1T_ps[:, :cw])
                for b in range(cw // P):
                    nn = c0 + b * P
                    w = w0 + b
                    hnm = sb.tile([P, D1], BF16, tag="hnm")
                    nc.sync.dma_start_transpose(hnm[:], h1T_b[:, b * P:(b + 1) * P])
                    nc.sync.dma_start(out=t1loc[nn:nn + P, 0:D1].bitcast(BF16),
                                      in_=hnm[:])
                    asT_ps = pp1.tile([P, H], F32, space="PSUM", tag="t1")
                    nc.tensor.matmul(out=asT_ps[:], lhsT=as1T_f[:, b * P:(b + 1) * P],
                                     rhs=I8[:], is_transpose=True, start=True, stop=True)
                    asnm = sb.tile([P, H], F32, tag="asnm")
                    nc.vector.tensor_copy(out=asnm[:], in_=asT_ps[:])
                    nc.sync.dma_start(out=t1loc[nn:nn + P, D1:TCOL].bitcast(F32),
                                      in_=asnm[:])
                    adT_ps = pp1.tile([P, H], F32, space="PSUM", tag="t2")
                    nc.tensor.matmul(out=adT_ps[:], lhsT=ad1T_f[:, b * P:(b + 1) * P],
                                     rhs=I8[:], is_transpose=True, start=True, stop=True)
                    adnm = sb.tile([P, H], F32, tag="adnm")
                    nc.vector.tensor_copy(out=adnm[:], in_=adT_ps[:])
                    split_hilo(ad1_sb[:, w * 16:w * 16 + 8],
                               ad1_sb[:, w * 16 + 8:w * 16 + 16], adnm[:])

            nc.gpsimd.collective_compute(
                "AllGather", mybir.AluOpType.bypass,
                replica_groups=[list(range(NC))],
                ins=[t1loc[:, :].opt()],
                outs=[t1full[0:NC * SPAD, :].opt()],
            )

            def edge_phase(tfull, ad_sb, layer):
                NCOLS = D1 if layer == 1 else H * F2
                FV = F1 if layer == 1 else F2
                for w in range(NW):
                    tw = T_w[w]
                    cwid = col0[w]
                    UCOLS = NCOLS + H if layer == 1 else NCOLS
                    U_ps = pp.tile([P, UCOLS], F32, space="PSUM", tag="U")
                    den_ps = None
                    if layer == 2:
                        den_ps = pp1.tile([P, H], F32, space="PSUM", tag="den",
                                          name="den_ps")
                    g_all = gp.tile([P, TMAX * TCOL], U16, tag="ga", bufs=6)
                    s_all = gp.tile([P, TMAX * P], BF16, tag="sa", bufs=4)
                    ad_all = gp.tile([P, TMAX * 16], F32, tag="ada")
                    e_all = gp.tile([P, TMAX * H], F32, tag="ea")
                    lr_all = gp.tile([P, TMAX * H], F32, tag="la")
                    p_all = gp.tile([P, TMAX * H], BF16, tag="pa")
                    for t in range(tw):
                        nc.gpsimd.indirect_dma_start(
                            out=g_all[:, t * TCOL:(t + 1) * TCOL], out_offset=None,
                            in_=tfull[:, :],
                            in_offset=bass.IndirectOffsetOnAxis(
                                ap=so_sb[:, cwid + t:cwid + t + 1], axis=0),
                        )
                    nc.vector.tensor_tensor(
                        out=view(s_all[:], 0, [[P, tw], [1, P]]),
                        in0=view(dr_sb[:], cwid, [[1, tw], [0, P]]),
                        in1=view(iotaC[:], 0, [[0, tw], [1, P]]),
                        op=mybir.AluOpType.is_equal)
                    for t in range(tw):
                        st_ps = pp1.tile([P, P], BF16, space="PSUM", tag="stp")
                        nc.tensor.matmul(out=st_ps[:],
                                         lhsT=s_all[:, t * P:(t + 1) * P],
                                         rhs=I128b[:], is_transpose=True,
                                         start=True, stop=True)
                        st_t = sb8.tile([P, P], BF16, tag="st")
                        nc.scalar.activation(st_t[:], st_ps[:],
                                             mybir.ActivationFunctionType.Identity)
                        ad_ps = pp1.tile([P, 16], F32, space="PSUM", tag="adps")
                        nc.tensor.matmul(out=ad_ps[:], lhsT=st_t[:],
                                         rhs=ad_sb[:, w * 16:(w + 1) * 16],
                                         start=True, stop=True)
                        nc.scalar.activation(ad_all[:, t * 16:(t + 1) * 16],
                                             ad_ps[:],
                                             mybir.ActivationFunctionType.Identity)
                    gf = g_all[:].bitcast(F32)
                    nc.vector.tensor_tensor(
                        out=view(e_all[:], 0, [[H, tw], [1, H]]),
                        in0=view(gf, 32, [[40, tw], [1, H]]),
                        in1=view(ad_all[:], 0, [[16, tw], [1, H]]),
                        op=mybir.AluOpType.add)
                    nc.vector.tensor_tensor(
                        out=view(e_all[:], 0, [[H, tw], [1, H]]),
                        in0=view(e_all[:], 0, [[H, tw], [1, H]]),
                        in1=view(ad_all[:], 8, [[16, tw], [1, H]]),
                        op=mybir.AluOpType.add)
                    nc.vector.tensor_scalar_mul(out=lr_all[:, :tw * H],
                                                in0=e_all[:, :tw * H], scalar1=NEG)
                    nc.vector.tensor_tensor(out=lr_all[:, :tw * H],
                                            in0=lr_all[:, :tw * H],
                                            in1=e_all[:, :tw * H],
                                            op=mybir.AluOpType.max)
                    nc.scalar.activation(p_all[:, :tw * H], lr_all[:, :tw * H],
                                         mybir.ActivationFunctionType.Exp)
                    for t in range(tw):
                        w_t = sb8.tile([P, UCOLS], BF16, tag="wv")
                        gh = g_all[:, t * TCOL:t * TCOL + D1].bitcast(BF16)
                        if layer == 1:
                            in0 = gh.rearrange("p (h f) -> p h f", h=H)
                        else:
                            in0 = mid_bcast(gh, H)
                        nc.vector.tensor_tensor(
                            out=w_t[:, :H * FV].rearrange("p (h f) -> p h f", h=H),
                            in0=in0,
                            in1=p_all[:, t * H:(t + 1) * H].to_broadcast([P, H, FV]),
                            op=mybir.AluOpType.mult)
                        if layer == 1:
                            nc.scalar.activation(
                                w_t[:, H * FV:UCOLS],
                                p_all[:, t * H:(t + 1) * H],
                                mybir.ActivationFunctionType.Identity)
                        nc.tensor.matmul(out=U_ps[:],
                                         lhsT=s_all[:, t * P:(t + 1) * P],
                                         rhs=w_t[:], start=(t == 0),
                                         stop=(t == tw - 1))
                        if layer == 2:
                            nc.tensor.matmul(out=den_ps[:],
                                             lhsT=s_all[:, t * P:(t + 1) * P],
                                             rhs=p_all[:, t * H:(t + 1) * H],
                                             start=(t == 0), stop=(t == tw - 1))
                    den_src = (U_ps[:, H * FV:UCOLS] if layer == 1 else den_ps[:])
                    dse = sb.tile([P, H], F32, tag="dse")
                    nc.vector.tensor_scalar_add(out=dse[:], in0=den_src, scalar1=1e-30)
                    rd = sb.tile([P, H], F32, tag="rd")
                    nc.vector.reciprocal(out=rd[:], in_=dse[:])
                    if layer == 1:
                        h2a = sb.tile([P, D1], F32, tag="h2a")
                        nc.vector.tensor_tensor(
                            out=h2a[:].rearrange("p (h f) -> p h f", h=H),
                            in0=U_ps[:, 0:D1].rearrange("p (h f) -> p h f", h=H),
                            in1=rd[:].to_broadcast([P, H, F1]),
                            op=mybir.AluOpType.mult)
                        nc.vector.tensor_tensor(out=h2a[:], in0=h2a[:], in1=b1rep[:],
                                                op=mybir.AluOpType.add)
                        ex = sb.tile([P, D1], F32, tag="ex")
                        nc.scalar.activation(ex[:], h2a[:],
                                             mybir.ActivationFunctionType.Exp)
                        exm = sb.tile([P, D1], F32, tag="exm")
                        nc.vector.tensor_scalar(out=exm[:], in0=ex[:], scalar1=1.0,
                                                scalar2=-1.0, op0=mybir.AluOpType.min,
                                                op1=mybir.AluOpType.add)
                        rl = sb.tile([P, D1], F32, tag="rl")
                        nc.vector.tensor_scalar_max(out=rl[:], in0=h2a[:], scalar1=0.0)
                        h2e = sb.tile([P, D1], F32, tag="h2e")
                        nc.vector.tensor_tensor(out=h2e[:], in0=exm[:], in1=rl[:],
                                                op=mybir.AluOpType.add)
                        h2eb = sb.tile([P, D1], BF16, tag="h2eb")
                        nc.vector.tensor_copy(out=h2eb[:], in_=h2e[:])
                        nc.sync.dma_start(
                            out=t2loc[w * P:(w + 1) * P, 0:D1].bitcast(BF16),
                            in_=h2eb[:])
                        hT_ps = pp1.tile([D1, P], F32, space="PSUM", tag="t1")
                        nc.tensor.matmul(out=hT_ps[:], lhsT=h2e[:], rhs=I128[:],
                                         is_transpose=True, start=True, stop=True)
                        hT = sb.tile([D1, P], F32, tag="hT")
                        nc.vector.tensor_copy(out=hT[:], in_=hT_ps[:])
                        a2T_ps = pp1.tile([H, P], F32, space="PSUM", tag="t2")
                        nc.tensor.matmul(out=a2T_ps[:], lhsT=As2[:], rhs=hT[:],
                                         start=True, stop=True)
                        d2T_ps = pp1.tile([H, P], F32, space="PSUM", tag="t3")
                        nc.tensor.matmul(out=d2T_ps[:], lhsT=Ad2[:], rhs=hT[:],
                                         start=True, stop=True)
                        a2T = sb.tile([H, P], F32, tag="a2T")
                        d2T = sb.tile([H, P], F32, tag="d2T")
                        nc.vector.tensor_copy(out=a2T[:], in_=a2T_ps[:])
                        nc.vector.tensor_copy(out=d2T[:], in_=d2T_ps[:])
                        a2nm_ps = pp1.tile([P, H], F32, space="PSUM", tag="t1")
                        nc.tensor.matmul(out=a2nm_ps[:], lhsT=a2T[:], rhs=I8[:],
                                         is_transpose=True, start=True, stop=True)
                        a2nm = sb.tile([P, H], F32, tag="a2nm")
                        nc.vector.tensor_copy(out=a2nm[:], in_=a2nm_ps[:])
                        nc.sync.dma_start(
                            out=t2loc[w * P:(w + 1) * P, D1:TCOL].bitcast(F32),
                            in_=a2nm[:])
                        d2nm_ps = pp1.tile([P, H], F32, space="PSUM", tag="t2")
                        nc.tensor.matmul(out=d2nm_ps[:], lhsT=d2T[:], rhs=I8[:],
                                         is_transpose=True, start=True, stop=True)
                        d2nm = sb.tile([P, H], F32, tag="d2nm")
                        nc.vector.tensor_copy(out=d2nm[:], in_=d2nm_ps[:])
                        split_hilo(ad2_sb[:, w * 16:w * 16 + 8],
                                   ad2_sb[:, w * 16 + 8:w * 16 + 16], d2nm[:])
                    else:
                        U2n = sb.tile([P, H * F2], F32, tag="U2n")
                        nc.vector.tensor_tensor(
                            out=U2n[:].rearrange("p (h f) -> p h f", h=H),
                            in0=U_ps[:].rearrange("p (h f) -> p h f", h=H),
                            in1=rd[:].to_broadcast([P, H, F2]),
                            op=mybir.AluOpType.mult)
                        YT_ps = pp1.tile([F2, P], F32, space="PSUM", tag="t3")
                        for h in range(H):
                            uT_ps = pp1.tile([F2, P], F32, space="PSUM", tag="t1")
                            nc.tensor.matmul(out=uT_ps[:],
                                             lhsT=U2n[:, h * F2:(h + 1) * F2],
                                             rhs=I128[:], is_transpose=True,
                                             start=True, stop=True)
                            uT = sb.tile([F2, P], F32, tag="uTs")
                            nc.vector.tensor_copy(out=uT[:], in_=uT_ps[:])
                            nc.tensor.matmul(out=YT_ps[:],
                                             lhsT=W2f[:, h * F2:(h + 1) * F2],
                                             rhs=uT[:], start=(h == 0),
                                             stop=(h == H - 1))
                        Y = sb.tile([F2, P], F32, tag="Y")
                        nc.scalar.activation(Y[:], YT_ps[:],
                                             mybir.ActivationFunctionType.Identity,
                                             bias=b2col[:], scale=1.0 / H)
                        o_ps = pp1.tile([P, F2], F32, space="PSUM", tag="t2")
                        nc.tensor.matmul(out=o_ps[:], lhsT=Y[:], rhs=I64[:],
                                         is_transpose=True, start=True, stop=True)
                        ow = sb.tile([P, F2], F32, tag="ow")
                        nc.vector.tensor_copy(out=ow[:], in_=o_ps[:])
                        rows = min(P, SLICE - w * P)
                        nc.sync.dma_start(out=out_d[w * P:w * P + rows, :],
                                          in_=ow[:rows, :])

            edge_phase(t1full, ad1_sb, 1)
            nc.gpsimd.collective_compute(
                "AllGather", mybir.AluOpType.bypass,
                replica_groups=[list(range(NC))],
                ins=[t2loc[:, :].opt()],
                outs=[t2full[0:NC * SPAD, :].opt()],
            )
            edge_phase(t2full, ad2_sb, 2)

    nc.compile()
    return nc


def kernel(**inputs):
    global LAST_EXEC_NS
    _register_profile_hook()
    from concourse import bass_utils

    in_maps, meta = host_prep(inputs)
    nc = build(meta)
    trace = os.environ.get("GAT_TRACE", "1") == "1"
    try:
        res = bass_utils.run_bass_kernel_spmd(
            nc, in_maps, core_ids=list(range(NC)), trace=trace)
    except Exception:
        if not trace:
            raise
        res = bass_utils.run_bass_kernel_spmd(
            nc, in_maps, core_ids=list(range(NC)), trace=False)
    LAST_EXEC_NS = res.exec_time_ns
    SLICE = meta["SLICE"]
    out = np.empty((N, F2), np.float32)
    for c in range(NC):
        out[c * SLICE:(c + 1) * SLICE] = res.results[c]["out"]
    return out



# revision 9
# speedup vs baseline: 1.0786x; 1.0118x over previous
"""Self-contained GAT kernel for 8 TRN2 NeuronCores.

kernel(**inputs) takes the FULL unsharded inputs (as produced by
setup_inputs) and returns the FULL [100000, 64] float32 output.

Architecture (see module gat_kernel-style doc):
- nodes dst-partitioned across 8 cores; edges dst-sorted into 128-dst windows,
  128-edge tiles.
- per-node table rows [h bf16 x64 | a_src f32 x8] packed as uint16[80];
  per-tile [128,1]-offset indirect-DMA gather.
- segment softmax/sums via one-hot selection matrices + PE matmuls; a_dst
  expanded per edge via DMA-transposed one-hot (S^T) matmuls from SBUF
  tables (bf16 hi+lo split for f32 accuracy).
- layer 2 aggregates 64-dim h2 per head and applies W2 after aggregation;
  head-mean via PSUM-accumulated per-head matmuls.
- AllGather collectives replicate node tables between phases.
"""
import os
import sys
import types

import numpy as np

sys.path.insert(0, "/opt/trn_rl_repo")

import ml_dtypes

import concourse.bass as bass
import concourse.bacc as bacc
import concourse.mybir as mybir
import concourse.tile as tile

BF16 = mybir.dt.bfloat16
F32 = mybir.dt.float32
I32 = mybir.dt.int32
U16 = mybir.dt.uint16

P = 128
H = 8
F1 = 8
F2 = 64
D1 = H * F1
IN_DIM = 256
NEG = 0.2
GHOST_AS = -300.0
TCOL = 80
NC = 8
N = 100000

LAST_EXEC_NS = None

_hook_registered = [False]


def _register_profile_hook():
    if _hook_registered[0]:
        return
    try:
        import antenv
        mod = types.ModuleType("antenv.axon_hooks")
        _h = [None]
        mod.set_axon_ntff_profile_hook = lambda f: _h.__setitem__(0, f)
        mod.get_axon_ntff_profile_hook = lambda: _h[0]
        sys.modules.setdefault("antenv.axon_hooks", mod)
        if not hasattr(antenv, "axon_hooks"):
            antenv.axon_hooks = mod
        from trn_agent_boot.trn_boot import _ntff_profile_via_ctypes
        sys.modules["antenv.axon_hooks"].set_axon_ntff_profile_hook(
            _ntff_profile_via_ctypes('/opt/axon/libaxon_pjrt.so'))
        _hook_registered[0] = True
    except Exception:
        pass


def mid_bcast(ap2d, reps):
    return bass.AP(ap2d.tensor, ap2d.offset, [ap2d.ap[0], [0, reps], ap2d.ap[1]])


def view(ap, off_elems, dims):
    """Custom strided view: dims = [[stride, count], ...] in ap-dtype elems."""
    return bass.AP(ap.tensor, ap.offset + off_elems, [ap.ap[0]] + dims)


def host_prep(inputs):
    SLICE = N // NC
    NW = (SLICE + P - 1) // P
    SPAD = NW * P
    GHOST = NC * SPAD

    edge = np.asarray(inputs["edge"])
    src = np.concatenate([np.asarray(edge[0]), np.arange(N, dtype=np.int64)])
    dst = np.concatenate([np.asarray(edge[1]), np.arange(N, dtype=np.int64)])

    core = (dst // SLICE).astype(np.int32)
    srcpad = ((src // SLICE) * SPAD + (src % SLICE)).astype(np.int32)
    dstl = (dst % SLICE).astype(np.int32)
    win = dstl // P

    counts = np.zeros((NC, NW), np.int64)
    for c in range(NC):
        m = core == c
        w, cnt = np.unique(win[m], return_counts=True)
        counts[c, w] = cnt
    T_w = np.maximum(1, (counts.max(axis=0) + P - 1) // P).astype(np.int64)
    T_tot = int(T_w.sum())
    col0 = np.concatenate([[0], np.cumsum(T_w)[:-1]])

    srcoff = np.full((NC, P, T_tot), GHOST, np.int32)
    dstrel = np.zeros((NC, P, T_tot), np.float32)
    order = np.argsort(core * np.int64(SLICE * 2) + dstl, kind="stable")
    s_s, d_s, c_s, w_s = srcpad[order], dstl[order], core[order], win[order]
    for c in range(NC):
        m = c_s == c
        sc, dc, wc = s_s[m], d_s[m], w_s[m]
        for w in range(NW):
            mw = wc == w
            k = int(mw.sum())
            tw = int(T_w[w])
            sl = np.full(tw * P, GHOST, np.int32)
            rl = np.zeros(tw * P, np.float32)
            sl[:k] = sc[mw]
            rl[:k] = (dc[mw] - w * P).astype(np.float32)
            cw = int(col0[w])
            srcoff[c, :, cw:cw + tw] = sl.reshape(tw, P).T
            dstrel[c, :, cw:cw + tw] = rl.reshape(tw, P).T

    grow = np.zeros(TCOL, np.uint16)
    grow[64:80] = np.full(8, GHOST_AS, np.float32).view(np.uint16)

    W1 = np.asarray(inputs["W1"], np.float32)
    a_src1 = np.asarray(inputs["a_src1"], np.float32)
    a_dst1 = np.asarray(inputs["a_dst1"], np.float32)
    b1 = np.asarray(inputs["b1"], np.float32)
    W2 = np.asarray(inputs["W2"], np.float32)
    a_src2 = np.asarray(inputs["a_src2"], np.float32)
    a_dst2 = np.asarray(inputs["a_dst2"], np.float32)
    b2 = np.asarray(inputs["b2"], np.float32)
    x = np.asarray(inputs["x"], np.float32)

    A1s = np.zeros((D1, H), np.float32)
    A1d = np.zeros((D1, H), np.float32)
    for h in range(H):
        A1s[h * F1:(h + 1) * F1, h] = a_src1[h]
        A1d[h * F1:(h + 1) * F1, h] = a_dst1[h]
    A2s = np.zeros((H * F2, H), np.float32)
    A2d = np.zeros((H * F2, H), np.float32)
    for h in range(H):
        A2s[h * F2:(h + 1) * F2, h] = a_src2[h]
        A2d[h * F2:(h + 1) * F2, h] = a_dst2[h]

    iotaC = np.broadcast_to(np.arange(P, dtype=np.float32), (P, P)).astype(ml_dtypes.bfloat16)

    shared = dict(
        W1b=W1.astype(ml_dtypes.bfloat16),
        A1s=A1s, A1d=A1d,
        W2Tb=np.ascontiguousarray(W2.T).astype(ml_dtypes.bfloat16),
        A2sb=A2s.astype(ml_dtypes.bfloat16), A2db=A2d.astype(ml_dtypes.bfloat16),
        W2f=W2,
        b1rep=np.broadcast_to(b1, (P, D1)).copy(),
        b2col=np.ascontiguousarray(b2.reshape(F2, 1)),
        iotaC=np.ascontiguousarray(iotaC),
        I128=np.eye(P, dtype=np.float32),
        I128b=np.eye(P, dtype=ml_dtypes.bfloat16),
        I64b=np.eye(F2, dtype=ml_dtypes.bfloat16),
        I64=np.eye(F2, dtype=np.float32),
        I8=np.eye(H, dtype=np.float32),
        ghostrow=grow.reshape(1, TCOL),
    )
    in_maps = []
    for c in range(NC):
        xs = np.zeros((SPAD, IN_DIM), np.float32)
        xs[:SLICE] = x[c * SLICE:(c + 1) * SLICE]
        m = dict(shared)
        m["xT"] = np.ascontiguousarray(xs.T)
        m["srcoff"] = np.ascontiguousarray(srcoff[c])
        m["dstrel"] = np.ascontiguousarray(dstrel[c]).astype(ml_dtypes.bfloat16)
        in_maps.append(m)

    meta = dict(SLICE=SLICE, NW=NW, SPAD=SPAD, GHOST=GHOST,
                T_w=[int(t) for t in T_w], col0=[int(cc) for cc in col0],
                T_tot=T_tot, NC=NC)
    return in_maps, meta


def build(meta):
    SLICE, NW, SPAD, GHOST, T_tot = (meta["SLICE"], meta["NW"], meta["SPAD"],
                                     meta["GHOST"], meta["T_tot"])
    T_w, col0 = meta["T_w"], meta["col0"]
    TMAX = max(T_w)
    CH = min(512, SPAD)
    n_chunks = (SPAD + CH - 1) // CH

    nc = bacc.Bacc('TRN2', num_devices=NC)
    xT = nc.dram_tensor("xT", [IN_DIM, SPAD], F32, kind="ExternalInput")
    srcoff = nc.dram_tensor("srcoff", [P, T_tot], I32, kind="ExternalInput")
    dstrel = nc.dram_tensor("dstrel", [P, T_tot], BF16, kind="ExternalInput")
    W1b_d = nc.dram_tensor("W1b", [IN_DIM, D1], BF16, kind="ExternalInput")
    A1s_d = nc.dram_tensor("A1s", [D1, H], F32, kind="ExternalInput")
    A1d_d = nc.dram_tensor("A1d", [D1, H], F32, kind="ExternalInput")
    W2Tb_d = nc.dram_tensor("W2Tb", [H * F2, F2], BF16, kind="ExternalInput")
    A2sb_d = nc.dram_tensor("A2sb", [H * F2, H], BF16, kind="ExternalInput")
    A2db_d = nc.dram_tensor("A2db", [H * F2, H], BF16, kind="ExternalInput")
    W2f_d = nc.dram_tensor("W2f", [F2, H * F2], F32, kind="ExternalInput")
    b1rep_d = nc.dram_tensor("b1rep", [P, D1], F32, kind="ExternalInput")
    b2col_d = nc.dram_tensor("b2col", [F2, 1], F32, kind="ExternalInput")
    iotaC_d = nc.dram_tensor("iotaC", [P, P], BF16, kind="ExternalInput")
    I128_d = nc.dram_tensor("I128", [P, P], F32, kind="ExternalInput")
    I128b_d = nc.dram_tensor("I128b", [P, P], BF16, kind="ExternalInput")
    I64b_d = nc.dram_tensor("I64b", [F2, F2], BF16, kind="ExternalInput")
    I64_d = nc.dram_tensor("I64", [F2, F2], F32, kind="ExternalInput")
    I8_d = nc.dram_tensor("I8", [H, H], F32, kind="ExternalInput")
    ghostrow_d = nc.dram_tensor("ghostrow", [1, TCOL], U16, kind="ExternalInput")
    out_d = nc.dram_tensor("out", [SLICE, F2], F32, kind="ExternalOutput")
    t1loc = nc.dram_tensor("t1loc", [SPAD, TCOL], U16)
    t1full = nc.dram_tensor("t1full", [NC * SPAD + 1, TCOL], U16)
    t2loc = nc.dram_tensor("t2loc", [SPAD, TCOL], U16)
    t2full = nc.dram_tensor("t2full", [NC * SPAD + 1, TCOL], U16)

    with tile.TileContext(nc) as tc:
        with tc.tile_pool(name="consts", bufs=1) as cpool, \
             tc.tile_pool(name="sb", bufs=3) as sb, \
             tc.tile_pool(name="sb8", bufs=8) as sb8, \
             tc.tile_pool(name="gp", bufs=3) as gp, \
             tc.tile_pool(name="pp", bufs=2, space="PSUM") as pp, \
             tc.tile_pool(name="pp1", bufs=1, space="PSUM") as pp1:

            def cload(dram, shape, dtype, tag):
                t = cpool.tile(shape, dtype, tag=tag)
                nc.sync.dma_start(out=t[:], in_=dram[:, :])
                return t

            A1s = cload(A1s_d, [D1, H], F32, "cA1s")
            A1d = cload(A1d_d, [D1, H], F32, "cA1d")
            W2f = cload(W2f_d, [F2, H * F2], F32, "cW2f")
            b1rep = cload(b1rep_d, [P, D1], F32, "cb1")
            b2col = cload(b2col_d, [F2, 1], F32, "cb2")
            iotaC = cload(iotaC_d, [P, P], BF16, "ciota")
            I128 = cload(I128_d, [P, P], F32, "cI128")
            I128b = cload(I128b_d, [P, P], BF16, "cI128b")
            I64b = cload(I64b_d, [F2, F2], BF16, "cI64b")
            I64 = cload(I64_d, [F2, F2], F32, "cI64")
            I8 = cload(I8_d, [H, H], F32, "cI8")

            W1a = cpool.tile([P, D1], BF16, tag="W1a")
            W1c = cpool.tile([P, D1], BF16, tag="W1c")
            nc.sync.dma_start(out=W1a[:], in_=W1b_d[0:P, :])
            nc.sync.dma_start(out=W1c[:], in_=W1b_d[P:2 * P, :])

            As2 = cpool.tile([F2, H], F32, tag="As2")
            Ad2 = cpool.tile([F2, H], F32, tag="Ad2")
            As2_ps = pp1.tile([F2, H], F32, space="PSUM", tag="t1")
            Ad2_ps = pp1.tile([F2, H], F32, space="PSUM", tag="t2")
            nchk = (H * F2) // P
            w2t_ch, a2s_ch, a2d_ch = [], [], []
            for i in range(nchk):
                wt = cpool.tile([P, F2], BF16, tag=f"w2t{i}")
                as_ = cpool.tile([P, H], BF16, tag=f"a2s{i}")
                ad_ = cpool.tile([P, H], BF16, tag=f"a2d{i}")
                nc.sync.dma_start(out=wt[:], in_=W2Tb_d[i * P:(i + 1) * P, :])
                nc.sync.dma_start(out=as_[:], in_=A2sb_d[i * P:(i + 1) * P, :])
                nc.sync.dma_start(out=ad_[:], in_=A2db_d[i * P:(i + 1) * P, :])
                w2t_ch.append(wt); a2s_ch.append(as_); a2d_ch.append(ad_)
            for i in range(nchk):
                nc.tensor.matmul(out=As2_ps[:], lhsT=w2t_ch[i][:], rhs=a2s_ch[i][:],
                                 start=(i == 0), stop=(i == nchk - 1))
            for i in range(nchk):
                nc.tensor.matmul(out=Ad2_ps[:], lhsT=w2t_ch[i][:], rhs=a2d_ch[i][:],
                                 start=(i == 0), stop=(i == nchk - 1))
            nc.vector.tensor_copy(out=As2[:], in_=As2_ps[:])
            nc.vector.tensor_copy(out=Ad2[:], in_=Ad2_ps[:])

            grow_sb = cpool.tile([1, TCOL], U16, tag="grow")
            nc.sync.dma_start(out=grow_sb[:], in_=ghostrow_d[:, :])
            nc.sync.dma_start(out=t1full[GHOST:GHOST + 1, :], in_=grow_sb[:])
            nc.sync.dma_start(out=t2full[GHOST:GHOST + 1, :], in_=grow_sb[:])

            ad1_sb = cpool.tile([P, NW * 16], BF16, tag="ad1sb")
            ad2_sb = cpool.tile([P, NW * 16], BF16, tag="ad2sb")

            so_sb = cpool.tile([P, T_tot], I32, tag="sosb")
            dr_sb = cpool.tile([P, T_tot], BF16, tag="drsb")
            nc.sync.dma_start(out=so_sb[:], in_=srcoff[:, :])
            nc.sync.dma_start(out=dr_sb[:], in_=dstrel[:, :])

            def split_hilo(hi_ap, lo_ap, src_f32):
                nc.vector.tensor_copy(out=hi_ap, in_=src_f32)
                nc.vector.tensor_tensor(out=lo_ap, in0=src_f32, in1=hi_ap,
                                        op=mybir.AluOpType.subtract)

            for k in range(n_chunks):
                c0, c1 = k * CH, min((k + 1) * CH, SPAD)
                cw = c1 - c0
                w0 = c0 // P
                xa = sb.tile([P, CH], F32, tag="xa")
                xb = sb.tile([P, CH], F32, tag="xb")
                nc.sync.dma_start(out=xa[:, :cw], in_=xT[0:P, c0:c1])
                nc.sync.dma_start(out=xb[:, :cw], in_=xT[P:2 * P, c0:c1])
                xab = sb.tile([P, CH], BF16, tag="xab")
                xbb = sb.tile([P, CH], BF16, tag="xbb")
                nc.vector.tensor_copy(out=xab[:, :cw], in_=xa[:, :cw])
                nc.vector.tensor_copy(out=xbb[:, :cw], in_=xb[:, :cw])
                h1T_ps = pp.tile([D1, CH], F32, space="PSUM", tag="U")
                nc.tensor.matmul(out=h1T_ps[:, :cw], lhsT=W1a[:], rhs=xab[:, :cw],
                                 start=True, stop=False)
                nc.tensor.matmul(out=h1T_ps[:, :cw], lhsT=W1c[:], rhs=xbb[:, :cw],
                                 start=False, stop=True)
                h1T_f = sb.tile([D1, CH], F32, tag="h1Tf")
                h1T_b = sb.tile([D1, CH], BF16, tag="h1Tb")
                nc.vector.tensor_copy(out=h1T_f[:, :cw], in_=h1T_ps[:, :cw])
                nc.vector.tensor_copy(out=h1T_b[:, :cw], in_=h1T_ps[:, :cw])
                as1T_ps = pp1.tile([H, CH], F32, space="PSUM", tag="den")
                ad1T_ps = pp1.tile([H, CH], F32, space="PSUM", tag="adps")
                nc.tensor.matmul(out=as1T_ps[:, :cw], lhsT=A1s[:], rhs=h1T_f[:, :cw],
                                 start=True, stop=True)
                nc.tensor.matmul(out=ad1T_ps[:, :cw], lhsT=A1d[:], rhs=h1T_f[:, :cw],
                                 start=True, stop=True)
                as1T_f = sb.tile([H, CH], F32, tag="as1Tf")
                ad1T_f = sb.tile([H, CH], F32, tag="ad1Tf")
                nc.vector.tensor_copy(out=as1T_f[:, :cw], in_=as1T_ps[:, :cw])
                nc.vector.tensor_copy(out=ad1T_f[:, :cw], in_=ad1T_ps[:, :cw])
                for b in range(cw // P):
                    nn = c0 + b * P
                    w = w0 + b
                    hnm_ps = pp1.tile([P, D1], BF16, space="PSUM", tag="stp",
                                      name="hnm_ps")
                    nc.tensor.matmul(out=hnm_ps[:],
                                     lhsT=h1T_b[:, b * P:(b + 1) * P],
                                     rhs=I64b[:], is_transpose=True,
                                     start=True, stop=True)
                    hnm = sb.tile([P, D1], BF16, tag="hnm")
                    nc.scalar.activation(hnm[:], hnm_ps[:],
                                         mybir.ActivationFunctionType.Identity)
                    nc.sync.dma_start(out=t1loc[nn:nn + P, 0:D1].bitcast(BF16),
                                      in_=hnm[:])
                    asT_ps = pp1.tile([P, H], F32, space="PSUM", tag="t1")
                    nc.tensor.matmul(out=asT_ps[:], lhsT=as1T_f[:, b * P:(b + 1) * P],
                                     rhs=I8[:], is_transpose=True, start=True, stop=True)
                    asnm = sb.tile([P, H], F32, tag="asnm")
                    nc.vector.tensor_copy(out=asnm[:], in_=asT_ps[:])
                    nc.sync.dma_start(out=t1loc[nn:nn + P, D1:TCOL].bitcast(F32),
                                      in_=asnm[:])
                    adT_ps = pp1.tile([P, H], F32, space="PSUM", tag="t2")
                    nc.tensor.matmul(out=adT_ps[:], lhsT=ad1T_f[:, b * P:(b + 1) * P],
                                     rhs=I8[:], is_transpose=True, start=True, stop=True)
                    adnm = sb.tile([P, H], F32, tag="adnm")
                    nc.vector.tensor_copy(out=adnm[:], in_=adT_ps[:])
                    split_hilo(ad1_sb[:, w * 16:w * 16 + 8],
                               ad1_sb[:, w * 16 + 8:w * 16 + 16], adnm[:])

            nc.gpsimd.collective_compute(
                "AllGather", mybir.AluOpType.bypass,
                replica_groups=[list(range(NC))],
                ins=[t1loc[:, :].opt()],
                outs=[t1full[0:NC * SPAD, :].opt()],
            )

            def edge_phase(tfull, ad_sb, layer):
                NCOLS = D1 if layer == 1 else H * F2
                FV = F1 if layer == 1 else F2
                for w in range(NW):
                    tw = T_w[w]
                    cwid = col0[w]
                    UCOLS = NCOLS + H if layer == 1 else NCOLS
                    U_ps = pp.tile([P, UCOLS], F32, space="PSUM", tag="U")
                    den_ps = None
                    if layer == 2:
                        den_ps = pp1.tile([P, H], F32, space="PSUM", tag="den",
                                          name="den_ps")
                    g_all = gp.tile([P, TMAX * TCOL], U16, tag="ga", bufs=6)
                    s_all = gp.tile([P, TMAX * P], BF16, tag="sa", bufs=4)
                    ad_all = gp.tile([P, TMAX * 16], F32, tag="ada")
                    e_all = gp.tile([P, TMAX * H], F32, tag="ea")
                    lr_all = gp.tile([P, TMAX * H], F32, tag="la")
                    p_all = gp.tile([P, TMAX * H], BF16, tag="pa")
                    for t in range(tw):
                        nc.gpsimd.indirect_dma_start(
                            out=g_all[:, t * TCOL:(t + 1) * TCOL], out_offset=None,
                            in_=tfull[:, :],
                            in_offset=bass.IndirectOffsetOnAxis(
                                ap=so_sb[:, cwid + t:cwid + t + 1], axis=0),
                        )
                    nc.vector.tensor_tensor(
                        out=view(s_all[:], 0, [[P, tw], [1, P]]),
                        in0=view(dr_sb[:], cwid, [[1, tw], [0, P]]),
                        in1=view(iotaC[:], 0, [[0, tw], [1, P]]),
                        op=mybir.AluOpType.is_equal)
                    for t in range(tw):
                        st_ps = pp1.tile([P, P], BF16, space="PSUM", tag="stp")
                        nc.tensor.matmul(out=st_ps[:],
                                         lhsT=s_all[:, t * P:(t + 1) * P],
                                         rhs=I128b[:], is_transpose=True,
                                         start=True, stop=True)
                        st_t = sb8.tile([P, P], BF16, tag="st")
                        nc.scalar.activation(st_t[:], st_ps[:],
                                             mybir.ActivationFunctionType.Identity)
                        ad_ps = pp1.tile([P, 16], F32, space="PSUM", tag="adps")
                        nc.tensor.matmul(out=ad_ps[:], lhsT=st_t[:],
                                         rhs=ad_sb[:, w * 16:(w + 1) * 16],
                                         start=True, stop=True)
                        nc.scalar.activation(ad_all[:, t * 16:(t + 1) * 16],
                                             ad_ps[:],
                                             mybir.ActivationFunctionType.Identity)
                    gf = g_all[:].bitcast(F32)
                    nc.vector.tensor_tensor(
                        out=view(e_all[:], 0, [[H, tw], [1, H]]),
                        in0=view(gf, 32, [[40, tw], [1, H]]),
                        in1=view(ad_all[:], 0, [[16, tw], [1, H]]),
                        op=mybir.AluOpType.add)
                    nc.vector.tensor_tensor(
                        out=view(e_all[:], 0, [[H, tw], [1, H]]),
                        in0=view(e_all[:], 0, [[H, tw], [1, H]]),
                        in1=view(ad_all[:], 8, [[16, tw], [1, H]]),
                        op=mybir.AluOpType.add)
                    nc.vector.tensor_scalar_mul(out=lr_all[:, :tw * H],
                                                in0=e_all[:, :tw * H], scalar1=NEG)
                    nc.vector.tensor_tensor(out=lr_all[:, :tw * H],
                                            in0=lr_all[:, :tw * H],
                                            in1=e_all[:, :tw * H],
                                            op=mybir.AluOpType.max)
                    nc.scalar.activation(p_all[:, :tw * H], lr_all[:, :tw * H],
                                         mybir.ActivationFunctionType.Exp)
                    for t in range(tw):
                        w_t = sb8.tile([P, UCOLS], BF16, tag="wv")
                        gh = g_all[:, t * TCOL:t * TCOL + D1].bitcast(BF16)
                        if layer == 1:
                            in0 = gh.rearrange("p (h f) -> p h f", h=H)
                        else:
                            in0 = mid_bcast(gh, H)
                        nc.vector.tensor_tensor(
                            out=w_t[:, :H * FV].rearrange("p (h f) -> p h f", h=H),
                            in0=in0,
                            in1=p_all[:, t * H:(t + 1) * H].to_broadcast([P, H, FV]),
                            op=mybir.AluOpType.mult)
                        if layer == 1:
                            nc.scalar.activation(
                                w_t[:, H * FV:UCOLS],
                                p_all[:, t * H:(t + 1) * H],
                                mybir.ActivationFunctionType.Identity)
                        nc.tensor.matmul(out=U_ps[:],
                                         lhsT=s_all[:, t * P:(t + 1) * P],
                                         rhs=w_t[:], start=(t == 0),
                                         stop=(t == tw - 1))
                        if layer == 2:
                            nc.tensor.matmul(out=den_ps[:],
                                             lhsT=s_all[:, t * P:(t + 1) * P],
                                             rhs=p_all[:, t * H:(t + 1) * H],
                                             start=(t == 0), stop=(t == tw - 1))
                    den_src = (U_ps[:, H * FV:UCOLS] if layer == 1 else den_ps[:])
                    dse = sb.tile([P, H], F32, tag="dse")
                    nc.vector.tensor_scalar_add(out=dse[:], in0=den_src, scalar1=1e-30)
                    rd = sb.tile([P, H], F32, tag="rd")
                    nc.vector.reciprocal(out=rd[:], in_=dse[:])
                    if layer == 1:
                        h2a = sb.tile([P, D1], F32, tag="h2a")
                        nc.vector.tensor_tensor(
                            out=h2a[:].rearrange("p (h f) -> p h f", h=H),
                            in0=U_ps[:, 0:D1].rearrange("p (h f) -> p h f", h=H),
                            in1=rd[:].to_broadcast([P, H, F1]),
                            op=mybir.AluOpType.mult)
                        nc.vector.tensor_tensor(out=h2a[:], in0=h2a[:], in1=b1rep[:],
                                                op=mybir.AluOpType.add)
                        ex = sb.tile([P, D1], F32, tag="ex")
                        nc.scalar.activation(ex[:], h2a[:],
                                             mybir.ActivationFunctionType.Exp)
                        exm = sb.tile([P, D1], F32, tag="exm")
                        nc.vector.tensor_scalar(out=exm[:], in0=ex[:], scalar1=1.0,
                                                scalar2=-1.0, op0=mybir.AluOpType.min,
                                                op1=mybir.AluOpType.add)
                        rl = sb.tile([P, D1], F32, tag="rl")
                        nc.vector.tensor_scalar_max(out=rl[:], in0=h2a[:], scalar1=0.0)
                        h2e = sb.tile([P, D1], F32, tag="h2e")
                        nc.vector.tensor_tensor(out=h2e[:], in0=exm[:], in1=rl[:],
                                                op=mybir.AluOpType.add)
                        h2eb = sb.tile([P, D1], BF16, tag="h2eb")
                        nc.vector.tensor_copy(out=h2eb[:], in_=h2e[:])
                        nc.sync.dma_start(
                            out=t2loc[w * P:(w + 1) * P, 0:D1].bitcast(BF16),
                            in_=h2eb[:])
                        hT_ps = pp1.tile([D1, P], F32, space="PSUM", tag="t1")
                        nc.tensor.matmul(out=hT_ps[:], lhsT=h2e[:], rhs=I128[:],
                                         is_transpose=True, start=True, stop=True)
                        hT = sb.tile([D1, P], F32, tag="hT")
                        nc.vector.tensor_copy(out=hT[:], in_=hT_ps[:])
                        a2T_ps = pp1.tile([H, P], F32, space="PSUM", tag="t2")
                        nc.tensor.matmul(out=a2T_ps[:], lhsT=As2[:], rhs=hT[:],
                                         start=True, stop=True)
                        d2T_ps = pp1.tile([H, P], F32, space="PSUM", tag="t3")
                        nc.tensor.matmul(out=d2T_ps[:], lhsT=Ad2[:], rhs=hT[:],
                                         start=True, stop=True)
                        a2T = sb.tile([H, P], F32, tag="a2T")
                        d2T = sb.tile([H, P], F32, tag="d2T")
                        nc.vector.tensor_copy(out=a2T[:], in_=a2T_ps[:])
                        nc.vector.tensor_copy(out=d2T[:], in_=d2T_ps[:])
                        a2nm_ps = pp1.tile([P, H], F32, space="PSUM", tag="t1")
                        nc.tensor.matmul(out=a2nm_ps[:], lhsT=a2T[:], rhs=I8[:],
                                         is_transpose=True, start=True, stop=True)
                        a2nm = sb.tile([P, H], F32, tag="a2nm")
                        nc.vector.tensor_copy(out=a2nm[:], in_=a2nm_ps[:])
                        nc.sync.dma_start(
                            out=t2loc[w * P:(w + 1) * P, D1:TCOL].bitcast(F32),
                            in_=a2nm[:])
                        d2nm_ps = pp1.tile([P, H], F32, space="PSUM", tag="t2")
                        nc.tensor.matmul(out=d2nm_ps[:], lhsT=d2T[:], rhs=I8[:],
                                         is_transpose=True, start=True, stop=True)
                        d2nm = sb.tile([P, H], F32, tag="d2nm")
                        nc.vector.tensor_copy(out=d2nm[:], in_=d2nm_ps[:])
                        split_hilo(ad2_sb[:, w * 16:w * 16 + 8],
                                   ad2_sb[:, w * 16 + 8:w * 16 + 16], d2nm[:])
                    else:
                        U2n = sb.tile([P, H * F2], F32, tag="U2n")
                        nc.vector.tensor_tensor(
                            out=U2n[:].rearrange("p (h f) -> p h f", h=H),
                            in0=U_ps[:].rearrange("p (h f) -> p h f", h=H),
                            in1=rd[:].to_broadcast([P, H, F2]),
                            op=mybir.AluOpType.mult)
                        YT_ps = pp1.tile([F2, P], F32, space="PSUM", tag="t3")
                        for h in range(H):
                            uT_ps = pp1.tile([F2, P], F32, space="PSUM", tag="t1")
                            nc.tensor.matmul(out=uT_ps[:],
                                             lhsT=U2n[:, h * F2:(h + 1) * F2],
                                             rhs=I128[:], is_transpose=True,
                                             start=True, stop=True)
                            uT = sb.tile([F2, P], F32, tag="uTs")
                            nc.vector.tensor_copy(out=uT[:], in_=uT_ps[:])
                            nc.tensor.matmul(out=YT_ps[:],
                                             lhsT=W2f[:, h * F2:(h + 1) * F2],
                                             rhs=uT[:], start=(h == 0),
                                             stop=(h == H - 1))
                        Y = sb.tile([F2, P], F32, tag="Y")
                        nc.scalar.activation(Y[:], YT_ps[:],
                                             mybir.ActivationFunctionType.Identity,
                                             bias=b2col[:], scale=1.0 / H)
                        o_ps = pp1.tile([P, F2], F32, space="PSUM", tag="t2")
                        nc.tensor.matmul(out=o_ps[:], lhsT=Y[:], rhs=I64[:],
                                         is_transpose=True, start=True, stop=True)
                        ow = sb.tile([P, F2], F32, tag="ow")
                        nc.vector.tensor_copy(out=ow[:], in_=o_ps[:])
                        rows = min(P, SLICE - w * P)
                        nc.sync.dma_start(out=out_d[w * P:w * P + rows, :],
                                          in_=ow[:rows, :])

            edge_phase(t1full, ad1_sb, 1)
            nc.gpsimd.collective_compute(
                "AllGather", mybir.AluOpType.bypass,
                replica_groups=[list(range(NC))],
                ins=[t2loc[:, :].opt()],
                outs=[t2full[0:NC * SPAD, :].opt()],
            )
            edge_phase(t2full, ad2_sb, 2)

    nc.compile()
    return nc


def kernel(**inputs):
    global LAST_EXEC_NS
    _register_profile_hook()
    from concourse import bass_utils

    in_maps, meta = host_prep(inputs)
    nc = build(meta)
    trace = os.environ.get("GAT_TRACE", "1") == "1"
    try:
        res = bass_utils.run_bass_kernel_spmd(
            nc, in_maps, core_ids=list(range(NC)), trace=trace)
    except Exception:
        if not trace:
            raise
        res = bass_utils.run_bass_kernel_spmd(
            nc, in_maps, core_ids=list(range(NC)), trace=False)
    LAST_EXEC_NS = res.exec_time_ns
    SLICE = meta["SLICE"]
    out = np.empty((N, F2), np.float32)
    for c in range(NC):
        out[c * SLICE:(c + 1) * SLICE] = res.results[c]["out"]
    return out



# revision 10
# speedup vs baseline: 1.0819x; 1.0031x over previous
"""Self-contained GAT kernel for 8 TRN2 NeuronCores.

kernel(**inputs) takes the FULL unsharded inputs (as produced by
setup_inputs) and returns the FULL [100000, 64] float32 output.

Architecture (see module gat_kernel-style doc):
- nodes dst-partitioned across 8 cores; edges dst-sorted into 128-dst windows,
  128-edge tiles.
- per-node table rows [h bf16 x64 | a_src f32 x8] packed as uint16[80];
  per-tile [128,1]-offset indirect-DMA gather.
- segment softmax/sums via one-hot selection matrices + PE matmuls; a_dst
  expanded per edge via DMA-transposed one-hot (S^T) matmuls from SBUF
  tables (bf16 hi+lo split for f32 accuracy).
- layer 2 aggregates 64-dim h2 per head and applies W2 after aggregation;
  head-mean via PSUM-accumulated per-head matmuls.
- AllGather collectives replicate node tables between phases.
"""
import os
import sys
import types

import numpy as np

sys.path.insert(0, "/opt/trn_rl_repo")

import ml_dtypes

import concourse.bass as bass
import concourse.bacc as bacc
import concourse.mybir as mybir
import concourse.tile as tile

BF16 = mybir.dt.bfloat16
F32 = mybir.dt.float32
I32 = mybir.dt.int32
U16 = mybir.dt.uint16

P = 128
H = 8
F1 = 8
F2 = 64
D1 = H * F1
IN_DIM = 256
NEG = 0.2
GHOST_AS = -300.0
TCOL = 80
NC = 8
N = 100000

LAST_EXEC_NS = None

_hook_registered = [False]


def _register_profile_hook():
    if _hook_registered[0]:
        return
    try:
        import antenv
        mod = types.ModuleType("antenv.axon_hooks")
        _h = [None]
        mod.set_axon_ntff_profile_hook = lambda f: _h.__setitem__(0, f)
        mod.get_axon_ntff_profile_hook = lambda: _h[0]
        sys.modules.setdefault("antenv.axon_hooks", mod)
        if not hasattr(antenv, "axon_hooks"):
            antenv.axon_hooks = mod
        from trn_agent_boot.trn_boot import _ntff_profile_via_ctypes
        sys.modules["antenv.axon_hooks"].set_axon_ntff_profile_hook(
            _ntff_profile_via_ctypes('/opt/axon/libaxon_pjrt.so'))
        _hook_registered[0] = True
    except Exception:
        pass


def mid_bcast(ap2d, reps):
    return bass.AP(ap2d.tensor, ap2d.offset, [ap2d.ap[0], [0, reps], ap2d.ap[1]])


def view(ap, off_elems, dims):
    """Custom strided view: dims = [[stride, count], ...] in ap-dtype elems."""
    return bass.AP(ap.tensor, ap.offset + off_elems, [ap.ap[0]] + dims)


def host_prep(inputs):
    SLICE = N // NC
    NW = (SLICE + P - 1) // P
    SPAD = NW * P
    GHOST = NC * SPAD

    edge = np.asarray(inputs["edge"])
    src = np.concatenate([np.asarray(edge[0]), np.arange(N, dtype=np.int64)])
    dst = np.concatenate([np.asarray(edge[1]), np.arange(N, dtype=np.int64)])

    core = (dst // SLICE).astype(np.int32)
    srcpad = ((src // SLICE) * SPAD + (src % SLICE)).astype(np.int32)
    dstl = (dst % SLICE).astype(np.int32)
    win = dstl // P

    counts = np.zeros((NC, NW), np.int64)
    for c in range(NC):
        m = core == c
        w, cnt = np.unique(win[m], return_counts=True)
        counts[c, w] = cnt
    T_w = np.maximum(1, (counts.max(axis=0) + P - 1) // P).astype(np.int64)
    T_tot = int(T_w.sum())
    col0 = np.concatenate([[0], np.cumsum(T_w)[:-1]])

    srcoff = np.full((NC, P, T_tot), GHOST, np.int32)
    dstrel = np.zeros((NC, P, T_tot), np.float32)
    order = np.argsort(core * np.int64(SLICE * 2) + dstl, kind="stable")
    s_s, d_s, c_s, w_s = srcpad[order], dstl[order], core[order], win[order]
    for c in range(NC):
        m = c_s == c
        sc, dc, wc = s_s[m], d_s[m], w_s[m]
        for w in range(NW):
            mw = wc == w
            k = int(mw.sum())
            tw = int(T_w[w])
            sl = np.full(tw * P, GHOST, np.int32)
            rl = np.zeros(tw * P, np.float32)
            sl[:k] = sc[mw]
            rl[:k] = (dc[mw] - w * P).astype(np.float32)
            cw = int(col0[w])
            srcoff[c, :, cw:cw + tw] = sl.reshape(tw, P).T
            dstrel[c, :, cw:cw + tw] = rl.reshape(tw, P).T

    grow = np.zeros(TCOL, np.uint16)
    grow[64:80] = np.full(8, GHOST_AS, np.float32).view(np.uint16)

    W1 = np.asarray(inputs["W1"], np.float32)
    a_src1 = np.asarray(inputs["a_src1"], np.float32)
    a_dst1 = np.asarray(inputs["a_dst1"], np.float32)
    b1 = np.asarray(inputs["b1"], np.float32)
    W2 = np.asarray(inputs["W2"], np.float32)
    a_src2 = np.asarray(inputs["a_src2"], np.float32)
    a_dst2 = np.asarray(inputs["a_dst2"], np.float32)
    b2 = np.asarray(inputs["b2"], np.float32)
    x = np.asarray(inputs["x"], np.float32)

    A1s = np.zeros((D1, H), np.float32)
    A1d = np.zeros((D1, H), np.float32)
    for h in range(H):
        A1s[h * F1:(h + 1) * F1, h] = a_src1[h]
        A1d[h * F1:(h + 1) * F1, h] = a_dst1[h]
    Wcat = np.concatenate([W1, W1 @ A1s, W1 @ A1d], axis=1)  # [256, 80]

    As2c = np.zeros((D1, H), np.float32)
    Ad2c = np.zeros((D1, H), np.float32)
    for h in range(H):
        As2c[:, h] = W2[:, h * F2:(h + 1) * F2] @ a_src2[h]
        Ad2c[:, h] = W2[:, h * F2:(h + 1) * F2] @ a_dst2[h]

    iotaC = np.broadcast_to(np.arange(P, dtype=np.float32), (P, P)).astype(ml_dtypes.bfloat16)

    shared = dict(
        Wcat=Wcat.astype(ml_dtypes.bfloat16),
        As2c=As2c, Ad2c=Ad2c,
        W2f=W2,
        b1rep=np.broadcast_to(b1, (P, D1)).copy(),
        b2col=np.ascontiguousarray(b2.reshape(F2, 1)),
        iotaC=np.ascontiguousarray(iotaC),
        I128=np.eye(P, dtype=np.float32),
        I128b=np.eye(P, dtype=ml_dtypes.bfloat16),
        I64=np.eye(F2, dtype=np.float32),
        ghostrow=grow.reshape(1, TCOL),
    )
    in_maps = []
    for c in range(NC):
        xs = np.zeros((SPAD, IN_DIM), np.float32)
        xs[:SLICE] = x[c * SLICE:(c + 1) * SLICE]
        m = dict(shared)
        m["xT"] = np.ascontiguousarray(xs.T)
        m["srcoff"] = np.ascontiguousarray(srcoff[c])
        m["dstrel"] = np.ascontiguousarray(dstrel[c]).astype(ml_dtypes.bfloat16)
        in_maps.append(m)

    meta = dict(SLICE=SLICE, NW=NW, SPAD=SPAD, GHOST=GHOST,
                T_w=[int(t) for t in T_w], col0=[int(cc) for cc in col0],
                T_tot=T_tot, NC=NC)
    return in_maps, meta


def build(meta):
    SLICE, NW, SPAD, GHOST, T_tot = (meta["SLICE"], meta["NW"], meta["SPAD"],
                                     meta["GHOST"], meta["T_tot"])
    T_w, col0 = meta["T_w"], meta["col0"]
    TMAX = max(T_w)
    CH = min(512, SPAD)
    n_chunks = (SPAD + CH - 1) // CH

    nc = bacc.Bacc('TRN2', num_devices=NC)
    xT = nc.dram_tensor("xT", [IN_DIM, SPAD], F32, kind="ExternalInput")
    srcoff = nc.dram_tensor("srcoff", [P, T_tot], I32, kind="ExternalInput")
    dstrel = nc.dram_tensor("dstrel", [P, T_tot], BF16, kind="ExternalInput")
    Wcat_d = nc.dram_tensor("Wcat", [IN_DIM, TCOL], BF16, kind="ExternalInput")
    As2c_d = nc.dram_tensor("As2c", [D1, H], F32, kind="ExternalInput")
    Ad2c_d = nc.dram_tensor("Ad2c", [D1, H], F32, kind="ExternalInput")
    W2f_d = nc.dram_tensor("W2f", [F2, H * F2], F32, kind="ExternalInput")
    b1rep_d = nc.dram_tensor("b1rep", [P, D1], F32, kind="ExternalInput")
    b2col_d = nc.dram_tensor("b2col", [F2, 1], F32, kind="ExternalInput")
    iotaC_d = nc.dram_tensor("iotaC", [P, P], BF16, kind="ExternalInput")
    I128_d = nc.dram_tensor("I128", [P, P], F32, kind="ExternalInput")
    I128b_d = nc.dram_tensor("I128b", [P, P], BF16, kind="ExternalInput")
    I64_d = nc.dram_tensor("I64", [F2, F2], F32, kind="ExternalInput")
    ghostrow_d = nc.dram_tensor("ghostrow", [1, TCOL], U16, kind="ExternalInput")
    out_d = nc.dram_tensor("out", [SLICE, F2], F32, kind="ExternalOutput")
    t1loc = nc.dram_tensor("t1loc", [SPAD, TCOL], U16)
    t1full = nc.dram_tensor("t1full", [NC * SPAD + 1, TCOL], U16)
    t2loc = nc.dram_tensor("t2loc", [SPAD, TCOL], U16)
    t2full = nc.dram_tensor("t2full", [NC * SPAD + 1, TCOL], U16)

    with tile.TileContext(nc) as tc:
        with tc.tile_pool(name="consts", bufs=1) as cpool, \
             tc.tile_pool(name="sb", bufs=3) as sb, \
             tc.tile_pool(name="sb8", bufs=8) as sb8, \
             tc.tile_pool(name="gp", bufs=3) as gp, \
             tc.tile_pool(name="pp", bufs=2, space="PSUM") as pp, \
             tc.tile_pool(name="pp1", bufs=1, space="PSUM") as pp1:

            def cload(dram, shape, dtype, tag):
                t = cpool.tile(shape, dtype, tag=tag)
                nc.sync.dma_start(out=t[:], in_=dram[:, :])
                return t

            As2c = cload(As2c_d, [D1, H], F32, "cAs2c")
            Ad2c = cload(Ad2c_d, [D1, H], F32, "cAd2c")
            W2f = cload(W2f_d, [F2, H * F2], F32, "cW2f")
            b1rep = cload(b1rep_d, [P, D1], F32, "cb1")
            b2col = cload(b2col_d, [F2, 1], F32, "cb2")
            iotaC = cload(iotaC_d, [P, P], BF16, "ciota")
            I128 = cload(I128_d, [P, P], F32, "cI128")
            I128b = cload(I128b_d, [P, P], BF16, "cI128b")
            I64 = cload(I64_d, [F2, F2], F32, "cI64")

            Wc0 = cpool.tile([P, TCOL], BF16, tag="Wc0")
            Wc1 = cpool.tile([P, TCOL], BF16, tag="Wc1")
            nc.sync.dma_start(out=Wc0[:], in_=Wcat_d[0:P, :])
            nc.sync.dma_start(out=Wc1[:], in_=Wcat_d[P:2 * P, :])

            grow_sb = cpool.tile([1, TCOL], U16, tag="grow")
            nc.sync.dma_start(out=grow_sb[:], in_=ghostrow_d[:, :])
            nc.sync.dma_start(out=t1full[GHOST:GHOST + 1, :], in_=grow_sb[:])
            nc.sync.dma_start(out=t2full[GHOST:GHOST + 1, :], in_=grow_sb[:])

            ad1_sb = cpool.tile([P, NW * 16], BF16, tag="ad1sb")
            ad2_sb = cpool.tile([P, NW * 16], BF16, tag="ad2sb")

            so_sb = cpool.tile([P, T_tot], I32, tag="sosb")
            dr_sb = cpool.tile([P, T_tot], BF16, tag="drsb")
            nc.sync.dma_start(out=so_sb[:], in_=srcoff[:, :])
            nc.sync.dma_start(out=dr_sb[:], in_=dstrel[:, :])

            def split_hilo(hi_ap, lo_ap, src_f32):
                nc.vector.tensor_copy(out=hi_ap, in_=src_f32)
                nc.vector.tensor_tensor(out=lo_ap, in0=src_f32, in1=hi_ap,
                                        op=mybir.AluOpType.subtract)

            for k in range(n_chunks):
                c0, c1 = k * CH, min((k + 1) * CH, SPAD)
                cw = c1 - c0
                w0 = c0 // P
                xa = sb.tile([P, CH], F32, tag="xa")
                xb = sb.tile([P, CH], F32, tag="xb")
                nc.sync.dma_start(out=xa[:, :cw], in_=xT[0:P, c0:c1])
                nc.sync.dma_start(out=xb[:, :cw], in_=xT[P:2 * P, c0:c1])
                xab = sb.tile([P, CH], BF16, tag="xab")
                xbb = sb.tile([P, CH], BF16, tag="xbb")
                nc.vector.tensor_copy(out=xab[:, :cw], in_=xa[:, :cw])
                nc.vector.tensor_copy(out=xbb[:, :cw], in_=xb[:, :cw])
                for b in range(cw // P):
                    nn = c0 + b * P
                    w = w0 + b
                    p0ps = pp.tile([P, TCOL], F32, space="PSUM", tag="U",
                                   name="p0ps")
                    nc.tensor.matmul(out=p0ps[:], lhsT=xab[:, b * P:(b + 1) * P],
                                     rhs=Wc0[:], start=True, stop=False)
                    nc.tensor.matmul(out=p0ps[:], lhsT=xbb[:, b * P:(b + 1) * P],
                                     rhs=Wc1[:], start=False, stop=True)
                    trow = sb.tile([P, TCOL], U16, tag="trow")
                    nc.vector.tensor_copy(out=trow[:, 0:D1].bitcast(BF16),
                                          in_=p0ps[:, 0:D1])
                    nc.vector.tensor_copy(out=trow[:, D1:TCOL].bitcast(F32),
                                          in_=p0ps[:, D1:D1 + H])
                    split_hilo(ad1_sb[:, w * 16:w * 16 + 8],
                               ad1_sb[:, w * 16 + 8:w * 16 + 16],
                               p0ps[:, D1 + H:TCOL])
                    nc.sync.dma_start(out=t1loc[nn:nn + P, :], in_=trow[:])

            nc.gpsimd.collective_compute(
                "AllGather", mybir.AluOpType.bypass,
                replica_groups=[list(range(NC))],
                ins=[t1loc[:, :].opt()],
                outs=[t1full[0:NC * SPAD, :].opt()],
            )

            def edge_phase(tfull, ad_sb, layer):
                NCOLS = D1 if layer == 1 else H * F2
                FV = F1 if layer == 1 else F2
                for w in range(NW):
                    tw = T_w[w]
                    cwid = col0[w]
                    UCOLS = NCOLS + H if layer == 1 else NCOLS
                    U_ps = pp.tile([P, UCOLS], F32, space="PSUM", tag="U")
                    den_ps = None
                    if layer == 2:
                        den_ps = pp1.tile([P, H], F32, space="PSUM", tag="den",
                                          name="den_ps")
                    g_all = gp.tile([P, TMAX * TCOL], U16, tag="ga", bufs=6)
                    s_all = gp.tile([P, TMAX * P], BF16, tag="sa", bufs=4)
                    ad_all = gp.tile([P, TMAX * 16], F32, tag="ada")
                    e_all = gp.tile([P, TMAX * H], F32, tag="ea")
                    lr_all = gp.tile([P, TMAX * H], F32, tag="la")
                    p_all = gp.tile([P, TMAX * H], BF16, tag="pa")
                    for t in range(tw):
                        nc.gpsimd.indirect_dma_start(
                            out=g_all[:, t * TCOL:(t + 1) * TCOL], out_offset=None,
                            in_=tfull[:, :],
                            in_offset=bass.IndirectOffsetOnAxis(
                                ap=so_sb[:, cwid + t:cwid + t + 1], axis=0),
                        )
                    nc.vector.tensor_tensor(
                        out=view(s_all[:], 0, [[P, tw], [1, P]]),
                        in0=view(dr_sb[:], cwid, [[1, tw], [0, P]]),
                        in1=view(iotaC[:], 0, [[0, tw], [1, P]]),
                        op=mybir.AluOpType.is_equal)
                    for t in range(tw):
                        st_ps = pp1.tile([P, P], BF16, space="PSUM", tag="stp")
                        nc.tensor.matmul(out=st_ps[:],
                                         lhsT=s_all[:, t * P:(t + 1) * P],
                                         rhs=I128b[:], is_transpose=True,
                                         start=True, stop=True)
                        st_t = sb8.tile([P, P], BF16, tag="st")
                        nc.scalar.activation(st_t[:], st_ps[:],
                                             mybir.ActivationFunctionType.Identity)
                        ad_ps = pp1.tile([P, 16], F32, space="PSUM", tag="adps")
                        nc.tensor.matmul(out=ad_ps[:], lhsT=st_t[:],
                                         rhs=ad_sb[:, w * 16:(w + 1) * 16],
                                         start=True, stop=True)
                        nc.scalar.activation(ad_all[:, t * 16:(t + 1) * 16],
                                             ad_ps[:],
                                             mybir.ActivationFunctionType.Identity)
                    gf = g_all[:].bitcast(F32)
                    nc.vector.tensor_tensor(
                        out=view(e_all[:], 0, [[H, tw], [1, H]]),
                        in0=view(gf, 32, [[40, tw], [1, H]]),
                        in1=view(ad_all[:], 0, [[16, tw], [1, H]]),
                        op=mybir.AluOpType.add)
                    nc.vector.tensor_tensor(
                        out=view(e_all[:], 0, [[H, tw], [1, H]]),
                        in0=view(e_all[:], 0, [[H, tw], [1, H]]),
                        in1=view(ad_all[:], 8, [[16, tw], [1, H]]),
                        op=mybir.AluOpType.add)
                    nc.vector.tensor_scalar_mul(out=lr_all[:, :tw * H],
                                                in0=e_all[:, :tw * H], scalar1=NEG)
                    nc.vector.tensor_tensor(out=lr_all[:, :tw * H],
                                            in0=lr_all[:, :tw * H],
                                            in1=e_all[:, :tw * H],
                                            op=mybir.AluOpType.max)
                    nc.scalar.activation(p_all[:, :tw * H], lr_all[:, :tw * H],
                                         mybir.ActivationFunctionType.Exp)
                    for t in range(tw):
                        w_t = sb8.tile([P, UCOLS], BF16, tag="wv")
                        gh = g_all[:, t * TCOL:t * TCOL + D1].bitcast(BF16)
                        if layer == 1:
                            in0 = gh.rearrange("p (h f) -> p h f", h=H)
                        else:
                            in0 = mid_bcast(gh, H)
                        nc.vector.tensor_tensor(
                            out=w_t[:, :H * FV].rearrange("p (h f) -> p h f", h=H),
                            in0=in0,
                            in1=p_all[:, t * H:(t + 1) * H].to_broadcast([P, H, FV]),
                            op=mybir.AluOpType.mult)
                        if layer == 1:
                            nc.scalar.activation(
                                w_t[:, H * FV:UCOLS],
                                p_all[:, t * H:(t + 1) * H],
                                mybir.ActivationFunctionType.Identity)
                        nc.tensor.matmul(out=U_ps[:],
                                         lhsT=s_all[:, t * P:(t + 1) * P],
                                         rhs=w_t[:], start=(t == 0),
                                         stop=(t == tw - 1))
                        if layer == 2:
                            nc.tensor.matmul(out=den_ps[:],
                                             lhsT=s_all[:, t * P:(t + 1) * P],
                                             rhs=p_all[:, t * H:(t + 1) * H],
                                             start=(t == 0), stop=(t == tw - 1))
                    den_src = (U_ps[:, H * FV:UCOLS] if layer == 1 else den_ps[:])
                    dse = sb.tile([P, H], F32, tag="dse")
                    nc.vector.tensor_scalar_add(out=dse[:], in0=den_src, scalar1=1e-30)
                    rd = sb.tile([P, H], F32, tag="rd")
                    nc.vector.reciprocal(out=rd[:], in_=dse[:])
                    if layer == 1:
                        h2a = sb.tile([P, D1], F32, tag="h2a")
                        nc.vector.tensor_tensor(
                            out=h2a[:].rearrange("p (h f) -> p h f", h=H),
                            in0=U_ps[:, 0:D1].rearrange("p (h f) -> p h f", h=H),
                            in1=rd[:].to_broadcast([P, H, F1]),
                            op=mybir.AluOpType.mult)
                        nc.vector.tensor_tensor(out=h2a[:], in0=h2a[:], in1=b1rep[:],
                                                op=mybir.AluOpType.add)
                        ex = sb.tile([P, D1], F32, tag="ex")
                        nc.scalar.activation(ex[:], h2a[:],
                                             mybir.ActivationFunctionType.Exp)
                        exm = sb.tile([P, D1], F32, tag="exm")
                        nc.vector.tensor_scalar(out=exm[:], in0=ex[:], scalar1=1.0,
                                                scalar2=-1.0, op0=mybir.AluOpType.min,
                                                op1=mybir.AluOpType.add)
                        rl = sb.tile([P, D1], F32, tag="rl")
                        nc.vector.tensor_scalar_max(out=rl[:], in0=h2a[:], scalar1=0.0)
                        h2e = sb.tile([P, D1], F32, tag="h2e")
                        nc.vector.tensor_tensor(out=h2e[:], in0=exm[:], in1=rl[:],
                                                op=mybir.AluOpType.add)
                        trow2 = sb.tile([P, TCOL], U16, tag="h2eb")
                        nc.vector.tensor_copy(out=trow2[:, 0:D1].bitcast(BF16),
                                              in_=h2e[:])
                        hT_ps = pp1.tile([D1, P], F32, space="PSUM", tag="t1")
                        nc.tensor.matmul(out=hT_ps[:], lhsT=h2e[:], rhs=I128[:],
                                         is_transpose=True, start=True, stop=True)
                        hT = sb.tile([D1, P], F32, tag="hT")
                        nc.vector.tensor_copy(out=hT[:], in_=hT_ps[:])
                        a2_ps = pp1.tile([P, H], F32, space="PSUM", tag="t2")
                        nc.tensor.matmul(out=a2_ps[:], lhsT=hT[:], rhs=As2c[:],
                                         start=True, stop=True)
                        nc.vector.tensor_copy(out=trow2[:, D1:TCOL].bitcast(F32),
                                              in_=a2_ps[:])
                        d2_ps = pp1.tile([P, H], F32, space="PSUM", tag="t3")
                        nc.tensor.matmul(out=d2_ps[:], lhsT=hT[:], rhs=Ad2c[:],
                                         start=True, stop=True)
                        split_hilo(ad2_sb[:, w * 16:w * 16 + 8],
                                   ad2_sb[:, w * 16 + 8:w * 16 + 16], d2_ps[:])
                        nc.sync.dma_start(out=t2loc[w * P:(w + 1) * P, :],
                                          in_=trow2[:])
                    else:
                        U2n = sb.tile([P, H * F2], F32, tag="U2n")
                        nc.vector.tensor_tensor(
                            out=U2n[:].rearrange("p (h f) -> p h f", h=H),
                            in0=U_ps[:].rearrange("p (h f) -> p h f", h=H),
                            in1=rd[:].to_broadcast([P, H, F2]),
                            op=mybir.AluOpType.mult)
                        YT_ps = pp1.tile([F2, P], F32, space="PSUM", tag="t3")
                        for h in range(H):
                            uT_ps = pp1.tile([F2, P], F32, space="PSUM", tag="t1")
                            nc.tensor.matmul(out=uT_ps[:],
                                             lhsT=U2n[:, h * F2:(h + 1) * F2],
                                             rhs=I128[:], is_transpose=True,
                                             start=True, stop=True)
                            uT = sb.tile([F2, P], F32, tag="uTs")
                            nc.vector.tensor_copy(out=uT[:], in_=uT_ps[:])
                            nc.tensor.matmul(out=YT_ps[:],
                                             lhsT=W2f[:, h * F2:(h + 1) * F2],
                                             rhs=uT[:], start=(h == 0),
                                             stop=(h == H - 1))
                        Y = sb.tile([F2, P], F32, tag="Y")
                        nc.scalar.activation(Y[:], YT_ps[:],
                                             mybir.ActivationFunctionType.Identity,
                                             bias=b2col[:], scale=1.0 / H)
                        o_ps = pp1.tile([P, F2], F32, space="PSUM", tag="t2")
                        nc.tensor.matmul(out=o_ps[:], lhsT=Y[:], rhs=I64[:],
                                         is_transpose=True, start=True, stop=True)
                        ow = sb.tile([P, F2], F32, tag="ow")
                        nc.vector.tensor_copy(out=ow[:], in_=o_ps[:])
                        rows = min(P, SLICE - w * P)
                        nc.sync.dma_start(out=out_d[w * P:w * P + rows, :],
                                          in_=ow[:rows, :])

            edge_phase(t1full, ad1_sb, 1)
            nc.gpsimd.collective_compute(
                "AllGather", mybir.AluOpType.bypass,
                replica_groups=[list(range(NC))],
                ins=[t2loc[:, :].opt()],
                outs=[t2full[0:NC * SPAD, :].opt()],
            )
            edge_phase(t2full, ad2_sb, 2)

    nc.compile()
    return nc


def kernel(**inputs):
    global LAST_EXEC_NS
    _register_profile_hook()
    from concourse import bass_utils

    in_maps, meta = host_prep(inputs)
    nc = build(meta)
    trace = os.environ.get("GAT_TRACE", "1") == "1"
    try:
        res = bass_utils.run_bass_kernel_spmd(
            nc, in_maps, core_ids=list(range(NC)), trace=trace)
    except Exception:
        if not trace:
            raise
        res = bass_utils.run_bass_kernel_spmd(
            nc, in_maps, core_ids=list(range(NC)), trace=False)
    LAST_EXEC_NS = res.exec_time_ns
    SLICE = meta["SLICE"]
    out = np.empty((N, F2), np.float32)
    for c in range(NC):
        out[c * SLICE:(c + 1) * SLICE] = res.results[c]["out"]
    return out



# revision 11
# speedup vs baseline: 1.1346x; 1.0487x over previous
"""Self-contained GAT kernel for 8 TRN2 NeuronCores.

kernel(**inputs) takes the FULL unsharded inputs (as produced by
setup_inputs) and returns the FULL [100000, 64] float32 output.

Architecture (see module gat_kernel-style doc):
- nodes dst-partitioned across 8 cores; edges dst-sorted into 128-dst windows,
  128-edge tiles.
- per-node table rows [h bf16 x64 | a_src f32 x8] packed as uint16[80];
  per-tile [128,1]-offset indirect-DMA gather.
- segment softmax/sums via one-hot selection matrices + PE matmuls; a_dst
  expanded per edge via DMA-transposed one-hot (S^T) matmuls from SBUF
  tables (bf16 hi+lo split for f32 accuracy).
- layer 2 aggregates 64-dim h2 per head and applies W2 after aggregation;
  head-mean via PSUM-accumulated per-head matmuls.
- AllGather collectives replicate node tables between phases.
"""
import os
import sys
import types

import numpy as np

sys.path.insert(0, "/opt/trn_rl_repo")

import ml_dtypes

import concourse.bass as bass
import concourse.bacc as bacc
import concourse.mybir as mybir
import concourse.tile as tile

BF16 = mybir.dt.bfloat16
F32 = mybir.dt.float32
I32 = mybir.dt.int32
U16 = mybir.dt.uint16

P = 128
H = 8
F1 = 8
F2 = 64
D1 = H * F1
IN_DIM = 256
NEG = 0.2
GHOST_AS = -300.0
TCOL = 80
NC = 8
N = 100000

LAST_EXEC_NS = None

_hook_registered = [False]


def _register_profile_hook():
    if _hook_registered[0]:
        return
    try:
        import antenv
        mod = types.ModuleType("antenv.axon_hooks")
        _h = [None]
        mod.set_axon_ntff_profile_hook = lambda f: _h.__setitem__(0, f)
        mod.get_axon_ntff_profile_hook = lambda: _h[0]
        sys.modules.setdefault("antenv.axon_hooks", mod)
        if not hasattr(antenv, "axon_hooks"):
            antenv.axon_hooks = mod
        from trn_agent_boot.trn_boot import _ntff_profile_via_ctypes
        sys.modules["antenv.axon_hooks"].set_axon_ntff_profile_hook(
            _ntff_profile_via_ctypes('/opt/axon/libaxon_pjrt.so'))
        _hook_registered[0] = True
    except Exception:
        pass


def mid_bcast(ap2d, reps):
    return bass.AP(ap2d.tensor, ap2d.offset, [ap2d.ap[0], [0, reps], ap2d.ap[1]])


def view(ap, off_elems, dims):
    """Custom strided view: dims = [[stride, count], ...] in ap-dtype elems."""
    return bass.AP(ap.tensor, ap.offset + off_elems, [ap.ap[0]] + dims)


def host_prep(inputs):
    SLICE = N // NC
    NW = (SLICE + P - 1) // P
    SPAD = NW * P
    GHOST = NC * SPAD

    edge = np.asarray(inputs["edge"])
    src = np.concatenate([np.asarray(edge[0]), np.arange(N, dtype=np.int64)])
    dst = np.concatenate([np.asarray(edge[1]), np.arange(N, dtype=np.int64)])

    core = (dst // SLICE).astype(np.int32)
    srcpad = ((src // SLICE) * SPAD + (src % SLICE)).astype(np.int32)
    dstl = (dst % SLICE).astype(np.int32)
    win = dstl // P

    counts = np.zeros((NC, NW), np.int64)
    for c in range(NC):
        m = core == c
        w, cnt = np.unique(win[m], return_counts=True)
        counts[c, w] = cnt
    T_w = np.maximum(1, (counts.max(axis=0) + P - 1) // P).astype(np.int64)
    T_tot = int(T_w.sum())
    col0 = np.concatenate([[0], np.cumsum(T_w)[:-1]])

    srcoff = np.full((NC, P, T_tot), GHOST, np.int32)
    dstrel = np.zeros((NC, P, T_tot), np.float32)
    order = np.argsort(core * np.int64(SLICE * 2) + dstl, kind="stable")
    s_s, d_s, c_s, w_s = srcpad[order], dstl[order], core[order], win[order]
    for c in range(NC):
        m = c_s == c
        sc, dc, wc = s_s[m], d_s[m], w_s[m]
        for w in range(NW):
            mw = wc == w
            k = int(mw.sum())
            tw = int(T_w[w])
            sl = np.full(tw * P, GHOST, np.int32)
            rl = np.zeros(tw * P, np.float32)
            sl[:k] = sc[mw]
            rl[:k] = (dc[mw] - w * P).astype(np.float32)
            cw = int(col0[w])
            srcoff[c, :, cw:cw + tw] = sl.reshape(tw, P).T
            dstrel[c, :, cw:cw + tw] = rl.reshape(tw, P).T

    grow = np.zeros(TCOL, np.uint16)
    grow[64:80] = np.full(8, GHOST_AS, np.float32).view(np.uint16)

    W1 = np.asarray(inputs["W1"], np.float32)
    a_src1 = np.asarray(inputs["a_src1"], np.float32)
    a_dst1 = np.asarray(inputs["a_dst1"], np.float32)
    b1 = np.asarray(inputs["b1"], np.float32)
    W2 = np.asarray(inputs["W2"], np.float32)
    a_src2 = np.asarray(inputs["a_src2"], np.float32)
    a_dst2 = np.asarray(inputs["a_dst2"], np.float32)
    b2 = np.asarray(inputs["b2"], np.float32)
    x = np.asarray(inputs["x"], np.float32)

    A1s = np.zeros((D1, H), np.float32)
    A1d = np.zeros((D1, H), np.float32)
    for h in range(H):
        A1s[h * F1:(h + 1) * F1, h] = a_src1[h]
        A1d[h * F1:(h + 1) * F1, h] = a_dst1[h]
    Wcat = np.concatenate([W1, W1 @ A1s, W1 @ A1d], axis=1)  # [256, 80]

    As2c = np.zeros((D1, H), np.float32)
    Ad2c = np.zeros((D1, H), np.float32)
    for h in range(H):
        As2c[:, h] = W2[:, h * F2:(h + 1) * F2] @ a_src2[h]
        Ad2c[:, h] = W2[:, h * F2:(h + 1) * F2] @ a_dst2[h]

    iotaC = np.broadcast_to(np.arange(P, dtype=np.float32), (P, P)).astype(ml_dtypes.bfloat16)

    shared = dict(
        Wcat=Wcat.astype(ml_dtypes.bfloat16),
        As2c=As2c, Ad2c=Ad2c,
        W2f=W2,
        b1rep=np.broadcast_to(b1, (P, D1)).copy(),
        b2col=np.ascontiguousarray(b2.reshape(F2, 1)),
        iotaC=np.ascontiguousarray(iotaC),
        I128=np.eye(P, dtype=np.float32),
        I128b=np.eye(P, dtype=ml_dtypes.bfloat16),
        I64=np.eye(F2, dtype=np.float32),
        ghostrow=grow.reshape(1, TCOL),
    )
    in_maps = []
    for c in range(NC):
        xs = np.zeros((SPAD, IN_DIM), np.float32)
        xs[:SLICE] = x[c * SLICE:(c + 1) * SLICE]
        m = dict(shared)
        m["xT"] = np.ascontiguousarray(xs.T)
        m["srcoff"] = np.ascontiguousarray(srcoff[c])
        m["dstrel"] = np.ascontiguousarray(dstrel[c]).astype(ml_dtypes.bfloat16)
        in_maps.append(m)

    meta = dict(SLICE=SLICE, NW=NW, SPAD=SPAD, GHOST=GHOST,
                T_w=[int(t) for t in T_w], col0=[int(cc) for cc in col0],
                T_tot=T_tot, NC=NC)
    return in_maps, meta


def build(meta):
    SLICE, NW, SPAD, GHOST, T_tot = (meta["SLICE"], meta["NW"], meta["SPAD"],
                                     meta["GHOST"], meta["T_tot"])
    T_w, col0 = meta["T_w"], meta["col0"]
    TMAX = max(T_w)
    CH = min(512, SPAD)
    n_chunks = (SPAD + CH - 1) // CH

    nc = bacc.Bacc('TRN2', num_devices=NC)
    xT = nc.dram_tensor("xT", [IN_DIM, SPAD], F32, kind="ExternalInput")
    srcoff = nc.dram_tensor("srcoff", [P, T_tot], I32, kind="ExternalInput")
    dstrel = nc.dram_tensor("dstrel", [P, T_tot], BF16, kind="ExternalInput")
    Wcat_d = nc.dram_tensor("Wcat", [IN_DIM, TCOL], BF16, kind="ExternalInput")
    As2c_d = nc.dram_tensor("As2c", [D1, H], F32, kind="ExternalInput")
    Ad2c_d = nc.dram_tensor("Ad2c", [D1, H], F32, kind="ExternalInput")
    W2f_d = nc.dram_tensor("W2f", [F2, H * F2], F32, kind="ExternalInput")
    b1rep_d = nc.dram_tensor("b1rep", [P, D1], F32, kind="ExternalInput")
    b2col_d = nc.dram_tensor("b2col", [F2, 1], F32, kind="ExternalInput")
    iotaC_d = nc.dram_tensor("iotaC", [P, P], BF16, kind="ExternalInput")
    I128_d = nc.dram_tensor("I128", [P, P], F32, kind="ExternalInput")
    I128b_d = nc.dram_tensor("I128b", [P, P], BF16, kind="ExternalInput")
    I64_d = nc.dram_tensor("I64", [F2, F2], F32, kind="ExternalInput")
    ghostrow_d = nc.dram_tensor("ghostrow", [1, TCOL], U16, kind="ExternalInput")
    out_d = nc.dram_tensor("out", [SLICE, F2], F32, kind="ExternalOutput")
    t1loc = nc.dram_tensor("t1loc", [SPAD, TCOL], U16)
    t1full = nc.dram_tensor("t1full", [NC * SPAD + 1, TCOL], U16)
    t2loc = nc.dram_tensor("t2loc", [SPAD, TCOL], U16)
    t2full = nc.dram_tensor("t2full", [NC * SPAD + 1, TCOL], U16)

    with tile.TileContext(nc) as tc:
        with tc.tile_pool(name="consts", bufs=1) as cpool, \
             tc.tile_pool(name="sb", bufs=3) as sb, \
             tc.tile_pool(name="sb8", bufs=8) as sb8, \
             tc.tile_pool(name="gp", bufs=3) as gp, \
             tc.tile_pool(name="pp", bufs=2, space="PSUM") as pp, \
             tc.tile_pool(name="pp1", bufs=1, space="PSUM") as pp1:

            def cload(dram, shape, dtype, tag):
                t = cpool.tile(shape, dtype, tag=tag)
                nc.sync.dma_start(out=t[:], in_=dram[:, :])
                return t

            As2c = cload(As2c_d, [D1, H], F32, "cAs2c")
            Ad2c = cload(Ad2c_d, [D1, H], F32, "cAd2c")
            W2f = cload(W2f_d, [F2, H * F2], F32, "cW2f")
            b1rep = cload(b1rep_d, [P, D1], F32, "cb1")
            b2col = cload(b2col_d, [F2, 1], F32, "cb2")
            iotaC = cload(iotaC_d, [P, P], BF16, "ciota")
            I128 = cload(I128_d, [P, P], F32, "cI128")
            I128b = cload(I128b_d, [P, P], BF16, "cI128b")
            I64 = cload(I64_d, [F2, F2], F32, "cI64")

            Wc0 = cpool.tile([P, TCOL], BF16, tag="Wc0")
            Wc1 = cpool.tile([P, TCOL], BF16, tag="Wc1")
            nc.sync.dma_start(out=Wc0[:], in_=Wcat_d[0:P, :])
            nc.sync.dma_start(out=Wc1[:], in_=Wcat_d[P:2 * P, :])

            grow_sb = cpool.tile([1, TCOL], U16, tag="grow")
            nc.sync.dma_start(out=grow_sb[:], in_=ghostrow_d[:, :])
            nc.sync.dma_start(out=t1full[GHOST:GHOST + 1, :], in_=grow_sb[:])
            nc.sync.dma_start(out=t2full[GHOST:GHOST + 1, :], in_=grow_sb[:])

            ad1_sb = cpool.tile([P, NW * 16], BF16, tag="ad1sb")
            ad2_sb = cpool.tile([P, NW * 16], BF16, tag="ad2sb")

            so_sb = cpool.tile([P, T_tot], I32, tag="sosb")
            dr_sb = cpool.tile([P, T_tot], BF16, tag="drsb")
            nc.sync.dma_start(out=so_sb[:], in_=srcoff[:, :])
            nc.sync.dma_start(out=dr_sb[:], in_=dstrel[:, :])

            def split_hilo(hi_ap, lo_ap, src_f32):
                nc.vector.tensor_copy(out=hi_ap, in_=src_f32)
                nc.vector.tensor_tensor(out=lo_ap, in0=src_f32, in1=hi_ap,
                                        op=mybir.AluOpType.subtract)

            for k in range(n_chunks):
                c0, c1 = k * CH, min((k + 1) * CH, SPAD)
                cw = c1 - c0
                w0 = c0 // P
                xa = sb.tile([P, CH], F32, tag="xa")
                xb = sb.tile([P, CH], F32, tag="xb")
                nc.sync.dma_start(out=xa[:, :cw], in_=xT[0:P, c0:c1])
                nc.sync.dma_start(out=xb[:, :cw], in_=xT[P:2 * P, c0:c1])
                xab = sb.tile([P, CH], BF16, tag="xab")
                xbb = sb.tile([P, CH], BF16, tag="xbb")
                nc.vector.tensor_copy(out=xab[:, :cw], in_=xa[:, :cw])
                nc.vector.tensor_copy(out=xbb[:, :cw], in_=xb[:, :cw])
                for b in range(cw // P):
                    nn = c0 + b * P
                    w = w0 + b
                    p0ps = pp.tile([P, TCOL], F32, space="PSUM", tag="U",
                                   name="p0ps")
                    nc.tensor.matmul(out=p0ps[:], lhsT=xab[:, b * P:(b + 1) * P],
                                     rhs=Wc0[:], start=True, stop=False)
                    nc.tensor.matmul(out=p0ps[:], lhsT=xbb[:, b * P:(b + 1) * P],
                                     rhs=Wc1[:], start=False, stop=True)
                    trow = sb.tile([P, TCOL], U16, tag="trow")
                    nc.vector.tensor_copy(out=trow[:, 0:D1].bitcast(BF16),
                                          in_=p0ps[:, 0:D1])
                    nc.vector.tensor_copy(out=trow[:, D1:TCOL].bitcast(F32),
                                          in_=p0ps[:, D1:D1 + H])
                    split_hilo(ad1_sb[:, w * 16:w * 16 + 8],
                               ad1_sb[:, w * 16 + 8:w * 16 + 16],
                               p0ps[:, D1 + H:TCOL])
                    nc.sync.dma_start(out=t1loc[nn:nn + P, :], in_=trow[:])

            nc.gpsimd.collective_compute(
                "AllGather", mybir.AluOpType.bypass,
                replica_groups=[list(range(NC))],
                ins=[t1loc[:, :].opt()],
                outs=[t1full[0:NC * SPAD, :].opt()],
            )

            def edge_phase(tfull, ad_sb, layer):
                NCOLS = D1 if layer == 1 else H * F2
                FV = F1 if layer == 1 else F2
                for w in range(NW):
                    tw = T_w[w]
                    cwid = col0[w]
                    UCOLS = NCOLS + H if layer == 1 else NCOLS
                    U_ps = pp.tile([P, UCOLS], F32, space="PSUM", tag="U")
                    den_ps = None
                    if layer == 2:
                        den_ps = pp1.tile([P, H], F32, space="PSUM", tag="den",
                                          name="den_ps")
                    g_all = gp.tile([P, TMAX * TCOL], U16, tag="ga", bufs=10)
                    s_all = gp.tile([P, TMAX * P], BF16, tag="sa", bufs=4)
                    ad_all = gp.tile([P, TMAX * 16], F32, tag="ada")
                    e_all = gp.tile([P, TMAX * H], F32, tag="ea")
                    lr_all = gp.tile([P, TMAX * H], F32, tag="la")
                    p_all = gp.tile([P, TMAX * H], BF16, tag="pa")
                    for t in range(tw):
                        nc.gpsimd.indirect_dma_start(
                            out=g_all[:, t * TCOL:(t + 1) * TCOL], out_offset=None,
                            in_=tfull[:, :],
                            in_offset=bass.IndirectOffsetOnAxis(
                                ap=so_sb[:, cwid + t:cwid + t + 1], axis=0),
                        )
                    nc.vector.tensor_tensor(
                        out=view(s_all[:], 0, [[P, tw], [1, P]]),
                        in0=view(dr_sb[:], cwid, [[1, tw], [0, P]]),
                        in1=view(iotaC[:], 0, [[0, tw], [1, P]]),
                        op=mybir.AluOpType.is_equal)
                    for t in range(tw):
                        st_ps = pp1.tile([P, P], BF16, space="PSUM", tag="stp",
                                         bufs=2)
                        nc.tensor.matmul(out=st_ps[:],
                                         lhsT=s_all[:, t * P:(t + 1) * P],
                                         rhs=I128b[:], is_transpose=True,
                                         start=True, stop=True)
                        st_t = sb8.tile([P, P], BF16, tag="st")
                        nc.scalar.activation(st_t[:], st_ps[:],
                                             mybir.ActivationFunctionType.Identity)
                        ad_ps = pp1.tile([P, 16], F32, space="PSUM", tag="adps")
                        nc.tensor.matmul(out=ad_ps[:], lhsT=st_t[:],
                                         rhs=ad_sb[:, w * 16:(w + 1) * 16],
                                         start=True, stop=True)
                        nc.scalar.activation(ad_all[:, t * 16:(t + 1) * 16],
                                             ad_ps[:],
                                             mybir.ActivationFunctionType.Identity)
                    gf = g_all[:].bitcast(F32)
                    nc.vector.tensor_tensor(
                        out=view(e_all[:], 0, [[H, tw], [1, H]]),
                        in0=view(gf, 32, [[40, tw], [1, H]]),
                        in1=view(ad_all[:], 0, [[16, tw], [1, H]]),
                        op=mybir.AluOpType.add)
                    nc.vector.tensor_tensor(
                        out=view(e_all[:], 0, [[H, tw], [1, H]]),
                        in0=view(e_all[:], 0, [[H, tw], [1, H]]),
                        in1=view(ad_all[:], 8, [[16, tw], [1, H]]),
                        op=mybir.AluOpType.add)
                    nc.vector.tensor_scalar_mul(out=lr_all[:, :tw * H],
                                                in0=e_all[:, :tw * H], scalar1=NEG)
                    nc.vector.tensor_tensor(out=lr_all[:, :tw * H],
                                            in0=lr_all[:, :tw * H],
                                            in1=e_all[:, :tw * H],
                                            op=mybir.AluOpType.max)
                    nc.scalar.activation(p_all[:, :tw * H], lr_all[:, :tw * H],
                                         mybir.ActivationFunctionType.Exp)
                    for t in range(tw):
                        w_t = sb8.tile([P, UCOLS], BF16, tag="wv")
                        gh = g_all[:, t * TCOL:t * TCOL + D1].bitcast(BF16)
                        if layer == 1:
                            in0 = gh.rearrange("p (h f) -> p h f", h=H)
                        else:
                            in0 = mid_bcast(gh, H)
                        nc.vector.tensor_tensor(
                            out=w_t[:, :H * FV].rearrange("p (h f) -> p h f", h=H),
                            in0=in0,
                            in1=p_all[:, t * H:(t + 1) * H].to_broadcast([P, H, FV]),
                            op=mybir.AluOpType.mult)
                        if layer == 1:
                            nc.scalar.activation(
                                w_t[:, H * FV:UCOLS],
                                p_all[:, t * H:(t + 1) * H],
                                mybir.ActivationFunctionType.Identity)
                        nc.tensor.matmul(out=U_ps[:],
                                         lhsT=s_all[:, t * P:(t + 1) * P],
                                         rhs=w_t[:], start=(t == 0),
                                         stop=(t == tw - 1))
                        if layer == 2:
                            nc.tensor.matmul(out=den_ps[:],
                                             lhsT=s_all[:, t * P:(t + 1) * P],
                                             rhs=p_all[:, t * H:(t + 1) * H],
                                             start=(t == 0), stop=(t == tw - 1))
                    den_src = (U_ps[:, H * FV:UCOLS] if layer == 1 else den_ps[:])
                    dse = sb.tile([P, H], F32, tag="dse")
                    nc.vector.tensor_scalar_add(out=dse[:], in0=den_src, scalar1=1e-30)
                    rd = sb.tile([P, H], F32, tag="rd")
                    nc.vector.reciprocal(out=rd[:], in_=dse[:])
                    if layer == 1:
                        h2a = sb.tile([P, D1], F32, tag="h2a")
                        nc.vector.tensor_tensor(
                            out=h2a[:].rearrange("p (h f) -> p h f", h=H),
                            in0=U_ps[:, 0:D1].rearrange("p (h f) -> p h f", h=H),
                            in1=rd[:].to_broadcast([P, H, F1]),
                            op=mybir.AluOpType.mult)
                        nc.vector.tensor_tensor(out=h2a[:], in0=h2a[:], in1=b1rep[:],
                                                op=mybir.AluOpType.add)
                        ex = sb.tile([P, D1], F32, tag="ex")
                        nc.scalar.activation(ex[:], h2a[:],
                                             mybir.ActivationFunctionType.Exp)
                        exm = sb.tile([P, D1], F32, tag="exm")
                        nc.vector.tensor_scalar(out=exm[:], in0=ex[:], scalar1=1.0,
                                                scalar2=-1.0, op0=mybir.AluOpType.min,
                                                op1=mybir.AluOpType.add)
                        rl = sb.tile([P, D1], F32, tag="rl")
                        nc.vector.tensor_scalar_max(out=rl[:], in0=h2a[:], scalar1=0.0)
                        h2e = sb.tile([P, D1], F32, tag="h2e")
                        nc.vector.tensor_tensor(out=h2e[:], in0=exm[:], in1=rl[:],
                                                op=mybir.AluOpType.add)
                        trow2 = sb.tile([P, TCOL], U16, tag="h2eb")
                        nc.vector.tensor_copy(out=trow2[:, 0:D1].bitcast(BF16),
                                              in_=h2e[:])
                        hT_ps = pp1.tile([D1, P], F32, space="PSUM", tag="t1")
                        nc.tensor.matmul(out=hT_ps[:], lhsT=h2e[:], rhs=I128[:],
                                         is_transpose=True, start=True, stop=True)
                        hT = sb.tile([D1, P], F32, tag="hT")
                        nc.vector.tensor_copy(out=hT[:], in_=hT_ps[:])
                        a2_ps = pp1.tile([P, H], F32, space="PSUM", tag="t2")
                        nc.tensor.matmul(out=a2_ps[:], lhsT=hT[:], rhs=As2c[:],
                                         start=True, stop=True)
                        nc.vector.tensor_copy(out=trow2[:, D1:TCOL].bitcast(F32),
                                              in_=a2_ps[:])
                        d2_ps = pp1.tile([P, H], F32, space="PSUM", tag="t2",
                                         name="d2_ps")
                        nc.tensor.matmul(out=d2_ps[:], lhsT=hT[:], rhs=Ad2c[:],
                                         start=True, stop=True)
                        split_hilo(ad2_sb[:, w * 16:w * 16 + 8],
                                   ad2_sb[:, w * 16 + 8:w * 16 + 16], d2_ps[:])
                        nc.sync.dma_start(out=t2loc[w * P:(w + 1) * P, :],
                                          in_=trow2[:])
                    else:
                        U2n = sb.tile([P, H * F2], F32, tag="U2n")
                        nc.vector.tensor_tensor(
                            out=U2n[:].rearrange("p (h f) -> p h f", h=H),
                            in0=U_ps[:].rearrange("p (h f) -> p h f", h=H),
                            in1=rd[:].to_broadcast([P, H, F2]),
                            op=mybir.AluOpType.mult)
                        YT_ps = pp1.tile([F2, P], F32, space="PSUM", tag="t2",
                                         name="YT_ps")
                        for h in range(H):
                            uT_ps = pp1.tile([F2, P], F32, space="PSUM", tag="t1")
                            nc.tensor.matmul(out=uT_ps[:],
                                             lhsT=U2n[:, h * F2:(h + 1) * F2],
                                             rhs=I128[:], is_transpose=True,
                                             start=True, stop=True)
                            uT = sb.tile([F2, P], F32, tag="uTs")
                            nc.vector.tensor_copy(out=uT[:], in_=uT_ps[:])
                            nc.tensor.matmul(out=YT_ps[:],
                                             lhsT=W2f[:, h * F2:(h + 1) * F2],
                                             rhs=uT[:], start=(h == 0),
                                             stop=(h == H - 1))
                        Y = sb.tile([F2, P], F32, tag="Y")
                        nc.scalar.activation(Y[:], YT_ps[:],
                                             mybir.ActivationFunctionType.Identity,
                                             bias=b2col[:], scale=1.0 / H)
                        o_ps = pp1.tile([P, F2], F32, space="PSUM", tag="t1",
                                        name="o_ps")
                        nc.tensor.matmul(out=o_ps[:], lhsT=Y[:], rhs=I64[:],
                                         is_transpose=True, start=True, stop=True)
                        ow = sb.tile([P, F2], F32, tag="ow")
                        nc.vector.tensor_copy(out=ow[:], in_=o_ps[:])
                        rows = min(P, SLICE - w * P)
                        nc.sync.dma_start(out=out_d[w * P:w * P + rows, :],
                                          in_=ow[:rows, :])

            edge_phase(t1full, ad1_sb, 1)
            nc.gpsimd.collective_compute(
                "AllGather", mybir.AluOpType.bypass,
                replica_groups=[list(range(NC))],
                ins=[t2loc[:, :].opt()],
                outs=[t2full[0:NC * SPAD, :].opt()],
            )
            edge_phase(t2full, ad2_sb, 2)

    nc.compile()
    return nc


def kernel(**inputs):
    global LAST_EXEC_NS
    _register_profile_hook()
    from concourse import bass_utils

    in_maps, meta = host_prep(inputs)
    nc = build(meta)
    trace = os.environ.get("GAT_TRACE", "1") == "1"
    try:
        res = bass_utils.run_bass_kernel_spmd(
            nc, in_maps, core_ids=list(range(NC)), trace=trace)
    except Exception:
        if not trace:
            raise
        res = bass_utils.run_bass_kernel_spmd(
            nc, in_maps, core_ids=list(range(NC)), trace=False)
    LAST_EXEC_NS = res.exec_time_ns
    SLICE = meta["SLICE"]
    out = np.empty((N, F2), np.float32)
    for c in range(NC):
        out[c * SLICE:(c + 1) * SLICE] = res.results[c]["out"]
    return out



# revision 12
# speedup vs baseline: 1.1902x; 1.0490x over previous
"""Self-contained GAT kernel for 8 TRN2 NeuronCores.

kernel(**inputs) takes the FULL unsharded inputs (as produced by
setup_inputs) and returns the FULL [100000, 64] float32 output.

Architecture (see module gat_kernel-style doc):
- nodes dst-partitioned across 8 cores; edges dst-sorted into 128-dst windows,
  128-edge tiles.
- per-node table rows [h bf16 x64 | a_src f32 x8] packed as uint16[80];
  per-tile [128,1]-offset indirect-DMA gather.
- segment softmax/sums via one-hot selection matrices + PE matmuls; a_dst
  expanded per edge via DMA-transposed one-hot (S^T) matmuls from SBUF
  tables (bf16 hi+lo split for f32 accuracy).
- layer 2 aggregates 64-dim h2 per head and applies W2 after aggregation;
  head-mean via PSUM-accumulated per-head matmuls.
- AllGather collectives replicate node tables between phases.
"""
import os
import sys
import types

import numpy as np

sys.path.insert(0, "/opt/trn_rl_repo")

import ml_dtypes

import concourse.bass as bass
import concourse.bacc as bacc
import concourse.mybir as mybir
import concourse.tile as tile

BF16 = mybir.dt.bfloat16
F32 = mybir.dt.float32
I32 = mybir.dt.int32
U16 = mybir.dt.uint16

P = 128
H = 8
F1 = 8
F2 = 64
D1 = H * F1
IN_DIM = 256
NEG = 0.2
GHOST_AS = -300.0
TCOL = 80
NC = 8
N = 100000

LAST_EXEC_NS = None

_hook_registered = [False]


def _register_profile_hook():
    if _hook_registered[0]:
        return
    try:
        import antenv
        mod = types.ModuleType("antenv.axon_hooks")
        _h = [None]
        mod.set_axon_ntff_profile_hook = lambda f: _h.__setitem__(0, f)
        mod.get_axon_ntff_profile_hook = lambda: _h[0]
        sys.modules.setdefault("antenv.axon_hooks", mod)
        if not hasattr(antenv, "axon_hooks"):
            antenv.axon_hooks = mod
        from trn_agent_boot.trn_boot import _ntff_profile_via_ctypes
        sys.modules["antenv.axon_hooks"].set_axon_ntff_profile_hook(
            _ntff_profile_via_ctypes('/opt/axon/libaxon_pjrt.so'))
        _hook_registered[0] = True
    except Exception:
        pass


def mid_bcast(ap2d, reps):
    return bass.AP(ap2d.tensor, ap2d.offset, [ap2d.ap[0], [0, reps], ap2d.ap[1]])


def view(ap, off_elems, dims):
    """Custom strided view: dims = [[stride, count], ...] in ap-dtype elems."""
    return bass.AP(ap.tensor, ap.offset + off_elems, [ap.ap[0]] + dims)


def host_prep(inputs):
    SLICE = N // NC
    NW = (SLICE + P - 1) // P
    SPAD = NW * P
    GHOST = NC * SPAD

    # self-loops are handled on-device from the window's own (local,
    # contiguous) table rows — only real edges go through the gather.
    edge = np.asarray(inputs["edge"])
    src = np.asarray(edge[0])
    dst = np.asarray(edge[1])

    core = (dst // SLICE).astype(np.int32)
    srcpad = ((src // SLICE) * SPAD + (src % SLICE)).astype(np.int32)
    dstl = (dst % SLICE).astype(np.int32)
    win = dstl // P

    counts = np.zeros((NC, NW), np.int64)
    for c in range(NC):
        m = core == c
        w, cnt = np.unique(win[m], return_counts=True)
        counts[c, w] = cnt
    T_w = np.maximum(1, (counts.max(axis=0) + P - 1) // P).astype(np.int64)
    T_tot = int(T_w.sum())
    col0 = np.concatenate([[0], np.cumsum(T_w)[:-1]])

    srcoff = np.full((NC, P, T_tot), GHOST, np.int32)
    dstrel = np.zeros((NC, P, T_tot), np.float32)
    order = np.argsort(core * np.int64(SLICE * 2) + dstl, kind="stable")
    s_s, d_s, c_s, w_s = srcpad[order], dstl[order], core[order], win[order]
    for c in range(NC):
        m = c_s == c
        sc, dc, wc = s_s[m], d_s[m], w_s[m]
        for w in range(NW):
            mw = wc == w
            k = int(mw.sum())
            tw = int(T_w[w])
            sl = np.full(tw * P, GHOST, np.int32)
            rl = np.zeros(tw * P, np.float32)
            sl[:k] = sc[mw]
            rl[:k] = (dc[mw] - w * P).astype(np.float32)
            cw = int(col0[w])
            srcoff[c, :, cw:cw + tw] = sl.reshape(tw, P).T
            dstrel[c, :, cw:cw + tw] = rl.reshape(tw, P).T

    grow = np.zeros(TCOL, np.uint16)
    grow[64:80] = np.full(8, GHOST_AS, np.float32).view(np.uint16)

    W1 = np.asarray(inputs["W1"], np.float32)
    a_src1 = np.asarray(inputs["a_src1"], np.float32)
    a_dst1 = np.asarray(inputs["a_dst1"], np.float32)
    b1 = np.asarray(inputs["b1"], np.float32)
    W2 = np.asarray(inputs["W2"], np.float32)
    a_src2 = np.asarray(inputs["a_src2"], np.float32)
    a_dst2 = np.asarray(inputs["a_dst2"], np.float32)
    b2 = np.asarray(inputs["b2"], np.float32)
    x = np.asarray(inputs["x"], np.float32)

    A1s = np.zeros((D1, H), np.float32)
    A1d = np.zeros((D1, H), np.float32)
    for h in range(H):
        A1s[h * F1:(h + 1) * F1, h] = a_src1[h]
        A1d[h * F1:(h + 1) * F1, h] = a_dst1[h]
    Wcat = np.concatenate([W1, W1 @ A1s, W1 @ A1d], axis=1)  # [256, 80]

    As2c = np.zeros((D1, H), np.float32)
    Ad2c = np.zeros((D1, H), np.float32)
    for h in range(H):
        As2c[:, h] = W2[:, h * F2:(h + 1) * F2] @ a_src2[h]
        Ad2c[:, h] = W2[:, h * F2:(h + 1) * F2] @ a_dst2[h]

    iotaC = np.broadcast_to(np.arange(P, dtype=np.float32), (P, P)).astype(ml_dtypes.bfloat16)

    shared = dict(
        Wcat=Wcat.astype(ml_dtypes.bfloat16),
        As2c=As2c, Ad2c=Ad2c,
        W2f=W2,
        b1rep=np.broadcast_to(b1, (P, D1)).copy(),
        b2col=np.ascontiguousarray(b2.reshape(F2, 1)),
        iotaC=np.ascontiguousarray(iotaC),
        I128=np.eye(P, dtype=np.float32),
        I128b=np.eye(P, dtype=ml_dtypes.bfloat16),
        I64=np.eye(F2, dtype=np.float32),
        ghostrow=grow.reshape(1, TCOL),
    )
    in_maps = []
    for c in range(NC):
        xs = np.zeros((SPAD, IN_DIM), np.float32)
        xs[:SLICE] = x[c * SLICE:(c + 1) * SLICE]
        m = dict(shared)
        m["xT"] = np.ascontiguousarray(xs.T)
        m["srcoff"] = np.ascontiguousarray(srcoff[c])
        m["dstrel"] = np.ascontiguousarray(dstrel[c]).astype(ml_dtypes.bfloat16)
        in_maps.append(m)

    meta = dict(SLICE=SLICE, NW=NW, SPAD=SPAD, GHOST=GHOST,
                T_w=[int(t) for t in T_w], col0=[int(cc) for cc in col0],
                T_tot=T_tot, NC=NC)
    return in_maps, meta


def build(meta):
    SLICE, NW, SPAD, GHOST, T_tot = (meta["SLICE"], meta["NW"], meta["SPAD"],
                                     meta["GHOST"], meta["T_tot"])
    T_w, col0 = meta["T_w"], meta["col0"]
    TMAX = max(T_w)
    CH = min(512, SPAD)
    n_chunks = (SPAD + CH - 1) // CH

    nc = bacc.Bacc('TRN2', num_devices=NC)
    xT = nc.dram_tensor("xT", [IN_DIM, SPAD], F32, kind="ExternalInput")
    srcoff = nc.dram_tensor("srcoff", [P, T_tot], I32, kind="ExternalInput")
    dstrel = nc.dram_tensor("dstrel", [P, T_tot], BF16, kind="ExternalInput")
    Wcat_d = nc.dram_tensor("Wcat", [IN_DIM, TCOL], BF16, kind="ExternalInput")
    As2c_d = nc.dram_tensor("As2c", [D1, H], F32, kind="ExternalInput")
    Ad2c_d = nc.dram_tensor("Ad2c", [D1, H], F32, kind="ExternalInput")
    W2f_d = nc.dram_tensor("W2f", [F2, H * F2], F32, kind="ExternalInput")
    b1rep_d = nc.dram_tensor("b1rep", [P, D1], F32, kind="ExternalInput")
    b2col_d = nc.dram_tensor("b2col", [F2, 1], F32, kind="ExternalInput")
    iotaC_d = nc.dram_tensor("iotaC", [P, P], BF16, kind="ExternalInput")
    I128_d = nc.dram_tensor("I128", [P, P], F32, kind="ExternalInput")
    I128b_d = nc.dram_tensor("I128b", [P, P], BF16, kind="ExternalInput")
    I64_d = nc.dram_tensor("I64", [F2, F2], F32, kind="ExternalInput")
    ghostrow_d = nc.dram_tensor("ghostrow", [1, TCOL], U16, kind="ExternalInput")
    out_d = nc.dram_tensor("out", [SLICE, F2], F32, kind="ExternalOutput")
    t1loc = nc.dram_tensor("t1loc", [SPAD, TCOL], U16)
    t1full = nc.dram_tensor("t1full", [NC * SPAD + 1, TCOL], U16)
    t2loc = nc.dram_tensor("t2loc", [SPAD, TCOL], U16)
    t2full = nc.dram_tensor("t2full", [NC * SPAD + 1, TCOL], U16)

    with tile.TileContext(nc) as tc:
        with tc.tile_pool(name="consts", bufs=1) as cpool, \
             tc.tile_pool(name="sb", bufs=3) as sb, \
             tc.tile_pool(name="sb8", bufs=8) as sb8, \
             tc.tile_pool(name="gp", bufs=3) as gp, \
             tc.tile_pool(name="pp", bufs=2, space="PSUM") as pp, \
             tc.tile_pool(name="pp1", bufs=1, space="PSUM") as pp1:

            def cload(dram, shape, dtype, tag):
                t = cpool.tile(shape, dtype, tag=tag)
                nc.sync.dma_start(out=t[:], in_=dram[:, :])
                return t

            As2c = cload(As2c_d, [D1, H], F32, "cAs2c")
            Ad2c = cload(Ad2c_d, [D1, H], F32, "cAd2c")
            W2f = cload(W2f_d, [F2, H * F2], F32, "cW2f")
            b1rep = cload(b1rep_d, [P, D1], F32, "cb1")
            b2col = cload(b2col_d, [F2, 1], F32, "cb2")
            iotaC = cload(iotaC_d, [P, P], BF16, "ciota")
            I128 = cload(I128_d, [P, P], F32, "cI128")
            I128b = cload(I128b_d, [P, P], BF16, "cI128b")
            I64 = cload(I64_d, [F2, F2], F32, "cI64")

            Wc0 = cpool.tile([P, TCOL], BF16, tag="Wc0")
            Wc1 = cpool.tile([P, TCOL], BF16, tag="Wc1")
            nc.sync.dma_start(out=Wc0[:], in_=Wcat_d[0:P, :])
            nc.sync.dma_start(out=Wc1[:], in_=Wcat_d[P:2 * P, :])

            grow_sb = cpool.tile([1, TCOL], U16, tag="grow")
            nc.sync.dma_start(out=grow_sb[:], in_=ghostrow_d[:, :])
            nc.sync.dma_start(out=t1full[GHOST:GHOST + 1, :], in_=grow_sb[:])
            nc.sync.dma_start(out=t2full[GHOST:GHOST + 1, :], in_=grow_sb[:])

            ad1_sb = cpool.tile([P, NW * 16], BF16, tag="ad1sb")
            ad2_sb = cpool.tile([P, NW * 16], BF16, tag="ad2sb")

            so_sb = cpool.tile([P, T_tot], I32, tag="sosb")
            dr_sb = cpool.tile([P, T_tot], BF16, tag="drsb")
            nc.sync.dma_start(out=so_sb[:], in_=srcoff[:, :])
            nc.sync.dma_start(out=dr_sb[:], in_=dstrel[:, :])

            def split_hilo(hi_ap, lo_ap, src_f32):
                nc.vector.tensor_copy(out=hi_ap, in_=src_f32)
                nc.vector.tensor_tensor(out=lo_ap, in0=src_f32, in1=hi_ap,
                                        op=mybir.AluOpType.subtract)

            for k in range(n_chunks):
                c0, c1 = k * CH, min((k + 1) * CH, SPAD)
                cw = c1 - c0
                w0 = c0 // P
                xa = sb.tile([P, CH], F32, tag="xa")
                xb = sb.tile([P, CH], F32, tag="xb")
                nc.sync.dma_start(out=xa[:, :cw], in_=xT[0:P, c0:c1])
                nc.sync.dma_start(out=xb[:, :cw], in_=xT[P:2 * P, c0:c1])
                xab = sb.tile([P, CH], BF16, tag="xab")
                xbb = sb.tile([P, CH], BF16, tag="xbb")
                nc.vector.tensor_copy(out=xab[:, :cw], in_=xa[:, :cw])
                nc.vector.tensor_copy(out=xbb[:, :cw], in_=xb[:, :cw])
                for b in range(cw // P):
                    nn = c0 + b * P
                    w = w0 + b
                    p0ps = pp.tile([P, TCOL], F32, space="PSUM", tag="U",
                                   name="p0ps")
                    nc.tensor.matmul(out=p0ps[:], lhsT=xab[:, b * P:(b + 1) * P],
                                     rhs=Wc0[:], start=True, stop=False)
                    nc.tensor.matmul(out=p0ps[:], lhsT=xbb[:, b * P:(b + 1) * P],
                                     rhs=Wc1[:], start=False, stop=True)
                    trow = sb.tile([P, TCOL], U16, tag="trow")
                    nc.vector.tensor_copy(out=trow[:, 0:D1].bitcast(BF16),
                                          in_=p0ps[:, 0:D1])
                    nc.vector.tensor_copy(out=trow[:, D1:TCOL].bitcast(F32),
                                          in_=p0ps[:, D1:D1 + H])
                    split_hilo(ad1_sb[:, w * 16:w * 16 + 8],
                               ad1_sb[:, w * 16 + 8:w * 16 + 16],
                               p0ps[:, D1 + H:TCOL])
                    nc.sync.dma_start(out=t1loc[nn:nn + P, :], in_=trow[:])

            nc.gpsimd.collective_compute(
                "AllGather", mybir.AluOpType.bypass,
                replica_groups=[list(range(NC))],
                ins=[t1loc[:, :].opt()],
                outs=[t1full[0:NC * SPAD, :].opt()],
            )

            def edge_phase(tfull, tloc, ad_sb, layer):
                NCOLS = D1 if layer == 1 else H * F2
                FV = F1 if layer == 1 else F2
                for w in range(NW):
                    tw = T_w[w]
                    cwid = col0[w]
                    UCOLS = NCOLS + H if layer == 1 else NCOLS
                    U_ps = pp.tile([P, UCOLS], F32, space="PSUM", tag="U")
                    den_ps = None
                    if layer == 2:
                        den_ps = pp1.tile([P, H], F32, space="PSUM", tag="den",
                                          name="den_ps")
                    g_all = gp.tile([P, TMAX * TCOL], U16, tag="ga", bufs=10)
                    s_all = gp.tile([P, TMAX * P], BF16, tag="sa", bufs=4)
                    ad_all = gp.tile([P, TMAX * 16], F32, tag="ada")
                    e_all = gp.tile([P, TMAX * H], F32, tag="ea")
                    lr_all = gp.tile([P, TMAX * H], F32, tag="la")
                    p_all = gp.tile([P, TMAX * H], BF16, tag="pa")
                    for t in range(tw):
                        nc.gpsimd.indirect_dma_start(
                            out=g_all[:, t * TCOL:(t + 1) * TCOL], out_offset=None,
                            in_=tfull[:, :],
                            in_offset=bass.IndirectOffsetOnAxis(
                                ap=so_sb[:, cwid + t:cwid + t + 1], axis=0),
                        )
                    nc.vector.tensor_tensor(
                        out=view(s_all[:], 0, [[P, tw], [1, P]]),
                        in0=view(dr_sb[:], cwid, [[1, tw], [0, P]]),
                        in1=view(iotaC[:], 0, [[0, tw], [1, P]]),
                        op=mybir.AluOpType.is_equal)
                    for t in range(tw):
                        st_ps = pp1.tile([P, P], BF16, space="PSUM", tag="stp",
                                         bufs=2)
                        nc.tensor.matmul(out=st_ps[:],
                                         lhsT=s_all[:, t * P:(t + 1) * P],
                                         rhs=I128b[:], is_transpose=True,
                                         start=True, stop=True)
                        st_t = sb8.tile([P, P], BF16, tag="st")
                        nc.scalar.activation(st_t[:], st_ps[:],
                                             mybir.ActivationFunctionType.Identity)
                        ad_ps = pp1.tile([P, 16], F32, space="PSUM", tag="adps")
                        nc.tensor.matmul(out=ad_ps[:], lhsT=st_t[:],
                                         rhs=ad_sb[:, w * 16:(w + 1) * 16],
                                         start=True, stop=True)
                        nc.scalar.activation(ad_all[:, t * 16:(t + 1) * 16],
                                             ad_ps[:],
                                             mybir.ActivationFunctionType.Identity)
                    gf = g_all[:].bitcast(F32)
                    nc.vector.tensor_tensor(
                        out=view(e_all[:], 0, [[H, tw], [1, H]]),
                        in0=view(gf, 32, [[40, tw], [1, H]]),
                        in1=view(ad_all[:], 0, [[16, tw], [1, H]]),
                        op=mybir.AluOpType.add)
                    nc.vector.tensor_tensor(
                        out=view(e_all[:], 0, [[H, tw], [1, H]]),
                        in0=view(e_all[:], 0, [[H, tw], [1, H]]),
                        in1=view(ad_all[:], 8, [[16, tw], [1, H]]),
                        op=mybir.AluOpType.add)
                    nc.vector.tensor_scalar_mul(out=lr_all[:, :tw * H],
                                                in0=e_all[:, :tw * H], scalar1=NEG)
                    nc.vector.tensor_tensor(out=lr_all[:, :tw * H],
                                            in0=lr_all[:, :tw * H],
                                            in1=e_all[:, :tw * H],
                                            op=mybir.AluOpType.max)
                    nc.scalar.activation(p_all[:, :tw * H], lr_all[:, :tw * H],
                                         mybir.ActivationFunctionType.Exp)
                    for t in range(tw):
                        w_t = sb8.tile([P, UCOLS], BF16, tag="wv")
                        gh = g_all[:, t * TCOL:t * TCOL + D1].bitcast(BF16)
                        if layer == 1:
                            in0 = gh.rearrange("p (h f) -> p h f", h=H)
                        else:
                            in0 = mid_bcast(gh, H)
                        nc.vector.tensor_tensor(
                            out=w_t[:, :H * FV].rearrange("p (h f) -> p h f", h=H),
                            in0=in0,
                            in1=p_all[:, t * H:(t + 1) * H].to_broadcast([P, H, FV]),
                            op=mybir.AluOpType.mult)
                        if layer == 1:
                            nc.scalar.activation(
                                w_t[:, H * FV:UCOLS],
                                p_all[:, t * H:(t + 1) * H],
                                mybir.ActivationFunctionType.Identity)
                        nc.tensor.matmul(out=U_ps[:],
                                         lhsT=s_all[:, t * P:(t + 1) * P],
                                         rhs=w_t[:], start=(t == 0), stop=False)
                        if layer == 2:
                            nc.tensor.matmul(out=den_ps[:],
                                             lhsT=s_all[:, t * P:(t + 1) * P],
                                             rhs=p_all[:, t * H:(t + 1) * H],
                                             start=(t == 0), stop=False)
                    own = gp.tile([P, TCOL], U16, tag="own")
                    nc.sync.dma_start(out=own[:], in_=tloc[w * P:(w + 1) * P, :])
                    es = gp.tile([P, H], F32, tag="es")
                    nc.vector.tensor_tensor(out=es[:],
                                            in0=own[:, D1:TCOL].bitcast(F32),
                                            in1=ad_sb[:, w * 16:w * 16 + 8],
                                            op=mybir.AluOpType.add)
                    nc.vector.tensor_tensor(out=es[:], in0=es[:],
                                            in1=ad_sb[:, w * 16 + 8:w * 16 + 16],
                                            op=mybir.AluOpType.add)
                    lrs = gp.tile([P, H], F32, tag="lrs")
                    nc.vector.tensor_scalar_mul(out=lrs[:], in0=es[:], scalar1=NEG)
                    nc.vector.tensor_tensor(out=lrs[:], in0=lrs[:], in1=es[:],
                                            op=mybir.AluOpType.max)
                    ps_b = gp.tile([P, H], BF16, tag="psb")
                    nc.scalar.activation(ps_b[:], lrs[:],
                                         mybir.ActivationFunctionType.Exp)
                    ws = sb8.tile([P, UCOLS], BF16, tag="wv", name="ws")
                    gh0 = own[:, 0:D1].bitcast(BF16)
                    if layer == 1:
                        in0s = gh0.rearrange("p (h f) -> p h f", h=H)
                    else:
                        in0s = mid_bcast(gh0, H)
                    nc.vector.tensor_tensor(
                        out=ws[:, :H * FV].rearrange("p (h f) -> p h f", h=H),
                        in0=in0s, in1=ps_b[:].to_broadcast([P, H, FV]),
                        op=mybir.AluOpType.mult)
                    if layer == 1:
                        nc.scalar.activation(ws[:, H * FV:UCOLS], ps_b[:],
                                             mybir.ActivationFunctionType.Identity)
                    nc.tensor.matmul(out=U_ps[:], lhsT=I128b[:], rhs=ws[:],
                                     start=False, stop=True)
                    if layer == 2:
                        nc.tensor.matmul(out=den_ps[:], lhsT=I128b[:], rhs=ps_b[:],
                                         start=False, stop=True)
                    den_src = (U_ps[:, H * FV:UCOLS] if layer == 1 else den_ps[:])
                    dse = sb.tile([P, H], F32, tag="dse")
                    nc.vector.tensor_scalar_add(out=dse[:], in0=den_src, scalar1=1e-30)
                    rd = sb.tile([P, H], F32, tag="rd")
                    nc.vector.reciprocal(out=rd[:], in_=dse[:])
                    if layer == 1:
                        h2a = sb.tile([P, D1], F32, tag="h2a")
                        nc.vector.tensor_tensor(
                            out=h2a[:].rearrange("p (h f) -> p h f", h=H),
                            in0=U_ps[:, 0:D1].rearrange("p (h f) -> p h f", h=H),
                            in1=rd[:].to_broadcast([P, H, F1]),
                            op=mybir.AluOpType.mult)
                        nc.vector.tensor_tensor(out=h2a[:], in0=h2a[:], in1=b1rep[:],
                                                op=mybir.AluOpType.add)
                        ex = sb.tile([P, D1], F32, tag="ex")
                        nc.scalar.activation(ex[:], h2a[:],
                                             mybir.ActivationFunctionType.Exp)
                        exm = sb.tile([P, D1], F32, tag="exm")
                        nc.vector.tensor_scalar(out=exm[:], in0=ex[:], scalar1=1.0,
                                                scalar2=-1.0, op0=mybir.AluOpType.min,
                                                op1=mybir.AluOpType.add)
                        rl = sb.tile([P, D1], F32, tag="rl")
                        nc.vector.tensor_scalar_max(out=rl[:], in0=h2a[:], scalar1=0.0)
                        h2e = sb.tile([P, D1], F32, tag="h2e")
                        nc.vector.tensor_tensor(out=h2e[:], in0=exm[:], in1=rl[:],
                                                op=mybir.AluOpType.add)
                        trow2 = sb.tile([P, TCOL], U16, tag="h2eb")
                        nc.vector.tensor_copy(out=trow2[:, 0:D1].bitcast(BF16),
                                              in_=h2e[:])
                        hT_ps = pp1.tile([D1, P], F32, space="PSUM", tag="t1")
                        nc.tensor.matmul(out=hT_ps[:], lhsT=h2e[:], rhs=I128[:],
                                         is_transpose=True, start=True, stop=True)
                        hT = sb.tile([D1, P], F32, tag="hT")
                        nc.vector.tensor_copy(out=hT[:], in_=hT_ps[:])
                        a2_ps = pp1.tile([P, H], F32, space="PSUM", tag="t2")
                        nc.tensor.matmul(out=a2_ps[:], lhsT=hT[:], rhs=As2c[:],
                                         start=True, stop=True)
                        nc.vector.tensor_copy(out=trow2[:, D1:TCOL].bitcast(F32),
                                              in_=a2_ps[:])
                        d2_ps = pp1.tile([P, H], F32, space="PSUM", tag="t2",
                                         name="d2_ps")
                        nc.tensor.matmul(out=d2_ps[:], lhsT=hT[:], rhs=Ad2c[:],
                                         start=True, stop=True)
                        split_hilo(ad2_sb[:, w * 16:w * 16 + 8],
                                   ad2_sb[:, w * 16 + 8:w * 16 + 16], d2_ps[:])
                        nc.sync.dma_start(out=t2loc[w * P:(w + 1) * P, :],
                                          in_=trow2[:])
                    else:
                        U2n = sb.tile([P, H * F2], F32, tag="U2n")
                        nc.vector.tensor_tensor(
                            out=U2n[:].rearrange("p (h f) -> p h f", h=H),
                            in0=U_ps[:].rearrange("p (h f) -> p h f", h=H),
                            in1=rd[:].to_broadcast([P, H, F2]),
                            op=mybir.AluOpType.mult)
                        YT_ps = pp1.tile([F2, P], F32, space="PSUM", tag="t2",
                                         name="YT_ps")
                        for h in range(H):
                            uT_ps = pp1.tile([F2, P], F32, space="PSUM", tag="t1")
                            nc.tensor.matmul(out=uT_ps[:],
                                             lhsT=U2n[:, h * F2:(h + 1) * F2],
                                             rhs=I128[:], is_transpose=True,
                                             start=True, stop=True)
                            uT = sb.tile([F2, P], F32, tag="uTs")
                            nc.vector.tensor_copy(out=uT[:], in_=uT_ps[:])
                            nc.tensor.matmul(out=YT_ps[:],
                                             lhsT=W2f[:, h * F2:(h + 1) * F2],
                                             rhs=uT[:], start=(h == 0),
                                             stop=(h == H - 1))
                        Y = sb.tile([F2, P], F32, tag="Y")
                        nc.scalar.activation(Y[:], YT_ps[:],
                                             mybir.ActivationFunctionType.Identity,
                                             bias=b2col[:], scale=1.0 / H)
                        o_ps = pp1.tile([P, F2], F32, space="PSUM", tag="t1",
                                        name="o_ps")
                        nc.tensor.matmul(out=o_ps[:], lhsT=Y[:], rhs=I64[:],
                                         is_transpose=True, start=True, stop=True)
                        ow = sb.tile([P, F2], F32, tag="ow")
                        nc.vector.tensor_copy(out=ow[:], in_=o_ps[:])
                        rows = min(P, SLICE - w * P)
                        nc.sync.dma_start(out=out_d[w * P:w * P + rows, :],
                                          in_=ow[:rows, :])

            edge_phase(t1full, t1loc, ad1_sb, 1)
            nc.gpsimd.collective_compute(
                "AllGather", mybir.AluOpType.bypass,
                replica_groups=[list(range(NC))],
                ins=[t2loc[:, :].opt()],
                outs=[t2full[0:NC * SPAD, :].opt()],
            )
            edge_phase(t2full, t2loc, ad2_sb, 2)

    nc.compile()
    return nc


def kernel(**inputs):
    global LAST_EXEC_NS
    _register_profile_hook()
    from concourse import bass_utils

    in_maps, meta = host_prep(inputs)
    nc = build(meta)
    trace = os.environ.get("GAT_TRACE", "1") == "1"
    try:
        res = bass_utils.run_bass_kernel_spmd(
            nc, in_maps, core_ids=list(range(NC)), trace=trace)
    except Exception:
        if not trace:
            raise
        res = bass_utils.run_bass_kernel_spmd(
            nc, in_maps, core_ids=list(range(NC)), trace=False)
    LAST_EXEC_NS = res.exec_time_ns
    SLICE = meta["SLICE"]
    out = np.empty((N, F2), np.float32)
    for c in range(NC):
        out[c * SLICE:(c + 1) * SLICE] = res.results[c]["out"]
    return out

